# revision 19
# baseline (speedup 1.0000x reference)
"""BiLSTM dual-pathway + CRF NLL kernel for 8 Trainium2 NeuronCores.

Sharding: direction-parallel for the LSTM recurrences, data-parallel for the
classifier/CRF. Phase 1 runs the four layer-0 directions (char fwd/bwd, word
fwd/bwd) on 8 cores as (direction x batch-half), batch 32 per core, so the
recurrent matmuls run at N=32 instead of N=8 and the sequential chain count
drops from 4096 to 1024 steps. A pairwise AllGather exchanges the L0 hidden
states between fwd/bwd cores, phase 2 runs the four layer-1 directions the
same way, then an 8-rank AllToAll redistributes hidden states to a
data-parallel layout (8 sequences per core) for the classifier and CRF.

SPMD uniformity: every core runs the identical program. Backward directions
receive host-time-reversed inputs; reversed reads of peer hidden states are
fixed negative-stride APs, with host-permuted weight columns absorbing the
f/b role differences. Peer-region selection after the AllGather uses per-core
0/1 blend masks delivered as input data.

The CRF forward scan runs in probability space with a constant per-step
prescale alpha folded into the emission exponentials (corrected analytically
on the host), so each step is one resident-weight 15x15 matmul plus one
vector multiply -- no per-step exp/ln activation-table swaps.
"""

import sys

sys.path.insert(0, "/opt/trn_rl_repo")

import numpy as np

import concourse.bass as bass
import concourse.mybir as mybir
from concourse import bacc
from concourse.bass import ds
from concourse.tile import TileContext
from concourse.bass_utils import run_bass_kernel_spmd

F16 = mybir.dt.float16
F32 = mybir.dt.float32
AF = mybir.ActivationFunctionType

B, T, V, K = 64, 512, 40, 15
NC_N = 8
BL2 = 32            # batch per core in phases 1-2
BL3 = 8             # sequences per core in phase 3
TB3 = T * BL3       # 4096 classifier/CRF columns per core
DK1, DK2 = 6, 4     # input chunks for L0 (word=768, char padded) and L1 (512)
UNROLL = 16
HCOL = T + 1        # h buffer columns per sequence (col 0 = zero init)

_BUILD_CACHE = {}


def _emit_rec_block(nc, stagep, ps_rec, hc, cst, whh, xg_dram, tv, fodder):
    """Emit 16 recurrence steps for time block tv.

    fodder: list of (items, per_step) - closures emitting one PE-side quantum
    each (GEMM matmuls for other pipeline stages), dispensed between the
    step's recurrent matmuls and its vector/scalar chain so they execute in
    the PE's dependency-stall gaps.
    """
    CB = UNROLL * BL2
    xgs = stagep.tile([128, 8, CB], F16, tag="xgs")
    nc.sync.dma_start(out=xgs, in_=xg_dram[:, :, ds(tv * BL2, CB)])
    cursors = [[items, 0, per_step] for items, per_step in fodder]
    for j in range(UNROLL):
        psA = ps_rec.tile([128, 4, BL2], F32, tag="recpsA")
        psB = ps_rec.tile([128, 4, BL2], F32, tag="recpsB")
        # i,f gates (chunks 0-3) first into psA so the first g-add can
        # start while the o/g~ matmuls (psB) still run
        for m in (0, 1, 2, 3):
            for k in range(2):
                nc.tensor.matmul(psA[:, m], whh[:, k, m],
                                 hc[:, k, j], start=(k == 0), stop=(k == 1))
        for m in (6, 7, 4, 5):
            for k in range(2):
                nc.tensor.matmul(psB[:, m - 4], whh[:, k, m],
                                 hc[:, k, j], start=(k == 0), stop=(k == 1))
        for cur in cursors:
            for _ in range(cur[2]):
                if cur[1] < len(cur[0]):
                    cur[0][cur[1]]()
                    cur[1] += 1
        g = stagep.tile([128, 8, BL2], F32, tag="g")
        xsl = xgs[:, :, ds(j * BL2, BL2)]
        nc.vector.tensor_add(g[:, 0:4], psA, xsl[:, 0:4])
        sig = stagep.tile([128, 6, BL2], F32, tag="sig")
        nc.scalar.activation(sig[:, 0:4], g[:, 0:4], AF.Sigmoid)
        nc.vector.tensor_mul(cst, cst, sig[:, 2:4])
        nc.vector.tensor_add(g[:, 4:8], psB, xsl[:, 4:8])
        tgg = stagep.tile([128, 2, BL2], F32, tag="tgg")
        nc.scalar.activation(tgg, g[:, 6:8], AF.Tanh)
        tmp = stagep.tile([128, 2, BL2], F32, tag="tmpig")
        nc.vector.tensor_mul(tmp, sig[:, 0:2], tgg)
        nc.scalar.activation(sig[:, 4:6], g[:, 4:6], AF.Sigmoid)
        nc.vector.tensor_add(cst, cst, tmp)
        tch = stagep.tile([128, 2, BL2], F32, tag="tch")
        nc.scalar.activation(tch, cst, AF.Tanh)
        nc.vector.tensor_mul(hc[:, :, j + 1], sig[:, 4:6], tch)
    for cur in cursors:
        while cur[1] < len(cur[0]):
            cur[0][cur[1]]()
            cur[1] += 1


def _build_nc():
    if "nc" in _BUILD_CACHE:
        return _BUILD_CACHE["nc"]
    nc = bacc.Bacc(target_bir_lowering=False, num_devices=NC_N)

    # ---- external parameters -------------------------------------------------
    x1_ext = nc.declare_dram_parameter("x1", [128, DK1, T * BL2], F16, isOutput=False)
    wih1_ext = nc.declare_dram_parameter("wih1", [128, DK1 * 8 * 128], F16, isOutput=False)
    whh1_ext = nc.declare_dram_parameter("whh1", [128, 2 * 8 * 128], F16, isOutput=False)
    bias1_ext = nc.declare_dram_parameter("bias1", [128, 8], F32, isOutput=False)
    wih2_ext = nc.declare_dram_parameter("wih2", [128, DK2 * 8 * 128], F16, isOutput=False)
    whh2_ext = nc.declare_dram_parameter("whh2", [128, 2 * 8 * 128], F16, isOutput=False)
    bias2_ext = nc.declare_dram_parameter("bias2", [128, 8], F32, isOutput=False)
    masks_ext = nc.declare_dram_parameter("masks", [128, 2], F32, isOutput=False)
    cls1_ext = nc.declare_dram_parameter("cls1", [128, 8 * 4 * 128], F16, isOutput=False)
    clsb1_ext = nc.declare_dram_parameter("clsb1", [128, 4], F32, isOutput=False)
    cls2_ext = nc.declare_dram_parameter("cls2", [128, 4 * 15], F16, isOutput=False)
    clsb2_ext = nc.declare_dram_parameter("clsb2", [15, 1], F32, isOutput=False)
    mexp_ext = nc.declare_dram_parameter("mexp", [15, 15], F32, isOutput=False)
    transn_ext = nc.declare_dram_parameter("transn", [15, 15], F16, isOutput=False)
    start_ext = nc.declare_dram_parameter("crfstart", [15, 1], F32, isOutput=False)
    end_ext = nc.declare_dram_parameter("crfend", [15, 1], F32, isOutput=False)
    lna_ext = nc.declare_dram_parameter("lnalpha", [15, 1], F32, isOutput=False)
    tago_ext = nc.declare_dram_parameter("tagoneT", [15, TB3], F16, isOutput=False)
    out_ext = nc.declare_dram_parameter("out", [1, 1], F32, isOutput=True)

    # ---- internal DRAM -------------------------------------------------------
    HSZ = 2 * BL2 * T  # 32768 cols/partition of h (f16)
    xg1_dram = nc.dram_tensor("xg1", [128, 8, T * BL2], F16)
    xg2_dram = nc.dram_tensor("xg2", [128, 8, T * BL2], F16)
    hmine = nc.dram_tensor("hmine", [128, HSZ], F16)
    agout = nc.dram_tensor("agout", [2, 128, HSZ], F16)
    sendb = nc.dram_tensor("sendb", [8, 128, 2 * 4 * T], F16)
    recvb = nc.dram_tensor("recvb", [8, 128, 2 * 4 * T], F16)

    with TileContext(nc) as tc:
        with (
            tc.tile_pool(name="consts", bufs=1) as consts,
            tc.tile_pool(name="seqs", bufs=1) as seqs,
            tc.tile_pool(name="work", bufs=2) as work,
            tc.tile_pool(name="stage", bufs=2) as stagep,
            tc.tile_pool(name="ps_big", bufs=3, space="PSUM") as ps_big,
            tc.tile_pool(name="ps_rec", bufs=1, space="PSUM") as ps_rec,
            tc.tile_pool(name="ps_small", bufs=3, space="PSUM") as ps_small,
        ):
            # h buffer, seq-major: [128, chunk2, b32, T+1], reused by phases 1+2
            h_sb = seqs.tile([128, 2, BL2, HCOL], F16, tag="h_sb")
            hc = seqs.tile([128, 2, UNROLL + 1, BL2], F16, tag="hcomp")
            cst = seqs.tile([128, 2, BL2], F32, tag="cstate")
            masks = consts.tile([128, 2], F32, tag="masks")
            nc.sync.dma_start(out=masks, in_=masks_ext[:, :])

            # ================= PHASE 1 (L0) =================
            wih1 = consts.tile([128, DK1, 8, 128], F16, tag="wbig")
            nc.sync.dma_start(
                out=wih1,
                in_=wih1_ext.ap().rearrange("p (k m c) -> p k m c", k=DK1, m=8))
            whh1 = consts.tile([128, 2, 8, 128], F16, tag="whh")
            nc.sync.dma_start(
                out=whh1,
                in_=whh1_ext.ap().rearrange("p (k m c) -> p k m c", k=2, m=8))
            bias1 = consts.tile([128, 8], F32, tag="bias1")
            nc.sync.dma_start(out=bias1, in_=bias1_ext[:, :])
            # wih2/bias2 load now: the L1 input GEMM's slotA half runs
            # interleaved inside the phase-1 recurrence
            wih2 = consts.tile([128, DK2, 8, 128], F16, tag="wih2")
            nc.sync.dma_start(
                out=wih2,
                in_=wih2_ext.ap().rearrange("p (k m c) -> p k m c", k=DK2, m=8))
            bias2 = consts.tile([128, 8], F32, tag="bias2")
            nc.sync.dma_start(out=bias2, in_=bias2_ext[:, :])

            CB = UNROLL * BL2
            # phase-1 h layout in DRAM is t-major: (p, c, t, b)
            hm_view = hmine.ap().rearrange("p (c t b) -> p c t b", c=2, t=T)

            def g1_items(tv_lead):
                # xg1 block tv_lead: input DMA now, 48 matmul quanta
                xb = stagep.tile([128, DK1, CB], F16, tag="xb1")
                nc.sync.dma_start(out=xb, in_=x1_ext[:, :, ds(tv_lead * BL2, CB)])
                items, pss = [], {}
                for m in range(8):
                    for k in range(DK1):
                        def it(m=m, k=k):
                            if k == 0:
                                pss[m] = ps_big.tile([128, CB], F32, tag="gemmps",
                                                     name=f"g1ps{m}")
                            nc.tensor.matmul(pss[m], wih1[:, k, m], xb[:, k],
                                             start=(k == 0), stop=(k == DK1 - 1))
                            if k == DK1 - 1:
                                st = stagep.tile([128, CB], F16, tag="xgstage",
                                                 name=f"g1st{m}")
                                nc.scalar.activation(st, pss[m], AF.Identity,
                                                     bias=bias1[:, m: m + 1])
                                nc.gpsimd.dma_start(
                                    out=xg1_dram[:, m, ds(tv_lead * BL2, CB)], in_=st)
                        items.append(it)
                return items

            def p1_end(tv):
                nc.gpsimd.dma_start(out=hm_view[:, :, ds(tv, UNROLL)],
                                    in_=hc[:, :, 1: UNROLL + 1])
                nc.gpsimd.tensor_copy(hc[:, :, 0], hc[:, :, UNROLL])

            def rec1_block(tv, fodder):
                _emit_rec_block(nc, stagep, ps_rec, hc, cst, whh1, xg1_dram,
                                tv, fodder)
                p1_end(tv)

            nc.vector.memset(cst, 0.0)
            nc.vector.memset(hc[:, :, 0], 0.0)
            for blk in range(2):
                for it in g1_items(16 * blk):
                    it()
            rec1_block(0, [(g1_items(32), 3)])
            rec1_block(16, [(g1_items(48), 3)])
            with tc.For_i(32, 480, 16) as tv:
                items1 = g1_items(tv + 32)
                _emit_rec_block(nc, stagep, ps_rec, hc, cst, whh1, xg1_dram,
                                tv, [(items1, 3)])
                p1_end(tv)
            rec1_block(480, [])
            rec1_block(496, [])

            nc.gpsimd.collective_compute(
                "AllGather", mybir.AluOpType.bypass,
                replica_groups=[[0, 2], [1, 3], [4, 6], [5, 7]],
                ins=[hmine.ap()], outs=[agout.ap()],
            )

            # ================= PHASE 2 (L1) =================
            whh2 = consts.tile([128, 2, 8, 128], F16, tag="whh")
            nc.sync.dma_start(
                out=whh2,
                in_=whh2_ext.ap().rearrange("p (k m c) -> p k m c", k=2, m=8))

            def slot_ap(tensor_offset, tensor, c, tv, reverse):
                # [128, UNROLL*32] chunk c of a t-major h DRAM region
                # (p, c2, T, b32): a contiguous ascending t window; reversed
                # slots flip t on the SBUF side.
                if not reverse:
                    w = tv
                else:
                    w = (T - UNROLL) - tv
                off = tensor_offset + c * (T * BL2) + w * BL2
                return bass.AP(tensor=tensor, offset=off,
                               ap=[[HSZ, 128], [1, UNROLL * BL2]])

            def g2_items(tv_lead):
                # full xg2 block: slotA = own h fwd, slotB = peer h reversed
                sa = stagep.tile([128, 2, UNROLL, BL2], F16, tag="slotA")
                r0 = stagep.tile([128, 2, UNROLL, BL2], F16, tag="slotR0")
                r1 = stagep.tile([128, 2, UNROLL, BL2], F16, tag="slotR1")
                for c in range(2):
                    nc.sync.dma_start(out=sa[:, c],
                                      in_=slot_ap(0, hmine, c, tv_lead, False))
                    nc.sync.dma_start(out=r0[:, c],
                                      in_=slot_ap(0, agout, c, tv_lead, True))
                    nc.sync.dma_start(out=r1[:, c],
                                      in_=slot_ap(128 * HSZ, agout, c, tv_lead, True))
                sbr = stagep.tile([128, 2, UNROLL, BL2], F16, tag="slotBr")
                items, pss = [], {}

                def blend0():
                    nc.vector.tensor_scalar_mul(r0, r0, masks[:, 0:1])

                def blend1():
                    nc.vector.tensor_scalar_mul(r1, r1, masks[:, 1:2])

                def blend2():
                    # add, writing with the t window reversed
                    p_step = sbr.ap[0][0]
                    rev = bass.AP(tensor=sbr.tensor,
                                  offset=sbr.offset + (UNROLL - 1) * BL2,
                                  ap=[[p_step, 128], [UNROLL * BL2, 2],
                                      [-BL2, UNROLL], [1, BL2]])
                    nc.vector.tensor_add(rev, r0, r1)

                items += [blend0, blend1, blend2]

                def src_k(k):
                    t = sa if k < 2 else sbr
                    return t[:, k % 2].rearrange("p t b -> p (t b)")

                for m in range(8):
                    for k in range(DK2):
                        def it(m=m, k=k):
                            if k == 0:
                                pss[m] = ps_big.tile([128, CB], F32, tag="gemmps",
                                                     name=f"g2ps{m}")
                            nc.tensor.matmul(pss[m], wih2[:, k, m], src_k(k),
                                             start=(k == 0), stop=(k == DK2 - 1))
                            if k == DK2 - 1:
                                st = stagep.tile([128, CB], F16, tag="xgstage",
                                                 name=f"g2st{m}")
                                nc.scalar.activation(st, pss[m], AF.Identity,
                                                     bias=bias2[:, m: m + 1])
                                nc.gpsimd.dma_start(
                                    out=xg2_dram[:, m, ds(tv_lead * BL2, CB)], in_=st)
                        items.append(it)
                return items

            def p2_end(tv):
                nc.gpsimd.tensor_copy(
                    h_sb[:, :, :, ds(tv + 1, UNROLL)],
                    hc[:, :, 1: UNROLL + 1].rearrange("p c t b -> p c b t"))
                nc.gpsimd.tensor_copy(hc[:, :, 0], hc[:, :, UNROLL])

            nc.vector.memset(cst, 0.0)
            nc.vector.memset(hc[:, :, 0], 0.0)
            for blk in (0, 1):
                for it in g2_items(16 * blk):
                    it()
            with tc.For_i(0, 480, 16) as tv:
                items = g2_items(tv + 32)
                _emit_rec_block(nc, stagep, ps_rec, hc, cst, whh2, xg2_dram,
                                tv, [(items, 3)])
                p2_end(tv)
            for blk in (30, 31):
                _emit_rec_block(nc, stagep, ps_rec, hc, cst, whh2, xg2_dram,
                                16 * blk, [])
                p2_end(16 * blk)

            # ---- AllToAll redistribution to data-parallel layout ----
            sb_view = sendb.ap().rearrange("j p (c b t) -> j p c b t", c=2, b=4)
            for j in range(8):
                for c in range(2):
                    nc.sync.dma_start(out=sb_view[j, :, c],
                                      in_=h_sb[:, c, ds(4 * j, 4), 1:HCOL])
            nc.gpsimd.collective_compute(
                "AllToAll", mybir.AluOpType.bypass,
                replica_groups=[list(range(8))],
                ins=[sendb.ap()], outs=[recvb.ap()],
            )

            # ================= PHASE 3: classifier + CRF =================
            cls1 = consts.tile([128, 8, 4, 128], F16, tag="wbig")
            nc.sync.dma_start(
                out=cls1,
                in_=cls1_ext.ap().rearrange("p (k m c) -> p k m c", k=8, m=4))
            clsb1 = consts.tile([128, 4], F32, tag="clsb1")
            nc.sync.dma_start(out=clsb1, in_=clsb1_ext[:, :])
            cls2 = consts.tile([128, 4, 15], F16, tag="cls2")
            nc.sync.dma_start(
                out=cls2, in_=cls2_ext.ap().rearrange("p (k j) -> p k j", k=4))
            clsb2 = consts.tile([15, 1], F32, tag="clsb2")
            nc.sync.dma_start(out=clsb2, in_=clsb2_ext[:, :])
            mexp = consts.tile([15, 15], F32, tag="mexp")
            nc.sync.dma_start(out=mexp, in_=mexp_ext[:, :])
            transn = consts.tile([15, 15], F16, tag="transn")
            nc.sync.dma_start(out=transn, in_=transn_ext[:, :])
            crfstart = consts.tile([15, 1], F32, tag="crfstart")
            nc.sync.dma_start(out=crfstart, in_=start_ext[:, :])
            crfend = consts.tile([15, 1], F32, tag="crfend")
            nc.sync.dma_start(out=crfend, in_=end_ext[:, :])
            lnalpha = consts.tile([15, 1], F32, tag="lnalpha")
            nc.sync.dma_start(out=lnalpha, in_=lna_ext[:, :])
            tago = consts.tile([15, TB3], F16, tag="tago")
            nc.sync.dma_start(out=tago, in_=tago_ext[:, :])

            logits = seqs.tile([15, TB3], F32, tag="logits")

            NT = 64  # t-steps per classifier n-tile (NT*BL3 = 512 cols)
            SHSZ = 128 * 2 * 4 * T  # elements per recv shard

            def comb_ap(kk, half, ns):
                # [128, 4, NT]: dir kk//2, chunk kk%2, half-shard, n-tile ns;
                # always an ascending t window (reversal done in the rhs view)
                d, c = kk // 2, kk % 2
                rev = d in (1, 3)  # c1b, w1b stored time-reversed
                base = (2 * d + half) * SHSZ + c * (4 * T)
                if not rev:
                    off = base + ns * NT
                else:
                    off = base + (T - NT) - ns * NT
                return bass.AP(tensor=recvb, offset=off,
                               ap=[[2 * 4 * T, 128], [T, 4], [1, NT]])

            for ns in range(8):
                comb = stagep.tile([128, 8, BL3, NT], F16, tag="comb", bufs=2)
                for kk in range(8):
                    for half in range(2):
                        nc.sync.dma_start(out=comb[:, kk, ds(4 * half, 4)],
                                          in_=comb_ap(kk, half, ns))
                hmt = []
                for m in range(4):
                    ps = ps_big.tile([128, NT * BL3], F32, tag="gemmps")
                    for kk in range(8):
                        if kk // 2 in (1, 3):
                            p_step = comb.ap[0][0]
                            off = (comb.offset + kk * BL3 * NT + (NT - 1))
                            rhs = bass.AP(tensor=comb.tensor, offset=off,
                                          ap=[[p_step, 128], [-1, NT],
                                              [NT, BL3]])
                        else:
                            rhs = comb[:, kk].rearrange("p b t -> p t b")
                        nc.tensor.matmul(ps, cls1[:, kk, m], rhs,
                                         start=(kk == 0), stop=(kk == 7))
                    hm = stagep.tile([128, NT * BL3], F16, tag="hm", bufs=4,
                                     name=f"hm{m}")
                    nc.scalar.activation(hm, ps, AF.Relu, bias=clsb1[:, m: m + 1])
                    hmt.append(hm)
                ps2 = ps_small.tile([15, NT * BL3], F32, tag="small")
                for m in range(4):
                    nc.tensor.matmul(ps2, cls2[:, m], hmt[m],
                                     start=(m == 0), stop=(m == 3))
                nc.vector.tensor_scalar_add(
                    logits[:, ds(ns * NT * BL3, NT * BL3)], ps2, clsb2)

            # fold CRF start/end into first/last emission columns
            nc.vector.tensor_scalar_add(logits[:, 0:BL3], logits[:, 0:BL3], crfstart)
            nc.vector.tensor_scalar_add(logits[:, TB3 - BL3: TB3],
                                        logits[:, TB3 - BL3: TB3], crfend)

            # ---- CRF numerator ----
            racc = work.tile([15, 16], F32, tag="racc")
            nc.vector.memset(racc, 0.0)
            for ns in range(8):
                pre = stagep.tile([15, 512], F32, tag="prodns")
                nc.vector.tensor_mul(pre, logits[:, ds(ns * 512, 512)],
                                     tago[:, ds(ns * 512, 512)])
                nc.vector.tensor_reduce(racc[:, 8 + ns: 9 + ns], pre,
                                        axis=mybir.AxisListType.X,
                                        op=mybir.AluOpType.add)
                psv = ps_small.tile([15, 512], F32, tag="small")
                nc.tensor.matmul(psv, transn, tago[:, ds(ns * 512, 512)],
                                 start=True, stop=True)
                w = 512 if ns < 7 else 512 - BL3
                pr = stagep.tile([15, 512], F32, tag="prodns")
                nc.vector.tensor_mul(pr[:, :w], psv[:, :w],
                                     tago[:, ds(ns * 512 + BL3, w)])
                nc.vector.tensor_reduce(racc[:, ns: ns + 1], pr[:, :w],
                                        axis=mybir.AxisListType.X,
                                        op=mybir.AluOpType.add)
            nv = stagep.tile([15, 1], F32, tag="nv")
            nc.vector.tensor_reduce(nv, racc, axis=mybir.AxisListType.X,
                                    op=mybir.AluOpType.add)
            ones15 = consts.tile([15, 1], F32, tag="ones15")
            nc.vector.memset(ones15, 1.0)
            psn = ps_small.tile([1, 1], F32, tag="small")
            nc.tensor.matmul(psn, ones15, nv, start=True, stop=True)
            num11 = work.tile([1, 1], F32, tag="num11")
            nc.vector.tensor_copy(num11, psn)

            # ---- CRF forward scan, probability space ----
            # two independent half-batch chains so the 15x15 matmul of one
            # overlaps the vector multiply of the other. p0 first, then
            # E = alpha*exp(logits) in place (last col already has e^end).
            pA = seqs.tile([15, 4], F32, tag="pvecA")
            pB = seqs.tile([15, 4], F32, tag="pvecB")
            nc.scalar.activation(pA, logits[:, 0:4], AF.Exp)
            nc.scalar.activation(pB, logits[:, 4:8], AF.Exp)
            E = logits
            nc.scalar.activation(E, logits, AF.Exp, bias=lnalpha)

            def crf_step(col):
                zA = ps_small.tile([15, 4], F32, tag="small", name="zA")
                nc.tensor.matmul(zA, mexp, pA, start=True, stop=True)
                zB = ps_small.tile([15, 4], F32, tag="small", name="zB")
                nc.tensor.matmul(zB, mexp, pB, start=True, stop=True)
                nc.vector.tensor_mul(pA, zA, E[:, col: col + 4] if isinstance(col, int)
                                     else E[:, ds(col, 4)])
                nc.vector.tensor_mul(pB, zB, E[:, col + 4: col + 8] if isinstance(col, int)
                                     else E[:, ds(col + 4, 4)])

            for t in range(1, 16):
                crf_step(t * BL3)
            with tc.For_i(0, 496, UNROLL) as tv:
                for j in range(UNROLL):
                    crf_step((16 + j) * BL3 + tv * BL3)

            # ---- denominator + output ----
            den11 = work.tile([1, 1], F32, tag="den11")
            for idx, pv in enumerate((pA, pB)):
                psd = ps_small.tile([1, 4], F32, tag="small", name=f"psd{idx}")
                nc.tensor.matmul(psd, ones15, pv, start=True, stop=True)
                ln4 = stagep.tile([1, 4], F32, tag="ln8", name=f"ln4{idx}")
                nc.scalar.activation(ln4, psd, AF.Ln)
                if idx == 0:
                    nc.vector.tensor_reduce(den11, ln4, axis=mybir.AxisListType.X,
                                            op=mybir.AluOpType.add)
                else:
                    dh = work.tile([1, 1], F32, tag="denh")
                    nc.vector.tensor_reduce(dh, ln4, axis=mybir.AxisListType.X,
                                            op=mybir.AluOpType.add)
                    nc.vector.tensor_add(den11, den11, dh)
            res = work.tile([1, 1], F32, tag="res")
            nc.vector.tensor_sub(res, den11, num11)
            nc.sync.dma_start(out=out_ext[:, :], in_=res)

    nc.finalize()
    _BUILD_CACHE["nc"] = nc
    return nc


# ---- host-side input prep ---------------------------------------------------

# gate perm [i(256), f(256), g(256), o(256)] -> [i, f, o, g~]
_GPERM = np.concatenate([np.arange(0, 512), np.arange(768, 1024), np.arange(512, 768)])

# core c -> (pathway, direction, half): 0..3 char f/f/b/b, 4..7 word
_ROLES = [("c", 0, 0), ("c", 0, 1), ("c", 1, 0), ("c", 1, 1),
          ("w", 0, 0), ("w", 0, 1), ("w", 1, 0), ("w", 1, 1)]


def _wih_prep(W, dk_n):
    Wp = W[_GPERM]
    return np.ascontiguousarray(
        Wp.reshape(8, 128, dk_n, 128).transpose(3, 2, 0, 1).reshape(128, dk_n * 8 * 128)
    ).astype(np.float16)


def _make_in_maps(inputs):
    char_ids = np.asarray(inputs["char_ids"])
    tags = np.asarray(inputs["tags"])
    wemb = np.asarray(inputs["word_embeddings"], np.float32)
    emb = np.asarray(inputs["char_emb_table"], np.float32)
    trans = np.asarray(inputs["crf_trans"], np.float32)

    alpha = 1.0 / (15.0 * float(np.exp(trans).mean()))
    common = {}
    w1 = np.asarray(inputs["cls_w1"], np.float32)
    common["cls1"] = np.ascontiguousarray(
        w1.reshape(4, 128, 8, 128).transpose(3, 2, 0, 1).reshape(128, 8 * 4 * 128)
    ).astype(np.float16)
    common["clsb1"] = np.ascontiguousarray(
        np.asarray(inputs["cls_b1"], np.float32).reshape(4, 128).T).astype(np.float32)
    w2 = np.asarray(inputs["cls_w2"], np.float32)
    common["cls2"] = np.ascontiguousarray(
        w2.reshape(15, 4, 128).transpose(2, 1, 0).reshape(128, 4 * 15)).astype(np.float16)
    common["clsb2"] = np.asarray(inputs["cls_b2"], np.float32).reshape(15, 1).copy()
    common["mexp"] = np.exp(trans).astype(np.float32)
    common["transn"] = trans.astype(np.float16)
    common["crfstart"] = np.asarray(inputs["crf_start"], np.float32).reshape(15, 1).copy()
    common["crfend"] = np.asarray(inputs["crf_end"], np.float32).reshape(15, 1).copy()
    common["lnalpha"] = np.full((15, 1), np.log(alpha), np.float32)

    in_maps = []
    for c in range(NC_N):
        pw, d, hf = _ROLES[c]
        lo, hi = hf * BL2, (hf + 1) * BL2
        m = dict(common)

        # phase-1 weights/input
        if pw == "c":
            Wih1 = np.zeros((1024, 768), np.float32)
            Wih1[:, :128] = np.asarray(inputs["c0_Wih"], np.float32)[d]
            Whh1 = np.asarray(inputs["c0_Whh"], np.float32)[d]
            b1 = (np.asarray(inputs["c0_bih"], np.float32)[d]
                  + np.asarray(inputs["c0_bhh"], np.float32)[d])
            ce = emb[char_ids[lo:hi]]  # (32, 512, 128)
            X = np.zeros((128, DK1, T, BL2), np.float32)
            X[:, 0] = ce.transpose(2, 1, 0)
            Wl1 = np.asarray(inputs["c1_Wih"], np.float32)[d]
            Whh2 = np.asarray(inputs["c1_Whh"], np.float32)[d]
            b2 = (np.asarray(inputs["c1_bih"], np.float32)[d]
                  + np.asarray(inputs["c1_bhh"], np.float32)[d])
        else:
            Wih1 = np.asarray(inputs["w0_Wih"], np.float32)[d]
            Whh1 = np.asarray(inputs["w0_Whh"], np.float32)[d]
            b1 = (np.asarray(inputs["w0_bih"], np.float32)[d]
                  + np.asarray(inputs["w0_bhh"], np.float32)[d])
            X = wemb[lo:hi].reshape(BL2, T, DK1, 128).transpose(3, 2, 1, 0)
            Wl1 = np.asarray(inputs["w1_Wih"], np.float32)[d]
            Whh2 = np.asarray(inputs["w1_Whh"], np.float32)[d]
            b2 = (np.asarray(inputs["w1_bih"], np.float32)[d]
                  + np.asarray(inputs["w1_bhh"], np.float32)[d])
        if d == 1:  # backward: reverse local time
            X = X[:, :, ::-1]
        m["x1"] = np.ascontiguousarray(X.reshape(128, DK1, T * BL2)).astype(np.float16)
        m["wih1"] = _wih_prep(Wih1, DK1)
        m["whh1"] = _wih_prep(Whh1, 2)
        m["bias1"] = np.ascontiguousarray(b1[_GPERM].reshape(8, 128).T).astype(np.float32)

        # phase-2 weights: columns [own(256) | peer(256)]
        if d == 1:
            Wl1 = Wl1[:, np.r_[256:512, 0:256]]
        m["wih2"] = _wih_prep(Wl1, DK2)
        m["whh2"] = _wih_prep(Whh2, 2)
        m["bias2"] = np.ascontiguousarray(b2[_GPERM].reshape(8, 128).T).astype(np.float32)
        # blend: f-core (d=0) picks AG region 1 (the b-core), b-core picks 0
        msk = np.zeros((128, 2), np.float32)
        msk[:, 1 - d] = 1.0
        m["masks"] = msk

        # phase-3 tags for this core's 8 sequences
        seqs3 = np.r_[4 * c: 4 * c + 4, 32 + 4 * c: 32 + 4 * c + 4]
        oh = (np.arange(K)[:, None, None] == tags[seqs3][None]).astype(np.float32)
        # (15, 8seq, 512t) -> (15, t, b)
        m["tagoneT"] = np.ascontiguousarray(
            oh.transpose(0, 2, 1).reshape(K, TB3)).astype(np.float16)
        in_maps.append(m)
    return in_maps, alpha


def kernel(**inputs):
    nc = _build_nc()
    in_maps, alpha = _make_in_maps(inputs)
    res = run_bass_kernel_spmd(nc, in_maps, core_ids=list(range(NC_N)))
    total = sum(float(res.results[c]["out"][0, 0]) for c in range(NC_N))
    total -= B * (T - 1) * np.log(alpha)
    return np.float32(total / B)


# revision 20
# speedup vs baseline: 1.0364x; 1.0364x over previous
"""BiLSTM dual-pathway + CRF NLL kernel for 8 Trainium2 NeuronCores.

Sharding: direction-parallel for the LSTM recurrences, data-parallel for the
classifier/CRF. Phase 1 runs the four layer-0 directions (char fwd/bwd, word
fwd/bwd) on 8 cores as (direction x batch-half), batch 32 per core, so the
recurrent matmuls run at N=32 instead of N=8 and the sequential chain count
drops from 4096 to 1024 steps. A pairwise AllGather exchanges the L0 hidden
states between fwd/bwd cores, phase 2 runs the four layer-1 directions the
same way, then an 8-rank AllToAll redistributes hidden states to a
data-parallel layout (8 sequences per core) for the classifier and CRF.

SPMD uniformity: every core runs the identical program. Backward directions
receive host-time-reversed inputs; reversed reads of peer hidden states are
fixed negative-stride APs, with host-permuted weight columns absorbing the
f/b role differences. Peer-region selection after the AllGather uses per-core
0/1 blend masks delivered as input data.

The CRF forward scan runs in probability space with a constant per-step
prescale alpha folded into the emission exponentials (corrected analytically
on the host), so each step is one resident-weight 15x15 matmul plus one
vector multiply -- no per-step exp/ln activation-table swaps.
"""

import sys

sys.path.insert(0, "/opt/trn_rl_repo")

import numpy as np

import concourse.bass as bass
import concourse.mybir as mybir
from concourse import bacc
from concourse.bass import ds
from concourse.tile import TileContext
from concourse.bass_utils import run_bass_kernel_spmd

F16 = mybir.dt.float16
F32 = mybir.dt.float32
AF = mybir.ActivationFunctionType

B, T, V, K = 64, 512, 40, 15
NC_N = 8
BL2 = 32            # batch per core in phases 1-2
BL3 = 8             # sequences per core in phase 3
TB3 = T * BL3       # 4096 classifier/CRF columns per core
DK1, DK2 = 6, 4     # input chunks for L0 (word=768, char padded) and L1 (512)
UNROLL = 16
HCOL = T + 1        # h buffer columns per sequence (col 0 = zero init)

_BUILD_CACHE = {}


def _emit_rec_block(nc, stagep, ps_rec, hc, cst, whh, xg_dram, tv, fodder):
    """Emit 16 recurrence steps for time block tv.

    fodder: list of (items, per_step) - closures emitting one PE-side quantum
    each (GEMM matmuls for other pipeline stages), dispensed between the
    step's recurrent matmuls and its vector/scalar chain so they execute in
    the PE's dependency-stall gaps.
    """
    CB = UNROLL * BL2
    xgs = stagep.tile([128, 8, CB], F16, tag="xgs")
    nc.sync.dma_start(out=xgs, in_=xg_dram[:, :, ds(tv * BL2, CB)])
    cursors = [[items, 0, per_step] for items, per_step in fodder]
    for j in range(UNROLL):
        psA = ps_rec.tile([128, 4, BL2], F32, tag="recpsA")
        psB = ps_rec.tile([128, 4, BL2], F32, tag="recpsB")
        # i,f gates (chunks 0-3) first into psA so the first g-add can
        # start while the o/g~ matmuls (psB) still run
        for m in (0, 1, 2, 3):
            for k in range(2):
                nc.tensor.matmul(psA[:, m], whh[:, k, m],
                                 hc[:, k, j], start=(k == 0), stop=(k == 1))
        for m in (6, 7, 4, 5):
            for k in range(2):
                nc.tensor.matmul(psB[:, m - 4], whh[:, k, m],
                                 hc[:, k, j], start=(k == 0), stop=(k == 1))
        for cur in cursors:
            for _ in range(cur[2]):
                if cur[1] < len(cur[0]):
                    cur[0][cur[1]]()
                    cur[1] += 1
        g = stagep.tile([128, 8, BL2], F32, tag="g")
        xsl = xgs[:, :, ds(j * BL2, BL2)]
        nc.vector.tensor_add(g[:, 0:4], psA, xsl[:, 0:4])
        sig = stagep.tile([128, 6, BL2], F32, tag="sig")
        nc.scalar.activation(sig[:, 0:4], g[:, 0:4], AF.Sigmoid)
        nc.vector.tensor_mul(cst, cst, sig[:, 2:4])
        nc.vector.tensor_add(g[:, 4:8], psB, xsl[:, 4:8])
        tgg = stagep.tile([128, 2, BL2], F32, tag="tgg")
        nc.scalar.activation(tgg, g[:, 6:8], AF.Tanh)
        tmp = stagep.tile([128, 2, BL2], F32, tag="tmpig")
        nc.vector.tensor_mul(tmp, sig[:, 0:2], tgg)
        nc.scalar.activation(sig[:, 4:6], g[:, 4:6], AF.Sigmoid)
        nc.vector.tensor_add(cst, cst, tmp)
        tch = stagep.tile([128, 2, BL2], F32, tag="tch")
        nc.scalar.activation(tch, cst, AF.Tanh)
        nc.vector.tensor_mul(hc[:, :, j + 1], sig[:, 4:6], tch)
    for cur in cursors:
        while cur[1] < len(cur[0]):
            cur[0][cur[1]]()
            cur[1] += 1


def _build_nc():
    if "nc" in _BUILD_CACHE:
        return _BUILD_CACHE["nc"]
    nc = bacc.Bacc(target_bir_lowering=False, num_devices=NC_N)

    # ---- external parameters -------------------------------------------------
    x1_ext = nc.declare_dram_parameter("x1", [128, DK1, T * BL2], F16, isOutput=False)
    wih1_ext = nc.declare_dram_parameter("wih1", [128, DK1 * 8 * 128], F16, isOutput=False)
    whh1_ext = nc.declare_dram_parameter("whh1", [128, 2 * 8 * 128], F16, isOutput=False)
    bias1_ext = nc.declare_dram_parameter("bias1", [128, 8], F32, isOutput=False)
    wih2_ext = nc.declare_dram_parameter("wih2", [128, DK2 * 8 * 128], F16, isOutput=False)
    whh2_ext = nc.declare_dram_parameter("whh2", [128, 2 * 8 * 128], F16, isOutput=False)
    bias2_ext = nc.declare_dram_parameter("bias2", [128, 8], F32, isOutput=False)
    masks_ext = nc.declare_dram_parameter("masks", [128, 2], F32, isOutput=False)
    cls1_ext = nc.declare_dram_parameter("cls1", [128, 8 * 4 * 128], F16, isOutput=False)
    clsb1_ext = nc.declare_dram_parameter("clsb1", [128, 4], F32, isOutput=False)
    cls2_ext = nc.declare_dram_parameter("cls2", [128, 4 * 15], F16, isOutput=False)
    clsb2_ext = nc.declare_dram_parameter("clsb2", [15, 1], F32, isOutput=False)
    mexp_ext = nc.declare_dram_parameter("mexp", [15, 15], F32, isOutput=False)
    transn_ext = nc.declare_dram_parameter("transn", [15, 15], F16, isOutput=False)
    start_ext = nc.declare_dram_parameter("crfstart", [15, 1], F32, isOutput=False)
    end_ext = nc.declare_dram_parameter("crfend", [15, 1], F32, isOutput=False)
    lna_ext = nc.declare_dram_parameter("lnalpha", [15, 1], F32, isOutput=False)
    tago_ext = nc.declare_dram_parameter("tagoneT", [15, TB3], F16, isOutput=False)
    out_ext = nc.declare_dram_parameter("out", [1, 1], F32, isOutput=True)

    # ---- internal DRAM -------------------------------------------------------
    HSZ = 2 * BL2 * T  # 32768 cols/partition of h (f16)
    xg1_dram = nc.dram_tensor("xg1", [128, 8, T * BL2], F16)
    xg2_dram = nc.dram_tensor("xg2", [128, 8, T * BL2], F16)
    hmine = nc.dram_tensor("hmine", [128, HSZ], F16)
    agout = nc.dram_tensor("agout", [2, 128, HSZ], F16)
    sendb = nc.dram_tensor("sendb", [8, 128, 2 * 4 * T], F16)
    recvb = nc.dram_tensor("recvb", [8, 128, 2 * 4 * T], F16)

    with TileContext(nc) as tc:
        with (
            tc.tile_pool(name="consts", bufs=1) as consts,
            tc.tile_pool(name="seqs", bufs=1) as seqs,
            tc.tile_pool(name="work", bufs=2) as work,
            tc.tile_pool(name="stage", bufs=2) as stagep,
            tc.tile_pool(name="ps_big", bufs=3, space="PSUM") as ps_big,
            tc.tile_pool(name="ps_rec", bufs=1, space="PSUM") as ps_rec,
            tc.tile_pool(name="ps_small", bufs=3, space="PSUM") as ps_small,
        ):
            # h buffer, seq-major: [128, chunk2, b32, T+1], reused by phases 1+2
            h_sb = seqs.tile([128, 2, BL2, HCOL], F16, tag="h_sb")
            hc = seqs.tile([128, 2, UNROLL + 1, BL2], F16, tag="hcomp")
            cst = seqs.tile([128, 2, BL2], F32, tag="cstate")
            masks = consts.tile([128, 2], F32, tag="masks")
            nc.sync.dma_start(out=masks, in_=masks_ext[:, :])

            # ================= PHASE 1 (L0) =================
            wih1 = consts.tile([128, DK1, 8, 128], F16, tag="wbig")
            nc.sync.dma_start(
                out=wih1,
                in_=wih1_ext.ap().rearrange("p (k m c) -> p k m c", k=DK1, m=8))
            whh1 = consts.tile([128, 2, 8, 128], F16, tag="whh")
            nc.sync.dma_start(
                out=whh1,
                in_=whh1_ext.ap().rearrange("p (k m c) -> p k m c", k=2, m=8))
            bias1 = consts.tile([128, 8], F32, tag="bias1")
            nc.sync.dma_start(out=bias1, in_=bias1_ext[:, :])
            # wih2/bias2 load now: the L1 input GEMM's slotA half runs
            # interleaved inside the phase-1 recurrence
            wih2 = consts.tile([128, DK2, 8, 128], F16, tag="wih2")
            nc.sync.dma_start(
                out=wih2,
                in_=wih2_ext.ap().rearrange("p (k m c) -> p k m c", k=DK2, m=8))
            bias2 = consts.tile([128, 8], F32, tag="bias2")
            nc.sync.dma_start(out=bias2, in_=bias2_ext[:, :])

            CB = UNROLL * BL2
            # phase-1 h layout in DRAM is t-major: (p, c, t, b)
            hm_view = hmine.ap().rearrange("p (c t b) -> p c t b", c=2, t=T)

            def g1_items(tv_lead):
                # xg1 block tv_lead: input DMA now, 48 matmul quanta
                xb = stagep.tile([128, DK1, CB], F16, tag="xb1")
                nc.sync.dma_start(out=xb, in_=x1_ext[:, :, ds(tv_lead * BL2, CB)])
                items, pss = [], {}
                for m in range(8):
                    for k in range(DK1):
                        def it(m=m, k=k):
                            if k == 0:
                                pss[m] = ps_big.tile([128, CB], F32, tag="gemmps",
                                                     name=f"g1ps{m}")
                            nc.tensor.matmul(pss[m], wih1[:, k, m], xb[:, k],
                                             start=(k == 0), stop=(k == DK1 - 1))
                            if k == DK1 - 1:
                                st = stagep.tile([128, CB], F16, tag="xgstage",
                                                 name=f"g1st{m}")
                                nc.scalar.activation(st, pss[m], AF.Identity,
                                                     bias=bias1[:, m: m + 1])
                                nc.sync.dma_start(
                                    out=xg1_dram[:, m, ds(tv_lead * BL2, CB)], in_=st)
                        items.append(it)
                return items

            def p1_end(tv):
                nc.sync.dma_start(out=hm_view[:, :, ds(tv, UNROLL)],
                                  in_=hc[:, :, 1: UNROLL + 1])
                nc.gpsimd.tensor_copy(hc[:, :, 0], hc[:, :, UNROLL])

            def rec1_block(tv, fodder):
                _emit_rec_block(nc, stagep, ps_rec, hc, cst, whh1, xg1_dram,
                                tv, fodder)
                p1_end(tv)

            nc.vector.memset(cst, 0.0)
            nc.vector.memset(hc[:, :, 0], 0.0)
            for blk in range(2):
                for it in g1_items(16 * blk):
                    it()
            rec1_block(0, [(g1_items(32), 3)])
            rec1_block(16, [(g1_items(48), 3)])
            with tc.For_i(32, 480, 16) as tv:
                items1 = g1_items(tv + 32)
                _emit_rec_block(nc, stagep, ps_rec, hc, cst, whh1, xg1_dram,
                                tv, [(items1, 3)])
                p1_end(tv)
            rec1_block(480, [])
            rec1_block(496, [])

            nc.gpsimd.collective_compute(
                "AllGather", mybir.AluOpType.bypass,
                replica_groups=[[0, 2], [1, 3], [4, 6], [5, 7]],
                ins=[hmine.ap()], outs=[agout.ap()],
            )

            # ================= PHASE 2 (L1) =================
            whh2 = consts.tile([128, 2, 8, 128], F16, tag="whh")
            nc.sync.dma_start(
                out=whh2,
                in_=whh2_ext.ap().rearrange("p (k m c) -> p k m c", k=2, m=8))

            def slot_ap(tensor_offset, tensor, c, tv, reverse):
                # [128, UNROLL*32] chunk c of a t-major h DRAM region
                # (p, c2, T, b32): a contiguous ascending t window; reversed
                # slots flip t on the SBUF side.
                if not reverse:
                    w = tv
                else:
                    w = (T - UNROLL) - tv
                off = tensor_offset + c * (T * BL2) + w * BL2
                return bass.AP(tensor=tensor, offset=off,
                               ap=[[HSZ, 128], [1, UNROLL * BL2]])

            def g2_items(tv_lead):
                # full xg2 block: slotA = own h fwd, slotB = peer h reversed
                sa = stagep.tile([128, 2, UNROLL, BL2], F16, tag="slotA")
                r0 = stagep.tile([128, 2, UNROLL, BL2], F16, tag="slotR0")
                r1 = stagep.tile([128, 2, UNROLL, BL2], F16, tag="slotR1")
                for c in range(2):
                    nc.sync.dma_start(out=sa[:, c],
                                      in_=slot_ap(0, hmine, c, tv_lead, False))
                    nc.sync.dma_start(out=r0[:, c],
                                      in_=slot_ap(0, agout, c, tv_lead, True))
                    nc.sync.dma_start(out=r1[:, c],
                                      in_=slot_ap(128 * HSZ, agout, c, tv_lead, True))
                sbr = stagep.tile([128, 2, UNROLL, BL2], F16, tag="slotBr")
                items, pss = [], {}

                def blend0():
                    nc.vector.tensor_scalar_mul(r0, r0, masks[:, 0:1])

                def blend1():
                    nc.vector.tensor_scalar_mul(r1, r1, masks[:, 1:2])

                def blend2():
                    # add, writing with the t window reversed
                    p_step = sbr.ap[0][0]
                    rev = bass.AP(tensor=sbr.tensor,
                                  offset=sbr.offset + (UNROLL - 1) * BL2,
                                  ap=[[p_step, 128], [UNROLL * BL2, 2],
                                      [-BL2, UNROLL], [1, BL2]])
                    nc.vector.tensor_add(rev, r0, r1)

                items += [blend0, blend1, blend2]

                def src_k(k):
                    t = sa if k < 2 else sbr
                    return t[:, k % 2].rearrange("p t b -> p (t b)")

                for m in range(8):
                    for k in range(DK2):
                        def it(m=m, k=k):
                            if k == 0:
                                pss[m] = ps_big.tile([128, CB], F32, tag="gemmps",
                                                     name=f"g2ps{m}")
                            nc.tensor.matmul(pss[m], wih2[:, k, m], src_k(k),
                                             start=(k == 0), stop=(k == DK2 - 1))
                            if k == DK2 - 1:
                                st = stagep.tile([128, CB], F16, tag="xgstage",
                                                 name=f"g2st{m}")
                                nc.scalar.activation(st, pss[m], AF.Identity,
                                                     bias=bias2[:, m: m + 1])
                                nc.sync.dma_start(
                                    out=xg2_dram[:, m, ds(tv_lead * BL2, CB)], in_=st)
                        items.append(it)
                return items

            def p2_end(tv):
                nc.gpsimd.tensor_copy(
                    h_sb[:, :, :, ds(tv + 1, UNROLL)],
                    hc[:, :, 1: UNROLL + 1].rearrange("p c t b -> p c b t"))
                nc.gpsimd.tensor_copy(hc[:, :, 0], hc[:, :, UNROLL])

            nc.vector.memset(cst, 0.0)
            nc.vector.memset(hc[:, :, 0], 0.0)
            for blk in (0, 1):
                for it in g2_items(16 * blk):
                    it()
            with tc.For_i(0, 480, 16) as tv:
                items = g2_items(tv + 32)
                _emit_rec_block(nc, stagep, ps_rec, hc, cst, whh2, xg2_dram,
                                tv, [(items, 3)])
                p2_end(tv)
            for blk in (30, 31):
                _emit_rec_block(nc, stagep, ps_rec, hc, cst, whh2, xg2_dram,
                                16 * blk, [])
                p2_end(16 * blk)

            # ---- AllToAll redistribution to data-parallel layout ----
            sb_view = sendb.ap().rearrange("j p (c b t) -> j p c b t", c=2, b=4)
            for j in range(8):
                for c in range(2):
                    nc.sync.dma_start(out=sb_view[j, :, c],
                                      in_=h_sb[:, c, ds(4 * j, 4), 1:HCOL])
            nc.gpsimd.collective_compute(
                "AllToAll", mybir.AluOpType.bypass,
                replica_groups=[list(range(8))],
                ins=[sendb.ap()], outs=[recvb.ap()],
            )

            # ================= PHASE 3: classifier + CRF =================
            cls1 = consts.tile([128, 8, 4, 128], F16, tag="wbig")
            nc.sync.dma_start(
                out=cls1,
                in_=cls1_ext.ap().rearrange("p (k m c) -> p k m c", k=8, m=4))
            clsb1 = consts.tile([128, 4], F32, tag="clsb1")
            nc.sync.dma_start(out=clsb1, in_=clsb1_ext[:, :])
            cls2 = consts.tile([128, 4, 15], F16, tag="cls2")
            nc.sync.dma_start(
                out=cls2, in_=cls2_ext.ap().rearrange("p (k j) -> p k j", k=4))
            clsb2 = consts.tile([15, 1], F32, tag="clsb2")
            nc.sync.dma_start(out=clsb2, in_=clsb2_ext[:, :])
            mexp = consts.tile([15, 15], F32, tag="mexp")
            nc.sync.dma_start(out=mexp, in_=mexp_ext[:, :])
            transn = consts.tile([15, 15], F16, tag="transn")
            nc.sync.dma_start(out=transn, in_=transn_ext[:, :])
            crfstart = consts.tile([15, 1], F32, tag="crfstart")
            nc.sync.dma_start(out=crfstart, in_=start_ext[:, :])
            crfend = consts.tile([15, 1], F32, tag="crfend")
            nc.sync.dma_start(out=crfend, in_=end_ext[:, :])
            lnalpha = consts.tile([15, 1], F32, tag="lnalpha")
            nc.sync.dma_start(out=lnalpha, in_=lna_ext[:, :])
            tago = consts.tile([15, TB3], F16, tag="tago")
            nc.sync.dma_start(out=tago, in_=tago_ext[:, :])

            logits = seqs.tile([15, TB3], F32, tag="logits")

            NT = 64  # t-steps per classifier n-tile (NT*BL3 = 512 cols)
            SHSZ = 128 * 2 * 4 * T  # elements per recv shard

            def comb_ap(kk, half, ns):
                # [128, 4, NT]: dir kk//2, chunk kk%2, half-shard, n-tile ns;
                # always an ascending t window (reversal done in the rhs view)
                d, c = kk // 2, kk % 2
                rev = d in (1, 3)  # c1b, w1b stored time-reversed
                base = (2 * d + half) * SHSZ + c * (4 * T)
                if not rev:
                    off = base + ns * NT
                else:
                    off = base + (T - NT) - ns * NT
                return bass.AP(tensor=recvb, offset=off,
                               ap=[[2 * 4 * T, 128], [T, 4], [1, NT]])

            for ns in range(8):
                comb = stagep.tile([128, 8, BL3, NT], F16, tag="comb", bufs=2)
                for kk in range(8):
                    for half in range(2):
                        nc.sync.dma_start(out=comb[:, kk, ds(4 * half, 4)],
                                          in_=comb_ap(kk, half, ns))
                hmt = []
                for m in range(4):
                    ps = ps_big.tile([128, NT * BL3], F32, tag="gemmps")
                    for kk in range(8):
                        if kk // 2 in (1, 3):
                            p_step = comb.ap[0][0]
                            off = (comb.offset + kk * BL3 * NT + (NT - 1))
                            rhs = bass.AP(tensor=comb.tensor, offset=off,
                                          ap=[[p_step, 128], [-1, NT],
                                              [NT, BL3]])
                        else:
                            rhs = comb[:, kk].rearrange("p b t -> p t b")
                        nc.tensor.matmul(ps, cls1[:, kk, m], rhs,
                                         start=(kk == 0), stop=(kk == 7))
                    hm = stagep.tile([128, NT * BL3], F16, tag="hm", bufs=4,
                                     name=f"hm{m}")
                    nc.scalar.activation(hm, ps, AF.Relu, bias=clsb1[:, m: m + 1])
                    hmt.append(hm)
                ps2 = ps_small.tile([15, NT * BL3], F32, tag="small")
                for m in range(4):
                    nc.tensor.matmul(ps2, cls2[:, m], hmt[m],
                                     start=(m == 0), stop=(m == 3))
                nc.vector.tensor_scalar_add(
                    logits[:, ds(ns * NT * BL3, NT * BL3)], ps2, clsb2)

            # fold CRF start/end into first/last emission columns
            nc.vector.tensor_scalar_add(logits[:, 0:BL3], logits[:, 0:BL3], crfstart)
            nc.vector.tensor_scalar_add(logits[:, TB3 - BL3: TB3],
                                        logits[:, TB3 - BL3: TB3], crfend)

            # ---- CRF numerator ----
            racc = work.tile([15, 16], F32, tag="racc")
            nc.vector.memset(racc, 0.0)
            for ns in range(8):
                pre = stagep.tile([15, 512], F32, tag="prodns")
                nc.vector.tensor_mul(pre, logits[:, ds(ns * 512, 512)],
                                     tago[:, ds(ns * 512, 512)])
                nc.vector.tensor_reduce(racc[:, 8 + ns: 9 + ns], pre,
                                        axis=mybir.AxisListType.X,
                                        op=mybir.AluOpType.add)
                psv = ps_small.tile([15, 512], F32, tag="small")
                nc.tensor.matmul(psv, transn, tago[:, ds(ns * 512, 512)],
                                 start=True, stop=True)
                w = 512 if ns < 7 else 512 - BL3
                pr = stagep.tile([15, 512], F32, tag="prodns")
                nc.vector.tensor_mul(pr[:, :w], psv[:, :w],
                                     tago[:, ds(ns * 512 + BL3, w)])
                nc.vector.tensor_reduce(racc[:, ns: ns + 1], pr[:, :w],
                                        axis=mybir.AxisListType.X,
                                        op=mybir.AluOpType.add)
            nv = stagep.tile([15, 1], F32, tag="nv")
            nc.vector.tensor_reduce(nv, racc, axis=mybir.AxisListType.X,
                                    op=mybir.AluOpType.add)
            ones15 = consts.tile([15, 1], F32, tag="ones15")
            nc.vector.memset(ones15, 1.0)
            psn = ps_small.tile([1, 1], F32, tag="small")
            nc.tensor.matmul(psn, ones15, nv, start=True, stop=True)
            num11 = work.tile([1, 1], F32, tag="num11")
            nc.vector.tensor_copy(num11, psn)

            # ---- CRF forward scan, probability space ----
            # two independent half-batch chains so the 15x15 matmul of one
            # overlaps the vector multiply of the other. p0 first, then
            # E = alpha*exp(logits) in place (last col already has e^end).
            pA = seqs.tile([15, 4], F32, tag="pvecA")
            pB = seqs.tile([15, 4], F32, tag="pvecB")
            nc.scalar.activation(pA, logits[:, 0:4], AF.Exp)
            nc.scalar.activation(pB, logits[:, 4:8], AF.Exp)
            E = logits
            nc.scalar.activation(E, logits, AF.Exp, bias=lnalpha)

            def crf_step(col):
                zA = ps_small.tile([15, 4], F32, tag="small", name="zA")
                nc.tensor.matmul(zA, mexp, pA, start=True, stop=True)
                zB = ps_small.tile([15, 4], F32, tag="small", name="zB")
                nc.tensor.matmul(zB, mexp, pB, start=True, stop=True)
                nc.vector.tensor_mul(pA, zA, E[:, col: col + 4] if isinstance(col, int)
                                     else E[:, ds(col, 4)])
                nc.vector.tensor_mul(pB, zB, E[:, col + 4: col + 8] if isinstance(col, int)
                                     else E[:, ds(col + 4, 4)])

            for t in range(1, 16):
                crf_step(t * BL3)
            with tc.For_i(0, 496, UNROLL) as tv:
                for j in range(UNROLL):
                    crf_step((16 + j) * BL3 + tv * BL3)

            # ---- denominator + output ----
            den11 = work.tile([1, 1], F32, tag="den11")
            for idx, pv in enumerate((pA, pB)):
                psd = ps_small.tile([1, 4], F32, tag="small", name=f"psd{idx}")
                nc.tensor.matmul(psd, ones15, pv, start=True, stop=True)
                ln4 = stagep.tile([1, 4], F32, tag="ln8", name=f"ln4{idx}")
                nc.scalar.activation(ln4, psd, AF.Ln)
                if idx == 0:
                    nc.vector.tensor_reduce(den11, ln4, axis=mybir.AxisListType.X,
                                            op=mybir.AluOpType.add)
                else:
                    dh = work.tile([1, 1], F32, tag="denh")
                    nc.vector.tensor_reduce(dh, ln4, axis=mybir.AxisListType.X,
                                            op=mybir.AluOpType.add)
                    nc.vector.tensor_add(den11, den11, dh)
            res = work.tile([1, 1], F32, tag="res")
            nc.vector.tensor_sub(res, den11, num11)
            nc.sync.dma_start(out=out_ext[:, :], in_=res)

    nc.finalize()
    _BUILD_CACHE["nc"] = nc
    return nc


# ---- host-side input prep ---------------------------------------------------

# gate perm [i(256), f(256), g(256), o(256)] -> [i, f, o, g~]
_GPERM = np.concatenate([np.arange(0, 512), np.arange(768, 1024), np.arange(512, 768)])

# core c -> (pathway, direction, half): 0..3 char f/f/b/b, 4..7 word
_ROLES = [("c", 0, 0), ("c", 0, 1), ("c", 1, 0), ("c", 1, 1),
          ("w", 0, 0), ("w", 0, 1), ("w", 1, 0), ("w", 1, 1)]


def _wih_prep(W, dk_n):
    Wp = W[_GPERM]
    return np.ascontiguousarray(
        Wp.reshape(8, 128, dk_n, 128).transpose(3, 2, 0, 1).reshape(128, dk_n * 8 * 128)
    ).astype(np.float16)


def _make_in_maps(inputs):
    char_ids = np.asarray(inputs["char_ids"])
    tags = np.asarray(inputs["tags"])
    wemb = np.asarray(inputs["word_embeddings"], np.float32)
    emb = np.asarray(inputs["char_emb_table"], np.float32)
    trans = np.asarray(inputs["crf_trans"], np.float32)

    alpha = 1.0 / (15.0 * float(np.exp(trans).mean()))
    common = {}
    w1 = np.asarray(inputs["cls_w1"], np.float32)
    common["cls1"] = np.ascontiguousarray(
        w1.reshape(4, 128, 8, 128).transpose(3, 2, 0, 1).reshape(128, 8 * 4 * 128)
    ).astype(np.float16)
    common["clsb1"] = np.ascontiguousarray(
        np.asarray(inputs["cls_b1"], np.float32).reshape(4, 128).T).astype(np.float32)
    w2 = np.asarray(inputs["cls_w2"], np.float32)
    common["cls2"] = np.ascontiguousarray(
        w2.reshape(15, 4, 128).transpose(2, 1, 0).reshape(128, 4 * 15)).astype(np.float16)
    common["clsb2"] = np.asarray(inputs["cls_b2"], np.float32).reshape(15, 1).copy()
    common["mexp"] = np.exp(trans).astype(np.float32)
    common["transn"] = trans.astype(np.float16)
    common["crfstart"] = np.asarray(inputs["crf_start"], np.float32).reshape(15, 1).copy()
    common["crfend"] = np.asarray(inputs["crf_end"], np.float32).reshape(15, 1).copy()
    common["lnalpha"] = np.full((15, 1), np.log(alpha), np.float32)

    in_maps = []
    for c in range(NC_N):
        pw, d, hf = _ROLES[c]
        lo, hi = hf * BL2, (hf + 1) * BL2
        m = dict(common)

        # phase-1 weights/input
        if pw == "c":
            Wih1 = np.zeros((1024, 768), np.float32)
            Wih1[:, :128] = np.asarray(inputs["c0_Wih"], np.float32)[d]
            Whh1 = np.asarray(inputs["c0_Whh"], np.float32)[d]
            b1 = (np.asarray(inputs["c0_bih"], np.float32)[d]
                  + np.asarray(inputs["c0_bhh"], np.float32)[d])
            ce = emb[char_ids[lo:hi]]  # (32, 512, 128)
            X = np.zeros((128, DK1, T, BL2), np.float32)
            X[:, 0] = ce.transpose(2, 1, 0)
            Wl1 = np.asarray(inputs["c1_Wih"], np.float32)[d]
            Whh2 = np.asarray(inputs["c1_Whh"], np.float32)[d]
            b2 = (np.asarray(inputs["c1_bih"], np.float32)[d]
                  + np.asarray(inputs["c1_bhh"], np.float32)[d])
        else:
            Wih1 = np.asarray(inputs["w0_Wih"], np.float32)[d]
            Whh1 = np.asarray(inputs["w0_Whh"], np.float32)[d]
            b1 = (np.asarray(inputs["w0_bih"], np.float32)[d]
                  + np.asarray(inputs["w0_bhh"], np.float32)[d])
            X = wemb[lo:hi].reshape(BL2, T, DK1, 128).transpose(3, 2, 1, 0)
            Wl1 = np.asarray(inputs["w1_Wih"], np.float32)[d]
            Whh2 = np.asarray(inputs["w1_Whh"], np.float32)[d]
            b2 = (np.asarray(inputs["w1_bih"], np.float32)[d]
                  + np.asarray(inputs["w1_bhh"], np.float32)[d])
        if d == 1:  # backward: reverse local time
            X = X[:, :, ::-1]
        m["x1"] = np.ascontiguousarray(X.reshape(128, DK1, T * BL2)).astype(np.float16)
        m["wih1"] = _wih_prep(Wih1, DK1)
        m["whh1"] = _wih_prep(Whh1, 2)
        m["bias1"] = np.ascontiguousarray(b1[_GPERM].reshape(8, 128).T).astype(np.float32)

        # phase-2 weights: columns [own(256) | peer(256)]
        if d == 1:
            Wl1 = Wl1[:, np.r_[256:512, 0:256]]
        m["wih2"] = _wih_prep(Wl1, DK2)
        m["whh2"] = _wih_prep(Whh2, 2)
        m["bias2"] = np.ascontiguousarray(b2[_GPERM].reshape(8, 128).T).astype(np.float32)
        # blend: f-core (d=0) picks AG region 1 (the b-core), b-core picks 0
        msk = np.zeros((128, 2), np.float32)
        msk[:, 1 - d] = 1.0
        m["masks"] = msk

        # phase-3 tags for this core's 8 sequences
        seqs3 = np.r_[4 * c: 4 * c + 4, 32 + 4 * c: 32 + 4 * c + 4]
        oh = (np.arange(K)[:, None, None] == tags[seqs3][None]).astype(np.float32)
        # (15, 8seq, 512t) -> (15, t, b)
        m["tagoneT"] = np.ascontiguousarray(
            oh.transpose(0, 2, 1).reshape(K, TB3)).astype(np.float16)
        in_maps.append(m)
    return in_maps, alpha


def kernel(**inputs):
    nc = _build_nc()
    in_maps, alpha = _make_in_maps(inputs)
    res = run_bass_kernel_spmd(nc, in_maps, core_ids=list(range(NC_N)))
    total = sum(float(res.results[c]["out"][0, 0]) for c in range(NC_N))
    total -= B * (T - 1) * np.log(alpha)
    return np.float32(total / B)


# revision 21
# speedup vs baseline: 1.1021x; 1.0634x over previous
"""BiLSTM dual-pathway + CRF NLL kernel for 8 Trainium2 NeuronCores.

Sharding: direction-parallel for the LSTM recurrences, data-parallel for the
classifier/CRF. Phase 1 runs the four layer-0 directions (char fwd/bwd, word
fwd/bwd) on 8 cores as (direction x batch-half), batch 32 per core, so the
recurrent matmuls run at N=32 instead of N=8 and the sequential chain count
drops from 4096 to 1024 steps. A pairwise AllGather exchanges the L0 hidden
states between fwd/bwd cores, phase 2 runs the four layer-1 directions the
same way, then an 8-rank AllToAll redistributes hidden states to a
data-parallel layout (8 sequences per core) for the classifier and CRF.

SPMD uniformity: every core runs the identical program. Backward directions
receive host-time-reversed inputs; reversed reads of peer hidden states are
fixed negative-stride APs, with host-permuted weight columns absorbing the
f/b role differences. Peer-region selection after the AllGather uses per-core
0/1 blend masks delivered as input data.

The CRF forward scan runs in probability space with a constant per-step
prescale alpha folded into the emission exponentials (corrected analytically
on the host), so each step is one resident-weight 15x15 matmul plus one
vector multiply -- no per-step exp/ln activation-table swaps.
"""

import sys

sys.path.insert(0, "/opt/trn_rl_repo")

import numpy as np

import concourse.bass as bass
import concourse.mybir as mybir
from concourse import bacc
from concourse.bass import ds
from concourse.tile import TileContext
from concourse.bass_utils import run_bass_kernel_spmd

F16 = mybir.dt.float16
F32 = mybir.dt.float32
AF = mybir.ActivationFunctionType

B, T, V, K = 64, 512, 40, 15
NC_N = 8
BL2 = 32            # batch per core in phases 1-2
BL3 = 8             # sequences per core in phase 3
TB3 = T * BL3       # 4096 classifier/CRF columns per core
DK1, DK2 = 6, 4     # input chunks for L0 (word=768, char padded) and L1 (512)
UNROLL = 16
HCOL = T + 1        # h buffer columns per sequence (col 0 = zero init)

_BUILD_CACHE = {}


def _emit_rec_block(nc, stagep, ps_rec, hc, cst, whh, xgs, xg_dram, tv, fodder,
                    prefetch_tv=None):
    """Emit 16 recurrence steps for time block tv.

    fodder: list of (items, per_step) - closures emitting one PE-side quantum
    each (GEMM matmuls for other pipeline stages), dispensed between the
    step's recurrent matmuls and its vector/scalar chain so they execute in
    the PE's dependency-stall gaps.
    """
    CB = UNROLL * BL2
    cursors = [[items, 0, per_step] for items, per_step in fodder]
    for j in range(UNROLL):
        psA = ps_rec.tile([128, 4, BL2], F32, tag="recpsA")
        psB = ps_rec.tile([128, 4, BL2], F32, tag="recpsB")
        # i,f gates (chunks 0-3) first into psA so the first g-add can
        # start while the o/g~ matmuls (psB) still run
        for m in (0, 1, 2, 3):
            for k in range(2):
                nc.tensor.matmul(psA[:, m], whh[:, k, m],
                                 hc[:, k, j], start=(k == 0), stop=(k == 1))
        for m in (6, 7, 4, 5):
            for k in range(2):
                nc.tensor.matmul(psB[:, m - 4], whh[:, k, m],
                                 hc[:, k, j], start=(k == 0), stop=(k == 1))
        for cur in cursors:
            for _ in range(cur[2]):
                if cur[1] < len(cur[0]):
                    cur[0][cur[1]]()
                    cur[1] += 1
        g = stagep.tile([128, 8, BL2], F32, tag="g")
        xsl = xgs[:, :, ds(j * BL2, BL2)]
        nc.vector.tensor_add(g[:, 0:4], psA, xsl[:, 0:4])
        sig = stagep.tile([128, 6, BL2], F32, tag="sig")
        nc.scalar.activation(sig[:, 0:4], g[:, 0:4], AF.Sigmoid)
        nc.vector.tensor_mul(cst, cst, sig[:, 2:4])
        nc.vector.tensor_add(g[:, 4:8], psB, xsl[:, 4:8])
        tgg = stagep.tile([128, 2, BL2], F32, tag="tgg")
        nc.scalar.activation(tgg, g[:, 6:8], AF.Tanh)
        tmp = stagep.tile([128, 2, BL2], F32, tag="tmpig")
        nc.vector.tensor_mul(tmp, sig[:, 0:2], tgg)
        nc.scalar.activation(sig[:, 4:6], g[:, 4:6], AF.Sigmoid)
        nc.vector.tensor_add(cst, cst, tmp)
        tch = stagep.tile([128, 2, BL2], F32, tag="tch")
        nc.scalar.activation(tch, cst, AF.Tanh)
        nc.vector.tensor_mul(hc[:, :, j + 1], sig[:, 4:6], tch)
    for cur in cursors:
        while cur[1] < len(cur[0]):
            cur[0][cur[1]]()
            cur[1] += 1
    if prefetch_tv is not None:
        # refill this block's (just-consumed) xgs buffer with the same-parity
        # block two ahead -- a full block of DMA lead time
        nc.sync.dma_start(out=xgs, in_=xg_dram[:, :, ds(prefetch_tv * BL2, CB)])


def _build_nc():
    if "nc" in _BUILD_CACHE:
        return _BUILD_CACHE["nc"]
    nc = bacc.Bacc(target_bir_lowering=False, num_devices=NC_N)

    # ---- external parameters -------------------------------------------------
    x1_ext = nc.declare_dram_parameter("x1", [128, DK1, T * BL2], F16, isOutput=False)
    wih1_ext = nc.declare_dram_parameter("wih1", [128, DK1 * 8 * 128], F16, isOutput=False)
    whh1_ext = nc.declare_dram_parameter("whh1", [128, 2 * 8 * 128], F16, isOutput=False)
    bias1_ext = nc.declare_dram_parameter("bias1", [128, 8], F32, isOutput=False)
    wih2_ext = nc.declare_dram_parameter("wih2", [128, DK2 * 8 * 128], F16, isOutput=False)
    whh2_ext = nc.declare_dram_parameter("whh2", [128, 2 * 8 * 128], F16, isOutput=False)
    bias2_ext = nc.declare_dram_parameter("bias2", [128, 8], F32, isOutput=False)
    masks_ext = nc.declare_dram_parameter("masks", [128, 2], F32, isOutput=False)
    cls1_ext = nc.declare_dram_parameter("cls1", [128, 8 * 4 * 128], F16, isOutput=False)
    clsb1_ext = nc.declare_dram_parameter("clsb1", [128, 4], F32, isOutput=False)
    cls2_ext = nc.declare_dram_parameter("cls2", [128, 4 * 15], F16, isOutput=False)
    clsb2_ext = nc.declare_dram_parameter("clsb2", [15, 1], F32, isOutput=False)
    mexp_ext = nc.declare_dram_parameter("mexp", [15, 15], F32, isOutput=False)
    transn_ext = nc.declare_dram_parameter("transn", [15, 15], F16, isOutput=False)
    start_ext = nc.declare_dram_parameter("crfstart", [15, 1], F32, isOutput=False)
    end_ext = nc.declare_dram_parameter("crfend", [15, 1], F32, isOutput=False)
    lna_ext = nc.declare_dram_parameter("lnalpha", [15, 1], F32, isOutput=False)
    tago_ext = nc.declare_dram_parameter("tagoneT", [15, TB3], F16, isOutput=False)
    out_ext = nc.declare_dram_parameter("out", [1, 1], F32, isOutput=True)

    # ---- internal DRAM -------------------------------------------------------
    HSZ = 2 * BL2 * T  # 32768 cols/partition of h (f16)
    xg1_dram = nc.dram_tensor("xg1", [128, 8, T * BL2], F16)
    xg2_dram = nc.dram_tensor("xg2", [128, 8, T * BL2], F16)
    hmine = nc.dram_tensor("hmine", [128, HSZ], F16)
    agout = nc.dram_tensor("agout", [2, 128, HSZ], F16)
    sendb = nc.dram_tensor("sendb", [8, 128, 2 * 4 * T], F16)
    recvb = nc.dram_tensor("recvb", [8, 128, 2 * 4 * T], F16)

    with TileContext(nc) as tc:
        with (
            tc.tile_pool(name="consts", bufs=1) as consts,
            tc.tile_pool(name="seqs", bufs=1) as seqs,
            tc.tile_pool(name="work", bufs=2) as work,
            tc.tile_pool(name="stage", bufs=2) as stagep,
            tc.tile_pool(name="ps_big", bufs=3, space="PSUM") as ps_big,
            tc.tile_pool(name="ps_rec", bufs=1, space="PSUM") as ps_rec,
            tc.tile_pool(name="ps_small", bufs=3, space="PSUM") as ps_small,
        ):
            # h buffer, seq-major: [128, chunk2, b32, T+1], reused by phases 1+2
            h_sb = seqs.tile([128, 2, BL2, HCOL], F16, tag="h_sb")
            hc = seqs.tile([128, 2, UNROLL + 1, BL2], F16, tag="hcomp")
            cst = seqs.tile([128, 2, BL2], F32, tag="cstate")
            masks = consts.tile([128, 2], F32, tag="masks")
            nc.sync.dma_start(out=masks, in_=masks_ext[:, :])

            # ================= PHASE 1 (L0) =================
            wih1 = consts.tile([128, DK1, 8, 128], F16, tag="wbig")
            nc.sync.dma_start(
                out=wih1,
                in_=wih1_ext.ap().rearrange("p (k m c) -> p k m c", k=DK1, m=8))
            whh1 = consts.tile([128, 2, 8, 128], F16, tag="whh")
            nc.sync.dma_start(
                out=whh1,
                in_=whh1_ext.ap().rearrange("p (k m c) -> p k m c", k=2, m=8))
            bias1 = consts.tile([128, 8], F32, tag="bias1")
            nc.sync.dma_start(out=bias1, in_=bias1_ext[:, :])
            # wih2/bias2 load now: the L1 input GEMM's slotA half runs
            # interleaved inside the phase-1 recurrence
            wih2 = consts.tile([128, DK2, 8, 128], F16, tag="wih2")
            nc.sync.dma_start(
                out=wih2,
                in_=wih2_ext.ap().rearrange("p (k m c) -> p k m c", k=DK2, m=8))
            bias2 = consts.tile([128, 8], F32, tag="bias2")
            nc.sync.dma_start(out=bias2, in_=bias2_ext[:, :])

            CB = UNROLL * BL2
            # phase-1 h layout in DRAM is t-major: (p, c, t, b)
            hm_view = hmine.ap().rearrange("p (c t b) -> p c t b", c=2, t=T)

            def g1_items(tv_lead):
                # xg1 block tv_lead: input DMA now, 48 matmul quanta
                xb = stagep.tile([128, DK1, CB], F16, tag="xb1")
                nc.sync.dma_start(out=xb, in_=x1_ext[:, :, ds(tv_lead * BL2, CB)])
                items, pss = [], {}
                for m in range(8):
                    for k in range(DK1):
                        def it(m=m, k=k):
                            if k == 0:
                                pss[m] = ps_big.tile([128, CB], F32, tag="gemmps",
                                                     name=f"g1ps{m}")
                            nc.tensor.matmul(pss[m], wih1[:, k, m], xb[:, k],
                                             start=(k == 0), stop=(k == DK1 - 1))
                            if k == DK1 - 1:
                                st = stagep.tile([128, CB], F16, tag="xgstage",
                                                 name=f"g1st{m}")
                                nc.scalar.activation(st, pss[m], AF.Identity,
                                                     bias=bias1[:, m: m + 1])
                                nc.sync.dma_start(
                                    out=xg1_dram[:, m, ds(tv_lead * BL2, CB)], in_=st)
                        items.append(it)
                return items

            def p1_end(tv):
                nc.sync.dma_start(out=hm_view[:, :, ds(tv, UNROLL)],
                                  in_=hc[:, :, 1: UNROLL + 1])
                nc.gpsimd.tensor_copy(hc[:, :, 0], hc[:, :, UNROLL])

            xgsA = stagep.tile([128, 8, CB], F16, tag="xgs", name="xgsA")
            xgsB = stagep.tile([128, 8, CB], F16, tag="xgs", name="xgsB")

            def rec1_block(tv, fodder, xgs, pf):
                _emit_rec_block(nc, stagep, ps_rec, hc, cst, whh1, xgs,
                                xg1_dram, tv, fodder, prefetch_tv=pf)
                p1_end(tv)

            nc.vector.memset(cst, 0.0)
            nc.vector.memset(hc[:, :, 0], 0.0)
            for blk in range(2):
                for it in g1_items(16 * blk):
                    it()
            nc.sync.dma_start(out=xgsA, in_=xg1_dram[:, :, ds(0, CB)])
            nc.sync.dma_start(out=xgsB, in_=xg1_dram[:, :, ds(16 * BL2, CB)])
            rec1_block(0, [(g1_items(32), 3)], xgsA, 32)
            rec1_block(16, [(g1_items(48), 3)], xgsB, 48)
            with tc.For_i(32, 480, 32) as tv:
                items1 = g1_items(tv + 32)
                _emit_rec_block(nc, stagep, ps_rec, hc, cst, whh1, xgsA,
                                xg1_dram, tv, [(items1, 3)], prefetch_tv=tv + 32)
                p1_end(tv)
                items1 = g1_items(tv + 48)
                _emit_rec_block(nc, stagep, ps_rec, hc, cst, whh1, xgsB,
                                xg1_dram, tv + 16, [(items1, 3)],
                                prefetch_tv=tv + 48)
                p1_end(tv + 16)
            rec1_block(480, [], xgsA, None)
            rec1_block(496, [], xgsB, None)

            nc.gpsimd.collective_compute(
                "AllGather", mybir.AluOpType.bypass,
                replica_groups=[[0, 2], [1, 3], [4, 6], [5, 7]],
                ins=[hmine.ap()], outs=[agout.ap()],
            )

            # ================= PHASE 2 (L1) =================
            whh2 = consts.tile([128, 2, 8, 128], F16, tag="whh")
            nc.sync.dma_start(
                out=whh2,
                in_=whh2_ext.ap().rearrange("p (k m c) -> p k m c", k=2, m=8))

            def slot_ap(tensor_offset, tensor, c, tv, reverse):
                # [128, UNROLL*32] chunk c of a t-major h DRAM region
                # (p, c2, T, b32): a contiguous ascending t window; reversed
                # slots flip t on the SBUF side.
                if not reverse:
                    w = tv
                else:
                    w = (T - UNROLL) - tv
                off = tensor_offset + c * (T * BL2) + w * BL2
                return bass.AP(tensor=tensor, offset=off,
                               ap=[[HSZ, 128], [1, UNROLL * BL2]])

            def g2_items(tv_lead):
                # full xg2 block: slotA = own h fwd, slotB = peer h reversed
                sa = stagep.tile([128, 2, UNROLL, BL2], F16, tag="slotA")
                r0 = stagep.tile([128, 2, UNROLL, BL2], F16, tag="slotR0")
                r1 = stagep.tile([128, 2, UNROLL, BL2], F16, tag="slotR1")
                for c in range(2):
                    nc.sync.dma_start(out=sa[:, c],
                                      in_=slot_ap(0, hmine, c, tv_lead, False))
                    nc.sync.dma_start(out=r0[:, c],
                                      in_=slot_ap(0, agout, c, tv_lead, True))
                    nc.sync.dma_start(out=r1[:, c],
                                      in_=slot_ap(128 * HSZ, agout, c, tv_lead, True))
                sbr = stagep.tile([128, 2, UNROLL, BL2], F16, tag="slotBr")
                items, pss = [], {}

                def blend0():
                    nc.vector.tensor_scalar_mul(r0, r0, masks[:, 0:1])

                def blend1():
                    nc.vector.tensor_scalar_mul(r1, r1, masks[:, 1:2])

                def blend2():
                    # add, writing with the t window reversed
                    p_step = sbr.ap[0][0]
                    rev = bass.AP(tensor=sbr.tensor,
                                  offset=sbr.offset + (UNROLL - 1) * BL2,
                                  ap=[[p_step, 128], [UNROLL * BL2, 2],
                                      [-BL2, UNROLL], [1, BL2]])
                    nc.vector.tensor_add(rev, r0, r1)

                items += [blend0, blend1, blend2]

                def src_k(k):
                    t = sa if k < 2 else sbr
                    return t[:, k % 2].rearrange("p t b -> p (t b)")

                for m in range(8):
                    for k in range(DK2):
                        def it(m=m, k=k):
                            if k == 0:
                                pss[m] = ps_big.tile([128, CB], F32, tag="gemmps",
                                                     name=f"g2ps{m}")
                            nc.tensor.matmul(pss[m], wih2[:, k, m], src_k(k),
                                             start=(k == 0), stop=(k == DK2 - 1))
                            if k == DK2 - 1:
                                st = stagep.tile([128, CB], F16, tag="xgstage",
                                                 name=f"g2st{m}")
                                nc.scalar.activation(st, pss[m], AF.Identity,
                                                     bias=bias2[:, m: m + 1])
                                nc.sync.dma_start(
                                    out=xg2_dram[:, m, ds(tv_lead * BL2, CB)], in_=st)
                        items.append(it)
                return items

            def p2_end(tv):
                nc.gpsimd.tensor_copy(
                    h_sb[:, :, :, ds(tv + 1, UNROLL)],
                    hc[:, :, 1: UNROLL + 1].rearrange("p c t b -> p c b t"))
                nc.gpsimd.tensor_copy(hc[:, :, 0], hc[:, :, UNROLL])

            nc.vector.memset(cst, 0.0)
            nc.vector.memset(hc[:, :, 0], 0.0)
            for blk in (0, 1):
                for it in g2_items(16 * blk):
                    it()
            nc.sync.dma_start(out=xgsA, in_=xg2_dram[:, :, ds(0, CB)])
            nc.sync.dma_start(out=xgsB, in_=xg2_dram[:, :, ds(16 * BL2, CB)])
            with tc.For_i(0, 480, 32) as tv:
                items = g2_items(tv + 32)
                _emit_rec_block(nc, stagep, ps_rec, hc, cst, whh2, xgsA,
                                xg2_dram, tv, [(items, 3)], prefetch_tv=tv + 32)
                p2_end(tv)
                items = g2_items(tv + 48)
                _emit_rec_block(nc, stagep, ps_rec, hc, cst, whh2, xgsB,
                                xg2_dram, tv + 16, [(items, 3)],
                                prefetch_tv=tv + 48)
                p2_end(tv + 16)
            _emit_rec_block(nc, stagep, ps_rec, hc, cst, whh2, xgsA,
                            xg2_dram, 480, [])
            p2_end(480)
            _emit_rec_block(nc, stagep, ps_rec, hc, cst, whh2, xgsB,
                            xg2_dram, 496, [])
            p2_end(496)

            # ---- AllToAll redistribution to data-parallel layout ----
            sb_view = sendb.ap().rearrange("j p (c b t) -> j p c b t", c=2, b=4)
            for j in range(8):
                for c in range(2):
                    nc.sync.dma_start(out=sb_view[j, :, c],
                                      in_=h_sb[:, c, ds(4 * j, 4), 1:HCOL])
            nc.gpsimd.collective_compute(
                "AllToAll", mybir.AluOpType.bypass,
                replica_groups=[list(range(8))],
                ins=[sendb.ap()], outs=[recvb.ap()],
            )

            # ================= PHASE 3: classifier + CRF =================
            cls1 = consts.tile([128, 8, 4, 128], F16, tag="wbig")
            nc.sync.dma_start(
                out=cls1,
                in_=cls1_ext.ap().rearrange("p (k m c) -> p k m c", k=8, m=4))
            clsb1 = consts.tile([128, 4], F32, tag="clsb1")
            nc.sync.dma_start(out=clsb1, in_=clsb1_ext[:, :])
            cls2 = consts.tile([128, 4, 15], F16, tag="cls2")
            nc.sync.dma_start(
                out=cls2, in_=cls2_ext.ap().rearrange("p (k j) -> p k j", k=4))
            clsb2 = consts.tile([15, 1], F32, tag="clsb2")
            nc.sync.dma_start(out=clsb2, in_=clsb2_ext[:, :])
            mexp = consts.tile([15, 15], F32, tag="mexp")
            nc.sync.dma_start(out=mexp, in_=mexp_ext[:, :])
            transn = consts.tile([15, 15], F16, tag="transn")
            nc.sync.dma_start(out=transn, in_=transn_ext[:, :])
            crfstart = consts.tile([15, 1], F32, tag="crfstart")
            nc.sync.dma_start(out=crfstart, in_=start_ext[:, :])
            crfend = consts.tile([15, 1], F32, tag="crfend")
            nc.sync.dma_start(out=crfend, in_=end_ext[:, :])
            lnalpha = consts.tile([15, 1], F32, tag="lnalpha")
            nc.sync.dma_start(out=lnalpha, in_=lna_ext[:, :])
            tago = consts.tile([15, TB3], F16, tag="tago")
            nc.sync.dma_start(out=tago, in_=tago_ext[:, :])

            logits = seqs.tile([15, TB3], F32, tag="logits")

            NT = 64  # t-steps per classifier n-tile (NT*BL3 = 512 cols)
            SHSZ = 128 * 2 * 4 * T  # elements per recv shard

            def comb_ap(kk, half, ns):
                # [128, 4, NT]: dir kk//2, chunk kk%2, half-shard, n-tile ns;
                # always an ascending t window (reversal done in the rhs view)
                d, c = kk // 2, kk % 2
                rev = d in (1, 3)  # c1b, w1b stored time-reversed
                base = (2 * d + half) * SHSZ + c * (4 * T)
                if not rev:
                    off = base + ns * NT
                else:
                    off = base + (T - NT) - ns * NT
                return bass.AP(tensor=recvb, offset=off,
                               ap=[[2 * 4 * T, 128], [T, 4], [1, NT]])

            for ns in range(8):
                comb = stagep.tile([128, 8, BL3, NT], F16, tag="comb", bufs=2)
                for kk in range(8):
                    for half in range(2):
                        nc.sync.dma_start(out=comb[:, kk, ds(4 * half, 4)],
                                          in_=comb_ap(kk, half, ns))
                hmt = []
                for m in range(4):
                    ps = ps_big.tile([128, NT * BL3], F32, tag="gemmps")
                    for kk in range(8):
                        if kk // 2 in (1, 3):
                            p_step = comb.ap[0][0]
                            off = (comb.offset + kk * BL3 * NT + (NT - 1))
                            rhs = bass.AP(tensor=comb.tensor, offset=off,
                                          ap=[[p_step, 128], [-1, NT],
                                              [NT, BL3]])
                        else:
                            rhs = comb[:, kk].rearrange("p b t -> p t b")
                        nc.tensor.matmul(ps, cls1[:, kk, m], rhs,
                                         start=(kk == 0), stop=(kk == 7))
                    hm = stagep.tile([128, NT * BL3], F16, tag="hm", bufs=4,
                                     name=f"hm{m}")
                    nc.scalar.activation(hm, ps, AF.Relu, bias=clsb1[:, m: m + 1])
                    hmt.append(hm)
                ps2 = ps_small.tile([15, NT * BL3], F32, tag="small")
                for m in range(4):
                    nc.tensor.matmul(ps2, cls2[:, m], hmt[m],
                                     start=(m == 0), stop=(m == 3))
                nc.vector.tensor_scalar_add(
                    logits[:, ds(ns * NT * BL3, NT * BL3)], ps2, clsb2)

            # fold CRF start/end into first/last emission columns
            nc.vector.tensor_scalar_add(logits[:, 0:BL3], logits[:, 0:BL3], crfstart)
            nc.vector.tensor_scalar_add(logits[:, TB3 - BL3: TB3],
                                        logits[:, TB3 - BL3: TB3], crfend)

            # ---- CRF numerator ----
            racc = work.tile([15, 16], F32, tag="racc")
            nc.vector.memset(racc, 0.0)
            for ns in range(8):
                pre = stagep.tile([15, 512], F32, tag="prodns")
                nc.vector.tensor_mul(pre, logits[:, ds(ns * 512, 512)],
                                     tago[:, ds(ns * 512, 512)])
                nc.vector.tensor_reduce(racc[:, 8 + ns: 9 + ns], pre,
                                        axis=mybir.AxisListType.X,
                                        op=mybir.AluOpType.add)
                psv = ps_small.tile([15, 512], F32, tag="small")
                nc.tensor.matmul(psv, transn, tago[:, ds(ns * 512, 512)],
                                 start=True, stop=True)
                w = 512 if ns < 7 else 512 - BL3
                pr = stagep.tile([15, 512], F32, tag="prodns")
                nc.vector.tensor_mul(pr[:, :w], psv[:, :w],
                                     tago[:, ds(ns * 512 + BL3, w)])
                nc.vector.tensor_reduce(racc[:, ns: ns + 1], pr[:, :w],
                                        axis=mybir.AxisListType.X,
                                        op=mybir.AluOpType.add)
            nv = stagep.tile([15, 1], F32, tag="nv")
            nc.vector.tensor_reduce(nv, racc, axis=mybir.AxisListType.X,
                                    op=mybir.AluOpType.add)
            ones15 = consts.tile([15, 1], F32, tag="ones15")
            nc.vector.memset(ones15, 1.0)
            psn = ps_small.tile([1, 1], F32, tag="small")
            nc.tensor.matmul(psn, ones15, nv, start=True, stop=True)
            num11 = work.tile([1, 1], F32, tag="num11")
            nc.vector.tensor_copy(num11, psn)

            # ---- CRF forward scan, probability space ----
            # two independent half-batch chains so the 15x15 matmul of one
            # overlaps the vector multiply of the other. p0 first, then
            # E = alpha*exp(logits) in place (last col already has e^end).
            pA = seqs.tile([15, 4], F32, tag="pvecA")
            pB = seqs.tile([15, 4], F32, tag="pvecB")
            nc.scalar.activation(pA, logits[:, 0:4], AF.Exp)
            nc.scalar.activation(pB, logits[:, 4:8], AF.Exp)
            E = logits
            nc.scalar.activation(E, logits, AF.Exp, bias=lnalpha)

            def crf_step(col):
                zA = ps_small.tile([15, 4], F32, tag="small", name="zA")
                nc.tensor.matmul(zA, mexp, pA, start=True, stop=True)
                zB = ps_small.tile([15, 4], F32, tag="small", name="zB")
                nc.tensor.matmul(zB, mexp, pB, start=True, stop=True)
                nc.vector.tensor_mul(pA, zA, E[:, col: col + 4] if isinstance(col, int)
                                     else E[:, ds(col, 4)])
                nc.vector.tensor_mul(pB, zB, E[:, col + 4: col + 8] if isinstance(col, int)
                                     else E[:, ds(col + 4, 4)])

            for t in range(1, 16):
                crf_step(t * BL3)
            with tc.For_i(0, 496, UNROLL) as tv:
                for j in range(UNROLL):
                    crf_step((16 + j) * BL3 + tv * BL3)

            # ---- denominator + output ----
            den11 = work.tile([1, 1], F32, tag="den11")
            for idx, pv in enumerate((pA, pB)):
                psd = ps_small.tile([1, 4], F32, tag="small", name=f"psd{idx}")
                nc.tensor.matmul(psd, ones15, pv, start=True, stop=True)
                ln4 = stagep.tile([1, 4], F32, tag="ln8", name=f"ln4{idx}")
                nc.scalar.activation(ln4, psd, AF.Ln)
                if idx == 0:
                    nc.vector.tensor_reduce(den11, ln4, axis=mybir.AxisListType.X,
                                            op=mybir.AluOpType.add)
                else:
                    dh = work.tile([1, 1], F32, tag="denh")
                    nc.vector.tensor_reduce(dh, ln4, axis=mybir.AxisListType.X,
                                            op=mybir.AluOpType.add)
                    nc.vector.tensor_add(den11, den11, dh)
            res = work.tile([1, 1], F32, tag="res")
            nc.vector.tensor_sub(res, den11, num11)
            nc.sync.dma_start(out=out_ext[:, :], in_=res)

    nc.finalize()
    _BUILD_CACHE["nc"] = nc
    return nc


# ---- host-side input prep ---------------------------------------------------

# gate perm [i(256), f(256), g(256), o(256)] -> [i, f, o, g~]
_GPERM = np.concatenate([np.arange(0, 512), np.arange(768, 1024), np.arange(512, 768)])

# core c -> (pathway, direction, half): 0..3 char f/f/b/b, 4..7 word
_ROLES = [("c", 0, 0), ("c", 0, 1), ("c", 1, 0), ("c", 1, 1),
          ("w", 0, 0), ("w", 0, 1), ("w", 1, 0), ("w", 1, 1)]


def _wih_prep(W, dk_n):
    Wp = W[_GPERM]
    return np.ascontiguousarray(
        Wp.reshape(8, 128, dk_n, 128).transpose(3, 2, 0, 1).reshape(128, dk_n * 8 * 128)
    ).astype(np.float16)


def _make_in_maps(inputs):
    char_ids = np.asarray(inputs["char_ids"])
    tags = np.asarray(inputs["tags"])
    wemb = np.asarray(inputs["word_embeddings"], np.float32)
    emb = np.asarray(inputs["char_emb_table"], np.float32)
    trans = np.asarray(inputs["crf_trans"], np.float32)

    alpha = 1.0 / (15.0 * float(np.exp(trans).mean()))
    common = {}
    w1 = np.asarray(inputs["cls_w1"], np.float32)
    common["cls1"] = np.ascontiguousarray(
        w1.reshape(4, 128, 8, 128).transpose(3, 2, 0, 1).reshape(128, 8 * 4 * 128)
    ).astype(np.float16)
    common["clsb1"] = np.ascontiguousarray(
        np.asarray(inputs["cls_b1"], np.float32).reshape(4, 128).T).astype(np.float32)
    w2 = np.asarray(inputs["cls_w2"], np.float32)
    common["cls2"] = np.ascontiguousarray(
        w2.reshape(15, 4, 128).transpose(2, 1, 0).reshape(128, 4 * 15)).astype(np.float16)
    common["clsb2"] = np.asarray(inputs["cls_b2"], np.float32).reshape(15, 1).copy()
    common["mexp"] = np.exp(trans).astype(np.float32)
    common["transn"] = trans.astype(np.float16)
    common["crfstart"] = np.asarray(inputs["crf_start"], np.float32).reshape(15, 1).copy()
    common["crfend"] = np.asarray(inputs["crf_end"], np.float32).reshape(15, 1).copy()
    common["lnalpha"] = np.full((15, 1), np.log(alpha), np.float32)

    in_maps = []
    for c in range(NC_N):
        pw, d, hf = _ROLES[c]
        lo, hi = hf * BL2, (hf + 1) * BL2
        m = dict(common)

        # phase-1 weights/input
        if pw == "c":
            Wih1 = np.zeros((1024, 768), np.float32)
            Wih1[:, :128] = np.asarray(inputs["c0_Wih"], np.float32)[d]
            Whh1 = np.asarray(inputs["c0_Whh"], np.float32)[d]
            b1 = (np.asarray(inputs["c0_bih"], np.float32)[d]
                  + np.asarray(inputs["c0_bhh"], np.float32)[d])
            ce = emb[char_ids[lo:hi]]  # (32, 512, 128)
            X = np.zeros((128, DK1, T, BL2), np.float32)
            X[:, 0] = ce.transpose(2, 1, 0)
            Wl1 = np.asarray(inputs["c1_Wih"], np.float32)[d]
            Whh2 = np.asarray(inputs["c1_Whh"], np.float32)[d]
            b2 = (np.asarray(inputs["c1_bih"], np.float32)[d]
                  + np.asarray(inputs["c1_bhh"], np.float32)[d])
        else:
            Wih1 = np.asarray(inputs["w0_Wih"], np.float32)[d]
            Whh1 = np.asarray(inputs["w0_Whh"], np.float32)[d]
            b1 = (np.asarray(inputs["w0_bih"], np.float32)[d]
                  + np.asarray(inputs["w0_bhh"], np.float32)[d])
            X = wemb[lo:hi].reshape(BL2, T, DK1, 128).transpose(3, 2, 1, 0)
            Wl1 = np.asarray(inputs["w1_Wih"], np.float32)[d]
            Whh2 = np.asarray(inputs["w1_Whh"], np.float32)[d]
            b2 = (np.asarray(inputs["w1_bih"], np.float32)[d]
                  + np.asarray(inputs["w1_bhh"], np.float32)[d])
        if d == 1:  # backward: reverse local time
            X = X[:, :, ::-1]
        m["x1"] = np.ascontiguousarray(X.reshape(128, DK1, T * BL2)).astype(np.float16)
        m["wih1"] = _wih_prep(Wih1, DK1)
        m["whh1"] = _wih_prep(Whh1, 2)
        m["bias1"] = np.ascontiguousarray(b1[_GPERM].reshape(8, 128).T).astype(np.float32)

        # phase-2 weights: columns [own(256) | peer(256)]
        if d == 1:
            Wl1 = Wl1[:, np.r_[256:512, 0:256]]
        m["wih2"] = _wih_prep(Wl1, DK2)
        m["whh2"] = _wih_prep(Whh2, 2)
        m["bias2"] = np.ascontiguousarray(b2[_GPERM].reshape(8, 128).T).astype(np.float32)
        # blend: f-core (d=0) picks AG region 1 (the b-core), b-core picks 0
        msk = np.zeros((128, 2), np.float32)
        msk[:, 1 - d] = 1.0
        m["masks"] = msk

        # phase-3 tags for this core's 8 sequences
        seqs3 = np.r_[4 * c: 4 * c + 4, 32 + 4 * c: 32 + 4 * c + 4]
        oh = (np.arange(K)[:, None, None] == tags[seqs3][None]).astype(np.float32)
        # (15, 8seq, 512t) -> (15, t, b)
        m["tagoneT"] = np.ascontiguousarray(
            oh.transpose(0, 2, 1).reshape(K, TB3)).astype(np.float16)
        in_maps.append(m)
    return in_maps, alpha


def kernel(**inputs):
    nc = _build_nc()
    in_maps, alpha = _make_in_maps(inputs)
    res = run_bass_kernel_spmd(nc, in_maps, core_ids=list(range(NC_N)))
    total = sum(float(res.results[c]["out"][0, 0]) for c in range(NC_N))
    total -= B * (T - 1) * np.log(alpha)
    return np.float32(total / B)


# revision 22
# speedup vs baseline: 1.1025x; 1.0003x over previous
"""BiLSTM dual-pathway + CRF NLL kernel for 8 Trainium2 NeuronCores.

Sharding: direction-parallel for the LSTM recurrences, data-parallel for the
classifier/CRF. Phase 1 runs the four layer-0 directions (char fwd/bwd, word
fwd/bwd) on 8 cores as (direction x batch-half), batch 32 per core, so the
recurrent matmuls run at N=32 instead of N=8 and the sequential chain count
drops from 4096 to 1024 steps. A pairwise AllGather exchanges the L0 hidden
states between fwd/bwd cores, phase 2 runs the four layer-1 directions the
same way, then an 8-rank AllToAll redistributes hidden states to a
data-parallel layout (8 sequences per core) for the classifier and CRF.

SPMD uniformity: every core runs the identical program. Backward directions
receive host-time-reversed inputs; reversed reads of peer hidden states are
fixed negative-stride APs, with host-permuted weight columns absorbing the
f/b role differences. Peer-region selection after the AllGather uses per-core
0/1 blend masks delivered as input data.

The CRF forward scan runs in probability space with a constant per-step
prescale alpha folded into the emission exponentials (corrected analytically
on the host), so each step is one resident-weight 15x15 matmul plus one
vector multiply -- no per-step exp/ln activation-table swaps.
"""

import sys

sys.path.insert(0, "/opt/trn_rl_repo")

import numpy as np

import concourse.bass as bass
import concourse.mybir as mybir
from concourse import bacc
from concourse.bass import ds
from concourse.tile import TileContext
from concourse.bass_utils import run_bass_kernel_spmd

F16 = mybir.dt.float16
F32 = mybir.dt.float32
AF = mybir.ActivationFunctionType

B, T, V, K = 64, 512, 40, 15
NC_N = 8
BL2 = 32            # batch per core in phases 1-2
BL3 = 8             # sequences per core in phase 3
TB3 = T * BL3       # 4096 classifier/CRF columns per core
DK1, DK2 = 6, 4     # input chunks for L0 (word=768, char padded) and L1 (512)
UNROLL = 16
HCOL = T + 1        # h buffer columns per sequence (col 0 = zero init)

_BUILD_CACHE = {}


def _emit_rec_block(nc, stagep, ps_rec, hc, cst, whh, xgs, xg_dram, tv, fodder,
                    prefetch_tv=None):
    """Emit 16 recurrence steps for time block tv.

    fodder: list of (items, per_step) - closures emitting one PE-side quantum
    each (GEMM matmuls for other pipeline stages), dispensed between the
    step's recurrent matmuls and its vector/scalar chain so they execute in
    the PE's dependency-stall gaps.
    """
    CB = UNROLL * BL2
    cursors = [[items, 0, per_step] for items, per_step in fodder]
    for j in range(UNROLL):
        psA = ps_rec.tile([128, 4, BL2], F32, tag="recpsA")
        psB = ps_rec.tile([128, 4, BL2], F32, tag="recpsB")
        # i,f gates (chunks 0-3) first into psA so the first g-add can
        # start while the o/g~ matmuls (psB) still run
        for m in (0, 1, 2, 3):
            for k in range(2):
                nc.tensor.matmul(psA[:, m], whh[:, k, m],
                                 hc[:, k, j], start=(k == 0), stop=(k == 1))
        for m in (6, 7, 4, 5):
            for k in range(2):
                nc.tensor.matmul(psB[:, m - 4], whh[:, k, m],
                                 hc[:, k, j], start=(k == 0), stop=(k == 1))
        for cur in cursors:
            for _ in range(cur[2]):
                if cur[1] < len(cur[0]):
                    cur[0][cur[1]]()
                    cur[1] += 1
        g = stagep.tile([128, 8, BL2], F32, tag="g")
        xsl = xgs[:, :, ds(j * BL2, BL2)]
        nc.vector.tensor_add(g[:, 0:4], psA, xsl[:, 0:4])
        sig = stagep.tile([128, 6, BL2], F32, tag="sig")
        nc.scalar.activation(sig[:, 0:4], g[:, 0:4], AF.Sigmoid)
        nc.vector.tensor_mul(cst, cst, sig[:, 2:4])
        nc.vector.tensor_add(g[:, 4:8], psB, xsl[:, 4:8])
        tgg = stagep.tile([128, 2, BL2], F32, tag="tgg")
        nc.scalar.activation(tgg, g[:, 6:8], AF.Tanh)
        tmp = stagep.tile([128, 2, BL2], F32, tag="tmpig")
        nc.vector.tensor_mul(tmp, sig[:, 0:2], tgg)
        nc.scalar.activation(sig[:, 4:6], g[:, 4:6], AF.Sigmoid)
        nc.vector.tensor_add(cst, cst, tmp)
        tch = stagep.tile([128, 2, BL2], F32, tag="tch")
        nc.scalar.activation(tch, cst, AF.Tanh)
        nc.vector.tensor_mul(hc[:, :, j + 1], sig[:, 4:6], tch)
    for cur in cursors:
        while cur[1] < len(cur[0]):
            cur[0][cur[1]]()
            cur[1] += 1
    if prefetch_tv is not None:
        # refill this block's (just-consumed) xgs buffer with the same-parity
        # block two ahead -- a full block of DMA lead time
        nc.sync.dma_start(out=xgs, in_=xg_dram[:, :, ds(prefetch_tv * BL2, CB)])


def _build_nc():
    if "nc" in _BUILD_CACHE:
        return _BUILD_CACHE["nc"]
    nc = bacc.Bacc(target_bir_lowering=False, num_devices=NC_N)

    # ---- external parameters -------------------------------------------------
    x1_ext = nc.declare_dram_parameter("x1", [128, DK1, T * BL2], F16, isOutput=False)
    wih1_ext = nc.declare_dram_parameter("wih1", [128, DK1 * 8 * 128], F16, isOutput=False)
    whh1_ext = nc.declare_dram_parameter("whh1", [128, 2 * 8 * 128], F16, isOutput=False)
    bias1_ext = nc.declare_dram_parameter("bias1", [128, 8], F32, isOutput=False)
    wih2_ext = nc.declare_dram_parameter("wih2", [128, DK2 * 8 * 128], F16, isOutput=False)
    whh2_ext = nc.declare_dram_parameter("whh2", [128, 2 * 8 * 128], F16, isOutput=False)
    bias2_ext = nc.declare_dram_parameter("bias2", [128, 8], F32, isOutput=False)
    masks_ext = nc.declare_dram_parameter("masks", [128, 2], F32, isOutput=False)
    cls1_ext = nc.declare_dram_parameter("cls1", [128, 8 * 4 * 128], F16, isOutput=False)
    clsb1_ext = nc.declare_dram_parameter("clsb1", [128, 4], F32, isOutput=False)
    cls2_ext = nc.declare_dram_parameter("cls2", [128, 4 * 15], F16, isOutput=False)
    clsb2_ext = nc.declare_dram_parameter("clsb2", [15, 1], F32, isOutput=False)
    mexp_ext = nc.declare_dram_parameter("mexp", [15, 15], F32, isOutput=False)
    transn_ext = nc.declare_dram_parameter("transn", [15, 15], F16, isOutput=False)
    start_ext = nc.declare_dram_parameter("crfstart", [15, 1], F32, isOutput=False)
    end_ext = nc.declare_dram_parameter("crfend", [15, 1], F32, isOutput=False)
    lna_ext = nc.declare_dram_parameter("lnalpha", [15, 1], F32, isOutput=False)
    tago_ext = nc.declare_dram_parameter("tagoneT", [15, TB3], F16, isOutput=False)
    out_ext = nc.declare_dram_parameter("out", [1, 1], F32, isOutput=True)

    # ---- internal DRAM -------------------------------------------------------
    HSZ = 2 * BL2 * T  # 32768 cols/partition of h (f16)
    xg1_dram = nc.dram_tensor("xg1", [128, 8, T * BL2], F16)
    xg2_dram = nc.dram_tensor("xg2", [128, 8, T * BL2], F16)
    hmine = nc.dram_tensor("hmine", [128, HSZ], F16)
    agout = nc.dram_tensor("agout", [2, 128, HSZ], F16)
    sendb = nc.dram_tensor("sendb", [8, 128, 2 * 4 * T], F16)
    recvb = nc.dram_tensor("recvb", [8, 128, 2 * 4 * T], F16)

    with TileContext(nc) as tc:
        with (
            tc.tile_pool(name="consts", bufs=1) as consts,
            tc.tile_pool(name="seqs", bufs=1) as seqs,
            tc.tile_pool(name="work", bufs=2) as work,
            tc.tile_pool(name="stage", bufs=2) as stagep,
            tc.tile_pool(name="ps_big", bufs=3, space="PSUM") as ps_big,
            tc.tile_pool(name="ps_rec", bufs=1, space="PSUM") as ps_rec,
            tc.tile_pool(name="ps_small", bufs=3, space="PSUM") as ps_small,
        ):
            # h buffer, seq-major: [128, chunk2, b32, T+1], reused by phases 1+2
            h_sb = seqs.tile([128, 2, BL2, HCOL], F16, tag="h_sb")
            hc = seqs.tile([128, 2, UNROLL + 1, BL2], F16, tag="hcomp")
            cst = seqs.tile([128, 2, BL2], F32, tag="cstate")
            masks = consts.tile([128, 2], F32, tag="masks")
            nc.sync.dma_start(out=masks, in_=masks_ext[:, :])

            # ================= PHASE 1 (L0) =================
            wih1 = consts.tile([128, DK1, 8, 128], F16, tag="wbig")
            nc.sync.dma_start(
                out=wih1,
                in_=wih1_ext.ap().rearrange("p (k m c) -> p k m c", k=DK1, m=8))
            whh1 = consts.tile([128, 2, 8, 128], F16, tag="whh")
            nc.sync.dma_start(
                out=whh1,
                in_=whh1_ext.ap().rearrange("p (k m c) -> p k m c", k=2, m=8))
            bias1 = consts.tile([128, 8], F32, tag="bias1")
            nc.sync.dma_start(out=bias1, in_=bias1_ext[:, :])
            # wih2/bias2 load now: the L1 input GEMM's slotA half runs
            # interleaved inside the phase-1 recurrence
            wih2 = consts.tile([128, DK2, 8, 128], F16, tag="wih2")
            nc.sync.dma_start(
                out=wih2,
                in_=wih2_ext.ap().rearrange("p (k m c) -> p k m c", k=DK2, m=8))
            bias2 = consts.tile([128, 8], F32, tag="bias2")
            nc.sync.dma_start(out=bias2, in_=bias2_ext[:, :])

            CB = UNROLL * BL2
            # phase-1 h layout in DRAM is t-major: (p, c, t, b)
            hm_view = hmine.ap().rearrange("p (c t b) -> p c t b", c=2, t=T)

            def g1_items(tv_lead):
                # xg1 block tv_lead: input DMA now, 48 matmul quanta
                xb = stagep.tile([128, DK1, CB], F16, tag="xb1")
                nc.sync.dma_start(out=xb, in_=x1_ext[:, :, ds(tv_lead * BL2, CB)])
                items, pss = [], {}
                for m in range(8):
                    for k in range(DK1):
                        def it(m=m, k=k):
                            if k == 0:
                                pss[m] = ps_big.tile([128, CB], F32, tag="gemmps",
                                                     name=f"g1ps{m}")
                            nc.tensor.matmul(pss[m], wih1[:, k, m], xb[:, k],
                                             start=(k == 0), stop=(k == DK1 - 1))
                            if k == DK1 - 1:
                                st = stagep.tile([128, CB], F16, tag="xgstage",
                                                 name=f"g1st{m}")
                                nc.scalar.activation(st, pss[m], AF.Identity,
                                                     bias=bias1[:, m: m + 1])
                                nc.sync.dma_start(
                                    out=xg1_dram[:, m, ds(tv_lead * BL2, CB)], in_=st)
                        items.append(it)
                return items

            def p1_end(tv):
                nc.sync.dma_start(out=hm_view[:, :, ds(tv, UNROLL)],
                                  in_=hc[:, :, 1: UNROLL + 1])
                nc.gpsimd.tensor_copy(hc[:, :, 0], hc[:, :, UNROLL])

            xgsA = stagep.tile([128, 8, CB], F16, tag="xgs", name="xgsA")
            xgsB = stagep.tile([128, 8, CB], F16, tag="xgs", name="xgsB")

            def rec1_block(tv, fodder, xgs, pf):
                _emit_rec_block(nc, stagep, ps_rec, hc, cst, whh1, xgs,
                                xg1_dram, tv, fodder, prefetch_tv=pf)
                p1_end(tv)

            nc.vector.memset(cst, 0.0)
            nc.vector.memset(hc[:, :, 0], 0.0)
            for blk in range(2):
                for it in g1_items(16 * blk):
                    it()
            nc.sync.dma_start(out=xgsA, in_=xg1_dram[:, :, ds(0, CB)])
            nc.sync.dma_start(out=xgsB, in_=xg1_dram[:, :, ds(16 * BL2, CB)])
            rec1_block(0, [(g1_items(32), 3)], xgsA, 32)
            rec1_block(16, [(g1_items(48), 3)], xgsB, 48)
            with tc.For_i(32, 480, 32) as tv:
                itemsA = g1_items(tv + 32)
                itemsB = g1_items(tv + 48)  # DMA issued early: a block of lead
                _emit_rec_block(nc, stagep, ps_rec, hc, cst, whh1, xgsA,
                                xg1_dram, tv, [(itemsA, 3)], prefetch_tv=tv + 32)
                p1_end(tv)
                _emit_rec_block(nc, stagep, ps_rec, hc, cst, whh1, xgsB,
                                xg1_dram, tv + 16, [(itemsB, 3)],
                                prefetch_tv=tv + 48)
                p1_end(tv + 16)
            rec1_block(480, [], xgsA, None)
            rec1_block(496, [], xgsB, None)

            nc.gpsimd.collective_compute(
                "AllGather", mybir.AluOpType.bypass,
                replica_groups=[[0, 2], [1, 3], [4, 6], [5, 7]],
                ins=[hmine.ap()], outs=[agout.ap()],
            )

            # ================= PHASE 2 (L1) =================
            whh2 = consts.tile([128, 2, 8, 128], F16, tag="whh")
            nc.sync.dma_start(
                out=whh2,
                in_=whh2_ext.ap().rearrange("p (k m c) -> p k m c", k=2, m=8))

            def slot_ap(tensor_offset, tensor, c, tv, reverse):
                # [128, UNROLL*32] chunk c of a t-major h DRAM region
                # (p, c2, T, b32): a contiguous ascending t window; reversed
                # slots flip t on the SBUF side.
                if not reverse:
                    w = tv
                else:
                    w = (T - UNROLL) - tv
                off = tensor_offset + c * (T * BL2) + w * BL2
                return bass.AP(tensor=tensor, offset=off,
                               ap=[[HSZ, 128], [1, UNROLL * BL2]])

            def g2_items(tv_lead):
                # full xg2 block: slotA = own h fwd, slotB = peer h reversed
                sa = stagep.tile([128, 2, UNROLL, BL2], F16, tag="slotA")
                r0 = stagep.tile([128, 2, UNROLL, BL2], F16, tag="slotR0")
                r1 = stagep.tile([128, 2, UNROLL, BL2], F16, tag="slotR1")
                for c in range(2):
                    nc.sync.dma_start(out=sa[:, c],
                                      in_=slot_ap(0, hmine, c, tv_lead, False))
                    nc.sync.dma_start(out=r0[:, c],
                                      in_=slot_ap(0, agout, c, tv_lead, True))
                    nc.sync.dma_start(out=r1[:, c],
                                      in_=slot_ap(128 * HSZ, agout, c, tv_lead, True))
                sbr = stagep.tile([128, 2, UNROLL, BL2], F16, tag="slotBr")
                items, pss = [], {}

                def blend0():
                    nc.vector.tensor_scalar_mul(r0, r0, masks[:, 0:1])

                def blend1():
                    nc.vector.tensor_scalar_mul(r1, r1, masks[:, 1:2])

                def blend2():
                    # add, writing with the t window reversed
                    p_step = sbr.ap[0][0]
                    rev = bass.AP(tensor=sbr.tensor,
                                  offset=sbr.offset + (UNROLL - 1) * BL2,
                                  ap=[[p_step, 128], [UNROLL * BL2, 2],
                                      [-BL2, UNROLL], [1, BL2]])
                    nc.vector.tensor_add(rev, r0, r1)

                items += [blend0, blend1, blend2]

                def src_k(k):
                    t = sa if k < 2 else sbr
                    return t[:, k % 2].rearrange("p t b -> p (t b)")

                for m in range(8):
                    for k in range(DK2):
                        def it(m=m, k=k):
                            if k == 0:
                                pss[m] = ps_big.tile([128, CB], F32, tag="gemmps",
                                                     name=f"g2ps{m}")
                            nc.tensor.matmul(pss[m], wih2[:, k, m], src_k(k),
                                             start=(k == 0), stop=(k == DK2 - 1))
                            if k == DK2 - 1:
                                st = stagep.tile([128, CB], F16, tag="xgstage",
                                                 name=f"g2st{m}")
                                nc.scalar.activation(st, pss[m], AF.Identity,
                                                     bias=bias2[:, m: m + 1])
                                nc.sync.dma_start(
                                    out=xg2_dram[:, m, ds(tv_lead * BL2, CB)], in_=st)
                        items.append(it)
                return items

            def p2_end(tv):
                nc.gpsimd.tensor_copy(
                    h_sb[:, :, :, ds(tv + 1, UNROLL)],
                    hc[:, :, 1: UNROLL + 1].rearrange("p c t b -> p c b t"))
                nc.gpsimd.tensor_copy(hc[:, :, 0], hc[:, :, UNROLL])

            nc.vector.memset(cst, 0.0)
            nc.vector.memset(hc[:, :, 0], 0.0)
            for blk in (0, 1):
                for it in g2_items(16 * blk):
                    it()
            nc.sync.dma_start(out=xgsA, in_=xg2_dram[:, :, ds(0, CB)])
            nc.sync.dma_start(out=xgsB, in_=xg2_dram[:, :, ds(16 * BL2, CB)])
            with tc.For_i(0, 480, 32) as tv:
                itemsA = g2_items(tv + 32)
                itemsB = g2_items(tv + 48)  # slot DMAs issued early
                _emit_rec_block(nc, stagep, ps_rec, hc, cst, whh2, xgsA,
                                xg2_dram, tv, [(itemsA, 3)], prefetch_tv=tv + 32)
                p2_end(tv)
                _emit_rec_block(nc, stagep, ps_rec, hc, cst, whh2, xgsB,
                                xg2_dram, tv + 16, [(itemsB, 3)],
                                prefetch_tv=tv + 48)
                p2_end(tv + 16)
            _emit_rec_block(nc, stagep, ps_rec, hc, cst, whh2, xgsA,
                            xg2_dram, 480, [])
            p2_end(480)
            _emit_rec_block(nc, stagep, ps_rec, hc, cst, whh2, xgsB,
                            xg2_dram, 496, [])
            p2_end(496)

            # ---- AllToAll redistribution to data-parallel layout ----
            sb_view = sendb.ap().rearrange("j p (c b t) -> j p c b t", c=2, b=4)
            for j in range(8):
                for c in range(2):
                    nc.sync.dma_start(out=sb_view[j, :, c],
                                      in_=h_sb[:, c, ds(4 * j, 4), 1:HCOL])
            nc.gpsimd.collective_compute(
                "AllToAll", mybir.AluOpType.bypass,
                replica_groups=[list(range(8))],
                ins=[sendb.ap()], outs=[recvb.ap()],
            )

            # ================= PHASE 3: classifier + CRF =================
            cls1 = consts.tile([128, 8, 4, 128], F16, tag="wbig")
            nc.sync.dma_start(
                out=cls1,
                in_=cls1_ext.ap().rearrange("p (k m c) -> p k m c", k=8, m=4))
            clsb1 = consts.tile([128, 4], F32, tag="clsb1")
            nc.sync.dma_start(out=clsb1, in_=clsb1_ext[:, :])
            cls2 = consts.tile([128, 4, 15], F16, tag="cls2")
            nc.sync.dma_start(
                out=cls2, in_=cls2_ext.ap().rearrange("p (k j) -> p k j", k=4))
            clsb2 = consts.tile([15, 1], F32, tag="clsb2")
            nc.sync.dma_start(out=clsb2, in_=clsb2_ext[:, :])
            mexp = consts.tile([15, 15], F32, tag="mexp")
            nc.sync.dma_start(out=mexp, in_=mexp_ext[:, :])
            transn = consts.tile([15, 15], F16, tag="transn")
            nc.sync.dma_start(out=transn, in_=transn_ext[:, :])
            crfstart = consts.tile([15, 1], F32, tag="crfstart")
            nc.sync.dma_start(out=crfstart, in_=start_ext[:, :])
            crfend = consts.tile([15, 1], F32, tag="crfend")
            nc.sync.dma_start(out=crfend, in_=end_ext[:, :])
            lnalpha = consts.tile([15, 1], F32, tag="lnalpha")
            nc.sync.dma_start(out=lnalpha, in_=lna_ext[:, :])
            tago = consts.tile([15, TB3], F16, tag="tago")
            nc.sync.dma_start(out=tago, in_=tago_ext[:, :])

            logits = seqs.tile([15, TB3], F32, tag="logits")

            NT = 64  # t-steps per classifier n-tile (NT*BL3 = 512 cols)
            SHSZ = 128 * 2 * 4 * T  # elements per recv shard

            def comb_ap(kk, half, ns):
                # [128, 4, NT]: dir kk//2, chunk kk%2, half-shard, n-tile ns;
                # always an ascending t window (reversal done in the rhs view)
                d, c = kk // 2, kk % 2
                rev = d in (1, 3)  # c1b, w1b stored time-reversed
                base = (2 * d + half) * SHSZ + c * (4 * T)
                if not rev:
                    off = base + ns * NT
                else:
                    off = base + (T - NT) - ns * NT
                return bass.AP(tensor=recvb, offset=off,
                               ap=[[2 * 4 * T, 128], [T, 4], [1, NT]])

            for ns in range(8):
                comb = stagep.tile([128, 8, BL3, NT], F16, tag="comb", bufs=2)
                for kk in range(8):
                    for half in range(2):
                        nc.sync.dma_start(out=comb[:, kk, ds(4 * half, 4)],
                                          in_=comb_ap(kk, half, ns))
                hmt = []
                for m in range(4):
                    ps = ps_big.tile([128, NT * BL3], F32, tag="gemmps")
                    for kk in range(8):
                        if kk // 2 in (1, 3):
                            p_step = comb.ap[0][0]
                            off = (comb.offset + kk * BL3 * NT + (NT - 1))
                            rhs = bass.AP(tensor=comb.tensor, offset=off,
                                          ap=[[p_step, 128], [-1, NT],
                                              [NT, BL3]])
                        else:
                            rhs = comb[:, kk].rearrange("p b t -> p t b")
                        nc.tensor.matmul(ps, cls1[:, kk, m], rhs,
                                         start=(kk == 0), stop=(kk == 7))
                    hm = stagep.tile([128, NT * BL3], F16, tag="hm", bufs=4,
                                     name=f"hm{m}")
                    nc.scalar.activation(hm, ps, AF.Relu, bias=clsb1[:, m: m + 1])
                    hmt.append(hm)
                ps2 = ps_small.tile([15, NT * BL3], F32, tag="small")
                for m in range(4):
                    nc.tensor.matmul(ps2, cls2[:, m], hmt[m],
                                     start=(m == 0), stop=(m == 3))
                nc.vector.tensor_scalar_add(
                    logits[:, ds(ns * NT * BL3, NT * BL3)], ps2, clsb2)

            # fold CRF start/end into first/last emission columns
            nc.vector.tensor_scalar_add(logits[:, 0:BL3], logits[:, 0:BL3], crfstart)
            nc.vector.tensor_scalar_add(logits[:, TB3 - BL3: TB3],
                                        logits[:, TB3 - BL3: TB3], crfend)

            # ---- CRF numerator ----
            racc = work.tile([15, 16], F32, tag="racc")
            nc.vector.memset(racc, 0.0)
            for ns in range(8):
                pre = stagep.tile([15, 512], F32, tag="prodns")
                nc.vector.tensor_mul(pre, logits[:, ds(ns * 512, 512)],
                                     tago[:, ds(ns * 512, 512)])
                nc.vector.tensor_reduce(racc[:, 8 + ns: 9 + ns], pre,
                                        axis=mybir.AxisListType.X,
                                        op=mybir.AluOpType.add)
                psv = ps_small.tile([15, 512], F32, tag="small")
                nc.tensor.matmul(psv, transn, tago[:, ds(ns * 512, 512)],
                                 start=True, stop=True)
                w = 512 if ns < 7 else 512 - BL3
                pr = stagep.tile([15, 512], F32, tag="prodns")
                nc.vector.tensor_mul(pr[:, :w], psv[:, :w],
                                     tago[:, ds(ns * 512 + BL3, w)])
                nc.vector.tensor_reduce(racc[:, ns: ns + 1], pr[:, :w],
                                        axis=mybir.AxisListType.X,
                                        op=mybir.AluOpType.add)
            nv = stagep.tile([15, 1], F32, tag="nv")
            nc.vector.tensor_reduce(nv, racc, axis=mybir.AxisListType.X,
                                    op=mybir.AluOpType.add)
            ones15 = consts.tile([15, 1], F32, tag="ones15")
            nc.vector.memset(ones15, 1.0)
            psn = ps_small.tile([1, 1], F32, tag="small")
            nc.tensor.matmul(psn, ones15, nv, start=True, stop=True)
            num11 = work.tile([1, 1], F32, tag="num11")
            nc.vector.tensor_copy(num11, psn)

            # ---- CRF forward scan, probability space ----
            # two independent half-batch chains so the 15x15 matmul of one
            # overlaps the vector multiply of the other. p0 first, then
            # E = alpha*exp(logits) in place (last col already has e^end).
            pA = seqs.tile([15, 4], F32, tag="pvecA")
            pB = seqs.tile([15, 4], F32, tag="pvecB")
            nc.scalar.activation(pA, logits[:, 0:4], AF.Exp)
            nc.scalar.activation(pB, logits[:, 4:8], AF.Exp)
            E = logits
            nc.scalar.activation(E, logits, AF.Exp, bias=lnalpha)

            def crf_step(col):
                zA = ps_small.tile([15, 4], F32, tag="small", name="zA")
                nc.tensor.matmul(zA, mexp, pA, start=True, stop=True)
                zB = ps_small.tile([15, 4], F32, tag="small", name="zB")
                nc.tensor.matmul(zB, mexp, pB, start=True, stop=True)
                nc.vector.tensor_mul(pA, zA, E[:, col: col + 4] if isinstance(col, int)
                                     else E[:, ds(col, 4)])
                nc.vector.tensor_mul(pB, zB, E[:, col + 4: col + 8] if isinstance(col, int)
                                     else E[:, ds(col + 4, 4)])

            for t in range(1, 16):
                crf_step(t * BL3)
            with tc.For_i(0, 496, UNROLL) as tv:
                for j in range(UNROLL):
                    crf_step((16 + j) * BL3 + tv * BL3)

            # ---- denominator + output ----
            den11 = work.tile([1, 1], F32, tag="den11")
            for idx, pv in enumerate((pA, pB)):
                psd = ps_small.tile([1, 4], F32, tag="small", name=f"psd{idx}")
                nc.tensor.matmul(psd, ones15, pv, start=True, stop=True)
                ln4 = stagep.tile([1, 4], F32, tag="ln8", name=f"ln4{idx}")
                nc.scalar.activation(ln4, psd, AF.Ln)
                if idx == 0:
                    nc.vector.tensor_reduce(den11, ln4, axis=mybir.AxisListType.X,
                                            op=mybir.AluOpType.add)
                else:
                    dh = work.tile([1, 1], F32, tag="denh")
                    nc.vector.tensor_reduce(dh, ln4, axis=mybir.AxisListType.X,
                                            op=mybir.AluOpType.add)
                    nc.vector.tensor_add(den11, den11, dh)
            res = work.tile([1, 1], F32, tag="res")
            nc.vector.tensor_sub(res, den11, num11)
            nc.sync.dma_start(out=out_ext[:, :], in_=res)

    nc.finalize()
    _BUILD_CACHE["nc"] = nc
    return nc


# ---- host-side input prep ---------------------------------------------------

# gate perm [i(256), f(256), g(256), o(256)] -> [i, f, o, g~]
_GPERM = np.concatenate([np.arange(0, 512), np.arange(768, 1024), np.arange(512, 768)])

# core c -> (pathway, direction, half): 0..3 char f/f/b/b, 4..7 word
_ROLES = [("c", 0, 0), ("c", 0, 1), ("c", 1, 0), ("c", 1, 1),
          ("w", 0, 0), ("w", 0, 1), ("w", 1, 0), ("w", 1, 1)]


def _wih_prep(W, dk_n):
    Wp = W[_GPERM]
    return np.ascontiguousarray(
        Wp.reshape(8, 128, dk_n, 128).transpose(3, 2, 0, 1).reshape(128, dk_n * 8 * 128)
    ).astype(np.float16)


def _make_in_maps(inputs):
    char_ids = np.asarray(inputs["char_ids"])
    tags = np.asarray(inputs["tags"])
    wemb = np.asarray(inputs["word_embeddings"], np.float32)
    emb = np.asarray(inputs["char_emb_table"], np.float32)
    trans = np.asarray(inputs["crf_trans"], np.float32)

    alpha = 1.0 / (15.0 * float(np.exp(trans).mean()))
    common = {}
    w1 = np.asarray(inputs["cls_w1"], np.float32)
    common["cls1"] = np.ascontiguousarray(
        w1.reshape(4, 128, 8, 128).transpose(3, 2, 0, 1).reshape(128, 8 * 4 * 128)
    ).astype(np.float16)
    common["clsb1"] = np.ascontiguousarray(
        np.asarray(inputs["cls_b1"], np.float32).reshape(4, 128).T).astype(np.float32)
    w2 = np.asarray(inputs["cls_w2"], np.float32)
    common["cls2"] = np.ascontiguousarray(
        w2.reshape(15, 4, 128).transpose(2, 1, 0).reshape(128, 4 * 15)).astype(np.float16)
    common["clsb2"] = np.asarray(inputs["cls_b2"], np.float32).reshape(15, 1).copy()
    common["mexp"] = np.exp(trans).astype(np.float32)
    common["transn"] = trans.astype(np.float16)
    common["crfstart"] = np.asarray(inputs["crf_start"], np.float32).reshape(15, 1).copy()
    common["crfend"] = np.asarray(inputs["crf_end"], np.float32).reshape(15, 1).copy()
    common["lnalpha"] = np.full((15, 1), np.log(alpha), np.float32)

    in_maps = []
    for c in range(NC_N):
        pw, d, hf = _ROLES[c]
        lo, hi = hf * BL2, (hf + 1) * BL2
        m = dict(common)

        # phase-1 weights/input
        if pw == "c":
            Wih1 = np.zeros((1024, 768), np.float32)
            Wih1[:, :128] = np.asarray(inputs["c0_Wih"], np.float32)[d]
            Whh1 = np.asarray(inputs["c0_Whh"], np.float32)[d]
            b1 = (np.asarray(inputs["c0_bih"], np.float32)[d]
                  + np.asarray(inputs["c0_bhh"], np.float32)[d])
            ce = emb[char_ids[lo:hi]]  # (32, 512, 128)
            X = np.zeros((128, DK1, T, BL2), np.float32)
            X[:, 0] = ce.transpose(2, 1, 0)
            Wl1 = np.asarray(inputs["c1_Wih"], np.float32)[d]
            Whh2 = np.asarray(inputs["c1_Whh"], np.float32)[d]
            b2 = (np.asarray(inputs["c1_bih"], np.float32)[d]
                  + np.asarray(inputs["c1_bhh"], np.float32)[d])
        else:
            Wih1 = np.asarray(inputs["w0_Wih"], np.float32)[d]
            Whh1 = np.asarray(inputs["w0_Whh"], np.float32)[d]
            b1 = (np.asarray(inputs["w0_bih"], np.float32)[d]
                  + np.asarray(inputs["w0_bhh"], np.float32)[d])
            X = wemb[lo:hi].reshape(BL2, T, DK1, 128).transpose(3, 2, 1, 0)
            Wl1 = np.asarray(inputs["w1_Wih"], np.float32)[d]
            Whh2 = np.asarray(inputs["w1_Whh"], np.float32)[d]
            b2 = (np.asarray(inputs["w1_bih"], np.float32)[d]
                  + np.asarray(inputs["w1_bhh"], np.float32)[d])
        if d == 1:  # backward: reverse local time
            X = X[:, :, ::-1]
        m["x1"] = np.ascontiguousarray(X.reshape(128, DK1, T * BL2)).astype(np.float16)
        m["wih1"] = _wih_prep(Wih1, DK1)
        m["whh1"] = _wih_prep(Whh1, 2)
        m["bias1"] = np.ascontiguousarray(b1[_GPERM].reshape(8, 128).T).astype(np.float32)

        # phase-2 weights: columns [own(256) | peer(256)]
        if d == 1:
            Wl1 = Wl1[:, np.r_[256:512, 0:256]]
        m["wih2"] = _wih_prep(Wl1, DK2)
        m["whh2"] = _wih_prep(Whh2, 2)
        m["bias2"] = np.ascontiguousarray(b2[_GPERM].reshape(8, 128).T).astype(np.float32)
        # blend: f-core (d=0) picks AG region 1 (the b-core), b-core picks 0
        msk = np.zeros((128, 2), np.float32)
        msk[:, 1 - d] = 1.0
        m["masks"] = msk

        # phase-3 tags for this core's 8 sequences
        seqs3 = np.r_[4 * c: 4 * c + 4, 32 + 4 * c: 32 + 4 * c + 4]
        oh = (np.arange(K)[:, None, None] == tags[seqs3][None]).astype(np.float32)
        # (15, 8seq, 512t) -> (15, t, b)
        m["tagoneT"] = np.ascontiguousarray(
            oh.transpose(0, 2, 1).reshape(K, TB3)).astype(np.float16)
        in_maps.append(m)
    return in_maps, alpha


def kernel(**inputs):
    nc = _build_nc()
    in_maps, alpha = _make_in_maps(inputs)
    res = run_bass_kernel_spmd(nc, in_maps, core_ids=list(range(NC_N)))
    total = sum(float(res.results[c]["out"][0, 0]) for c in range(NC_N))
    total -= B * (T - 1) * np.log(alpha)
    return np.float32(total / B)


# revision 23
# speedup vs baseline: 1.1261x; 1.0214x over previous
"""BiLSTM dual-pathway + CRF NLL kernel for 8 Trainium2 NeuronCores.

Sharding: direction-parallel for the LSTM recurrences, data-parallel for the
classifier/CRF. Phase 1 runs the four layer-0 directions (char fwd/bwd, word
fwd/bwd) on 8 cores as (direction x batch-half), batch 32 per core, so the
recurrent matmuls run at N=32 instead of N=8 and the sequential chain count
drops from 4096 to 1024 steps. A pairwise AllGather exchanges the L0 hidden
states between fwd/bwd cores, phase 2 runs the four layer-1 directions the
same way, then an 8-rank AllToAll redistributes hidden states to a
data-parallel layout (8 sequences per core) for the classifier and CRF.

SPMD uniformity: every core runs the identical program. Backward directions
receive host-time-reversed inputs; reversed reads of peer hidden states are
fixed negative-stride APs, with host-permuted weight columns absorbing the
f/b role differences. Peer-region selection after the AllGather uses per-core
0/1 blend masks delivered as input data.

The CRF forward scan runs in probability space with a constant per-step
prescale alpha folded into the emission exponentials (corrected analytically
on the host), so each step is one resident-weight 15x15 matmul plus one
vector multiply -- no per-step exp/ln activation-table swaps.
"""

import sys

sys.path.insert(0, "/opt/trn_rl_repo")

import numpy as np

import concourse.bass as bass
import concourse.mybir as mybir
from concourse import bacc
from concourse.bass import ds
from concourse.tile import TileContext
from concourse.bass_utils import run_bass_kernel_spmd

F16 = mybir.dt.float16
F32 = mybir.dt.float32
AF = mybir.ActivationFunctionType

B, T, V, K = 64, 512, 40, 15
NC_N = 8
BL2 = 32            # batch per core in phases 1-2
BL3 = 8             # sequences per core in phase 3
TB3 = T * BL3       # 4096 classifier/CRF columns per core
DK1, DK2 = 6, 4     # input chunks for L0 (word=768, char padded) and L1 (512)
UNROLL = 16
HCOL = T + 1        # h buffer columns per sequence (col 0 = zero init)

_BUILD_CACHE = {}


def _emit_rec_block(nc, stagep, ps_rec, hc, cst, whh, xgs, xg_dram, tv, fodder,
                    prefetch_tv=None):
    """Emit 16 recurrence steps for time block tv.

    fodder: list of (items, per_step) - closures emitting one PE-side quantum
    each (GEMM matmuls for other pipeline stages), dispensed between the
    step's recurrent matmuls and its vector/scalar chain so they execute in
    the PE's dependency-stall gaps.
    """
    CB = UNROLL * BL2
    cursors = [[items, 0, per_step] for items, per_step in fodder]
    for j in range(UNROLL):
        psA = ps_rec.tile([128, 4, BL2], F32, tag="recpsA")
        psB = ps_rec.tile([128, 4, BL2], F32, tag="recpsB")
        # i,f gates (chunks 0-3) first into psA so the first g-add can
        # start while the o/g~ matmuls (psB) still run
        for m in (0, 1, 2, 3):
            for k in range(2):
                nc.tensor.matmul(psA[:, m], whh[:, k, m],
                                 hc[:, k, j], start=(k == 0), stop=(k == 1))
        for m in (6, 7, 4, 5):
            for k in range(2):
                nc.tensor.matmul(psB[:, m - 4], whh[:, k, m],
                                 hc[:, k, j], start=(k == 0), stop=(k == 1))
        for cur in cursors:
            for _ in range(cur[2]):
                if cur[1] < len(cur[0]):
                    cur[0][cur[1]]()
                    cur[1] += 1
        g = stagep.tile([128, 8, BL2], F32, tag="g")
        xsl = xgs[:, :, ds(j * BL2, BL2)]
        nc.vector.tensor_add(g[:, 0:4], psA, xsl[:, 0:4])
        sig = stagep.tile([128, 6, BL2], F32, tag="sig")
        nc.scalar.activation(sig[:, 0:4], g[:, 0:4], AF.Sigmoid)
        nc.vector.tensor_mul(cst, cst, sig[:, 2:4])
        nc.vector.tensor_add(g[:, 4:8], psB, xsl[:, 4:8])
        tgg = stagep.tile([128, 2, BL2], F32, tag="tgg")
        nc.scalar.activation(tgg, g[:, 6:8], AF.Tanh)
        tmp = stagep.tile([128, 2, BL2], F32, tag="tmpig")
        nc.vector.tensor_mul(tmp, sig[:, 0:2], tgg)
        nc.scalar.activation(sig[:, 4:6], g[:, 4:6], AF.Sigmoid)
        nc.vector.tensor_add(cst, cst, tmp)
        tch = stagep.tile([128, 2, BL2], F32, tag="tch")
        nc.scalar.activation(tch, cst, AF.Tanh)
        nc.vector.tensor_mul(hc[:, :, j + 1], sig[:, 4:6], tch)
    for cur in cursors:
        while cur[1] < len(cur[0]):
            cur[0][cur[1]]()
            cur[1] += 1
    if prefetch_tv is not None:
        # refill this block's (just-consumed) xgs buffer with the same-parity
        # block two ahead -- a full block of DMA lead time
        nc.sync.dma_start(out=xgs, in_=xg_dram[:, :, ds(prefetch_tv * BL2, CB)])


def _build_nc():
    if "nc" in _BUILD_CACHE:
        return _BUILD_CACHE["nc"]
    nc = bacc.Bacc(target_bir_lowering=False, num_devices=NC_N)

    # ---- external parameters -------------------------------------------------
    x1_ext = nc.declare_dram_parameter("x1", [128, DK1, T * BL2], F16, isOutput=False)
    wih1_ext = nc.declare_dram_parameter("wih1", [128, DK1 * 8 * 128], F16, isOutput=False)
    whh1_ext = nc.declare_dram_parameter("whh1", [128, 2 * 8 * 128], F16, isOutput=False)
    bias1_ext = nc.declare_dram_parameter("bias1", [128, 8], F32, isOutput=False)
    wih2_ext = nc.declare_dram_parameter("wih2", [128, DK2 * 8 * 128], F16, isOutput=False)
    whh2_ext = nc.declare_dram_parameter("whh2", [128, 2 * 8 * 128], F16, isOutput=False)
    bias2_ext = nc.declare_dram_parameter("bias2", [128, 8], F32, isOutput=False)
    masks_ext = nc.declare_dram_parameter("masks", [128, 2], F32, isOutput=False)
    cls1_ext = nc.declare_dram_parameter("cls1", [128, 8 * 4 * 128], F16, isOutput=False)
    clsb1_ext = nc.declare_dram_parameter("clsb1", [128, 4], F32, isOutput=False)
    cls2_ext = nc.declare_dram_parameter("cls2", [128, 4 * 15], F16, isOutput=False)
    clsb2_ext = nc.declare_dram_parameter("clsb2", [15, 1], F32, isOutput=False)
    mexp_ext = nc.declare_dram_parameter("mexp", [15, 15], F32, isOutput=False)
    transn_ext = nc.declare_dram_parameter("transn", [15, 15], F16, isOutput=False)
    start_ext = nc.declare_dram_parameter("crfstart", [15, 1], F32, isOutput=False)
    end_ext = nc.declare_dram_parameter("crfend", [15, 1], F32, isOutput=False)
    lna_ext = nc.declare_dram_parameter("lnalpha", [15, 1], F32, isOutput=False)
    tago_ext = nc.declare_dram_parameter("tagoneT", [15, TB3], F16, isOutput=False)
    out_ext = nc.declare_dram_parameter("out", [1, 1], F32, isOutput=True)

    # ---- internal DRAM -------------------------------------------------------
    HSZ = 2 * BL2 * T  # 32768 cols/partition of h (f16)
    xg1_dram = nc.dram_tensor("xg1", [128, 8, T * BL2], F16)
    xg2_dram = nc.dram_tensor("xg2", [128, 8, T * BL2], F16)
    HSZH = HSZ // 2  # per-partition elements of a t-half of h
    hminA = nc.dram_tensor("hminA", [128, HSZH], F16)
    hminB = nc.dram_tensor("hminB", [128, HSZH], F16)
    agoutA = nc.dram_tensor("agoutA", [2, 128, HSZH], F16)
    agoutB = nc.dram_tensor("agoutB", [2, 128, HSZH], F16)
    sendb = nc.dram_tensor("sendb", [8, 128, 2 * 4 * T], F16)
    recvb = nc.dram_tensor("recvb", [8, 128, 2 * 4 * T], F16)

    with TileContext(nc) as tc:
        with (
            tc.tile_pool(name="consts", bufs=1) as consts,
            tc.tile_pool(name="seqs", bufs=1) as seqs,
            tc.tile_pool(name="work", bufs=2) as work,
            tc.tile_pool(name="stage", bufs=2) as stagep,
            tc.tile_pool(name="ps_big", bufs=3, space="PSUM") as ps_big,
            tc.tile_pool(name="ps_rec", bufs=1, space="PSUM") as ps_rec,
            tc.tile_pool(name="ps_small", bufs=3, space="PSUM") as ps_small,
        ):
            # h buffer, seq-major: [128, chunk2, b32, T+1], reused by phases 1+2
            h_sb = seqs.tile([128, 2, BL2, HCOL], F16, tag="h_sb")
            hc = seqs.tile([128, 2, UNROLL + 1, BL2], F16, tag="hcomp")
            cst = seqs.tile([128, 2, BL2], F32, tag="cstate")
            masks = consts.tile([128, 2], F32, tag="masks")
            nc.sync.dma_start(out=masks, in_=masks_ext[:, :])

            # ================= PHASE 1 (L0) =================
            wih1 = consts.tile([128, DK1, 8, 128], F16, tag="wbig")
            nc.sync.dma_start(
                out=wih1,
                in_=wih1_ext.ap().rearrange("p (k m c) -> p k m c", k=DK1, m=8))
            whh1 = consts.tile([128, 2, 8, 128], F16, tag="whh")
            nc.sync.dma_start(
                out=whh1,
                in_=whh1_ext.ap().rearrange("p (k m c) -> p k m c", k=2, m=8))
            bias1 = consts.tile([128, 8], F32, tag="bias1")
            nc.sync.dma_start(out=bias1, in_=bias1_ext[:, :])
            # wih2/bias2 load now: the L1 input GEMM's slotA half runs
            # interleaved inside the phase-1 recurrence
            wih2 = consts.tile([128, DK2, 8, 128], F16, tag="wih2")
            nc.sync.dma_start(
                out=wih2,
                in_=wih2_ext.ap().rearrange("p (k m c) -> p k m c", k=DK2, m=8))
            bias2 = consts.tile([128, 8], F32, tag="bias2")
            nc.sync.dma_start(out=bias2, in_=bias2_ext[:, :])

            CB = UNROLL * BL2
            # phase-1 h layout in DRAM is t-major: (p, c, t, b), split in
            # two t-halves so the first AllGather can overlap the rec tail
            hmvA = hminA.ap().rearrange("p (c t b) -> p c t b", c=2, t=T // 2)
            hmvB = hminB.ap().rearrange("p (c t b) -> p c t b", c=2, t=T // 2)

            def g1_items(tv_lead):
                # xg1 block tv_lead: input DMA now, 48 matmul quanta
                xb = stagep.tile([128, DK1, CB], F16, tag="xb1")
                nc.sync.dma_start(out=xb, in_=x1_ext[:, :, ds(tv_lead * BL2, CB)])
                items, pss = [], {}
                for m in range(8):
                    for k in range(DK1):
                        def it(m=m, k=k):
                            if k == 0:
                                pss[m] = ps_big.tile([128, CB], F32, tag="gemmps",
                                                     name=f"g1ps{m}")
                            nc.tensor.matmul(pss[m], wih1[:, k, m], xb[:, k],
                                             start=(k == 0), stop=(k == DK1 - 1))
                            if k == DK1 - 1:
                                st = stagep.tile([128, CB], F16, tag="xgstage",
                                                 name=f"g1st{m}")
                                nc.scalar.activation(st, pss[m], AF.Identity,
                                                     bias=bias1[:, m: m + 1])
                                nc.sync.dma_start(
                                    out=xg1_dram[:, m, ds(tv_lead * BL2, CB)], in_=st)
                        items.append(it)
                return items

            def p1_end(tv, half):
                if half == 0:
                    nc.sync.dma_start(out=hmvA[:, :, ds(tv, UNROLL)],
                                      in_=hc[:, :, 1: UNROLL + 1])
                else:
                    nc.sync.dma_start(out=hmvB[:, :, ds(tv - T // 2, UNROLL)],
                                      in_=hc[:, :, 1: UNROLL + 1])
                nc.gpsimd.tensor_copy(hc[:, :, 0], hc[:, :, UNROLL])

            xgsA = stagep.tile([128, 8, CB], F16, tag="xgs", name="xgsA")
            xgsB = stagep.tile([128, 8, CB], F16, tag="xgs", name="xgsB")

            def rec1_block(tv, fodder, xgs, pf, half):
                _emit_rec_block(nc, stagep, ps_rec, hc, cst, whh1, xgs,
                                xg1_dram, tv, fodder, prefetch_tv=pf)
                p1_end(tv, half)

            nc.vector.memset(cst, 0.0)
            nc.vector.memset(hc[:, :, 0], 0.0)
            for blk in range(2):
                for it in g1_items(16 * blk):
                    it()
            nc.sync.dma_start(out=xgsA, in_=xg1_dram[:, :, ds(0, CB)])
            nc.sync.dma_start(out=xgsB, in_=xg1_dram[:, :, ds(16 * BL2, CB)])
            rec1_block(0, [(g1_items(32), 3)], xgsA, 32, 0)
            rec1_block(16, [(g1_items(48), 3)], xgsB, 48, 0)

            def p1_loop(tv, half):
                itemsA = g1_items(tv + 32)
                itemsB = g1_items(tv + 48)  # DMA issued early: a block of lead
                _emit_rec_block(nc, stagep, ps_rec, hc, cst, whh1, xgsA,
                                xg1_dram, tv, [(itemsA, 3)], prefetch_tv=tv + 32)
                p1_end(tv, half)
                _emit_rec_block(nc, stagep, ps_rec, hc, cst, whh1, xgsB,
                                xg1_dram, tv + 16, [(itemsB, 3)],
                                prefetch_tv=tv + 48)
                p1_end(tv + 16, half)

            with tc.For_i(32, 256, 32) as tv:
                p1_loop(tv, 0)
            # first-half h is complete: exchange it while the rec tail runs
            nc.gpsimd.collective_compute(
                "AllGather", mybir.AluOpType.bypass,
                replica_groups=[[0, 2], [1, 3], [4, 6], [5, 7]],
                ins=[hminA.ap()], outs=[agoutA.ap()],
            )
            with tc.For_i(256, 480, 32) as tv:
                p1_loop(tv, 1)
            rec1_block(480, [], xgsA, None, 1)
            rec1_block(496, [], xgsB, None, 1)
            nc.gpsimd.collective_compute(
                "AllGather", mybir.AluOpType.bypass,
                replica_groups=[[0, 2], [1, 3], [4, 6], [5, 7]],
                ins=[hminB.ap()], outs=[agoutB.ap()],
            )

            # ================= PHASE 2 (L1) =================
            whh2 = consts.tile([128, 2, 8, 128], F16, tag="whh")
            nc.sync.dma_start(
                out=whh2,
                in_=whh2_ext.ap().rearrange("p (k m c) -> p k m c", k=2, m=8))

            def slot_ap(tensor, region, c, w_local):
                # [128, UNROLL*32] chunk c of a t-major half-h DRAM region
                # (p, c2, T/2, b32): contiguous ascending t window (w_local
                # in half-local units); reversed slots flip t SBUF-side.
                off = (region * 128 * HSZH + c * ((T // 2) * BL2)
                       + w_local * BL2)
                return bass.AP(tensor=tensor, offset=off,
                               ap=[[HSZH, 128], [1, UNROLL * BL2]])

            def g2_items(tv_lead, sa_t, sa_w, ag_t, rev_w):
                # full xg2 block: slotA = own h fwd, slotB = peer h reversed
                sa = stagep.tile([128, 2, UNROLL, BL2], F16, tag="slotA")
                r0 = stagep.tile([128, 2, UNROLL, BL2], F16, tag="slotR0")
                r1 = stagep.tile([128, 2, UNROLL, BL2], F16, tag="slotR1")
                for c in range(2):
                    nc.sync.dma_start(out=sa[:, c],
                                      in_=slot_ap(sa_t, 0, c, sa_w))
                    nc.sync.dma_start(out=r0[:, c],
                                      in_=slot_ap(ag_t, 0, c, rev_w))
                    nc.sync.dma_start(out=r1[:, c],
                                      in_=slot_ap(ag_t, 1, c, rev_w))
                sbr = stagep.tile([128, 2, UNROLL, BL2], F16, tag="slotBr")
                items, pss = [], {}

                def blend0():
                    nc.vector.tensor_scalar_mul(r0, r0, masks[:, 0:1])

                def blend1():
                    nc.vector.tensor_scalar_mul(r1, r1, masks[:, 1:2])

                def blend2():
                    # add, writing with the t window reversed
                    p_step = sbr.ap[0][0]
                    rev = bass.AP(tensor=sbr.tensor,
                                  offset=sbr.offset + (UNROLL - 1) * BL2,
                                  ap=[[p_step, 128], [UNROLL * BL2, 2],
                                      [-BL2, UNROLL], [1, BL2]])
                    nc.vector.tensor_add(rev, r0, r1)

                items += [blend0, blend1, blend2]

                def src_k(k):
                    t = sa if k < 2 else sbr
                    return t[:, k % 2].rearrange("p t b -> p (t b)")

                for m in range(8):
                    for k in range(DK2):
                        def it(m=m, k=k):
                            if k == 0:
                                pss[m] = ps_big.tile([128, CB], F32, tag="gemmps",
                                                     name=f"g2ps{m}")
                            nc.tensor.matmul(pss[m], wih2[:, k, m], src_k(k),
                                             start=(k == 0), stop=(k == DK2 - 1))
                            if k == DK2 - 1:
                                st = stagep.tile([128, CB], F16, tag="xgstage",
                                                 name=f"g2st{m}")
                                nc.scalar.activation(st, pss[m], AF.Identity,
                                                     bias=bias2[:, m: m + 1])
                                nc.sync.dma_start(
                                    out=xg2_dram[:, m, ds(tv_lead * BL2, CB)], in_=st)
                        items.append(it)
                return items

            def p2_end(tv):
                nc.gpsimd.tensor_copy(
                    h_sb[:, :, :, ds(tv + 1, UNROLL)],
                    hc[:, :, 1: UNROLL + 1].rearrange("p c t b -> p c b t"))
                nc.gpsimd.tensor_copy(hc[:, :, 0], hc[:, :, UNROLL])

            nc.vector.memset(cst, 0.0)
            nc.vector.memset(hc[:, :, 0], 0.0)
            HT = T // 2
            # leads 0,1: own h from half A, peer (reversed) from half B
            for blk in (0, 1):
                L = 16 * blk
                for it in g2_items(L, hminA, L, agoutB, (T - UNROLL) - L - HT):
                    it()
            nc.sync.dma_start(out=xgsA, in_=xg2_dram[:, :, ds(0, CB)])
            nc.sync.dma_start(out=xgsB, in_=xg2_dram[:, :, ds(16 * BL2, CB)])

            def p2_loop(tv, sa_t, sa_base, ag_t, rev_base):
                # halves at leads tv+32, tv+48; window args are half-local
                itemsA = g2_items(tv + 32, sa_t, sa_base + tv, ag_t,
                                  rev_base - tv)
                itemsB = g2_items(tv + 48, sa_t, sa_base + tv + 16, ag_t,
                                  rev_base - tv - 16)
                _emit_rec_block(nc, stagep, ps_rec, hc, cst, whh2, xgsA,
                                xg2_dram, tv, [(itemsA, 3)], prefetch_tv=tv + 32)
                p2_end(tv)
                _emit_rec_block(nc, stagep, ps_rec, hc, cst, whh2, xgsB,
                                xg2_dram, tv + 16, [(itemsB, 3)],
                                prefetch_tv=tv + 48)
                p2_end(tv + 16)

            with tc.For_i(0, 224, 32) as tv:
                # leads 2..15: own h half A, peer half B
                p2_loop(tv, hminA, 32, agoutB, (T - UNROLL) - 32 - HT)
            with tc.For_i(224, 480, 32) as tv:
                # leads 16..31: own h half B, peer half A
                p2_loop(tv, hminB, 32 - HT, agoutA, (T - UNROLL) - 32)
            _emit_rec_block(nc, stagep, ps_rec, hc, cst, whh2, xgsA,
                            xg2_dram, 480, [])
            p2_end(480)
            _emit_rec_block(nc, stagep, ps_rec, hc, cst, whh2, xgsB,
                            xg2_dram, 496, [])
            p2_end(496)

            # ---- AllToAll redistribution to data-parallel layout ----
            sb_view = sendb.ap().rearrange("j p (c b t) -> j p c b t", c=2, b=4)
            for j in range(8):
                for c in range(2):
                    nc.sync.dma_start(out=sb_view[j, :, c],
                                      in_=h_sb[:, c, ds(4 * j, 4), 1:HCOL])
            nc.gpsimd.collective_compute(
                "AllToAll", mybir.AluOpType.bypass,
                replica_groups=[list(range(8))],
                ins=[sendb.ap()], outs=[recvb.ap()],
            )

            # ================= PHASE 3: classifier + CRF =================
            cls1 = consts.tile([128, 8, 4, 128], F16, tag="wbig")
            nc.sync.dma_start(
                out=cls1,
                in_=cls1_ext.ap().rearrange("p (k m c) -> p k m c", k=8, m=4))
            clsb1 = consts.tile([128, 4], F32, tag="clsb1")
            nc.sync.dma_start(out=clsb1, in_=clsb1_ext[:, :])
            cls2 = consts.tile([128, 4, 15], F16, tag="cls2")
            nc.sync.dma_start(
                out=cls2, in_=cls2_ext.ap().rearrange("p (k j) -> p k j", k=4))
            clsb2 = consts.tile([15, 1], F32, tag="clsb2")
            nc.sync.dma_start(out=clsb2, in_=clsb2_ext[:, :])
            mexp = consts.tile([15, 15], F32, tag="mexp")
            nc.sync.dma_start(out=mexp, in_=mexp_ext[:, :])
            transn = consts.tile([15, 15], F16, tag="transn")
            nc.sync.dma_start(out=transn, in_=transn_ext[:, :])
            crfstart = consts.tile([15, 1], F32, tag="crfstart")
            nc.sync.dma_start(out=crfstart, in_=start_ext[:, :])
            crfend = consts.tile([15, 1], F32, tag="crfend")
            nc.sync.dma_start(out=crfend, in_=end_ext[:, :])
            lnalpha = consts.tile([15, 1], F32, tag="lnalpha")
            nc.sync.dma_start(out=lnalpha, in_=lna_ext[:, :])
            tago = consts.tile([15, TB3], F16, tag="tago")
            nc.sync.dma_start(out=tago, in_=tago_ext[:, :])

            logits = seqs.tile([15, TB3], F32, tag="logits")

            NT = 64  # t-steps per classifier n-tile (NT*BL3 = 512 cols)
            SHSZ = 128 * 2 * 4 * T  # elements per recv shard

            def comb_ap(kk, half, ns):
                # [128, 4, NT]: dir kk//2, chunk kk%2, half-shard, n-tile ns;
                # always an ascending t window (reversal done in the rhs view)
                d, c = kk // 2, kk % 2
                rev = d in (1, 3)  # c1b, w1b stored time-reversed
                base = (2 * d + half) * SHSZ + c * (4 * T)
                if not rev:
                    off = base + ns * NT
                else:
                    off = base + (T - NT) - ns * NT
                return bass.AP(tensor=recvb, offset=off,
                               ap=[[2 * 4 * T, 128], [T, 4], [1, NT]])

            for ns in range(8):
                comb = stagep.tile([128, 8, BL3, NT], F16, tag="comb", bufs=2)
                for kk in range(8):
                    for half in range(2):
                        nc.sync.dma_start(out=comb[:, kk, ds(4 * half, 4)],
                                          in_=comb_ap(kk, half, ns))
                hmt = []
                for m in range(4):
                    ps = ps_big.tile([128, NT * BL3], F32, tag="gemmps")
                    for kk in range(8):
                        if kk // 2 in (1, 3):
                            p_step = comb.ap[0][0]
                            off = (comb.offset + kk * BL3 * NT + (NT - 1))
                            rhs = bass.AP(tensor=comb.tensor, offset=off,
                                          ap=[[p_step, 128], [-1, NT],
                                              [NT, BL3]])
                        else:
                            rhs = comb[:, kk].rearrange("p b t -> p t b")
                        nc.tensor.matmul(ps, cls1[:, kk, m], rhs,
                                         start=(kk == 0), stop=(kk == 7))
                    hm = stagep.tile([128, NT * BL3], F16, tag="hm", bufs=4,
                                     name=f"hm{m}")
                    nc.scalar.activation(hm, ps, AF.Relu, bias=clsb1[:, m: m + 1])
                    hmt.append(hm)
                ps2 = ps_small.tile([15, NT * BL3], F32, tag="small")
                for m in range(4):
                    nc.tensor.matmul(ps2, cls2[:, m], hmt[m],
                                     start=(m == 0), stop=(m == 3))
                nc.vector.tensor_scalar_add(
                    logits[:, ds(ns * NT * BL3, NT * BL3)], ps2, clsb2)

            # fold CRF start/end into first/last emission columns
            nc.vector.tensor_scalar_add(logits[:, 0:BL3], logits[:, 0:BL3], crfstart)
            nc.vector.tensor_scalar_add(logits[:, TB3 - BL3: TB3],
                                        logits[:, TB3 - BL3: TB3], crfend)

            # ---- CRF numerator ----
            racc = work.tile([15, 16], F32, tag="racc")
            nc.vector.memset(racc, 0.0)
            for ns in range(8):
                pre = stagep.tile([15, 512], F32, tag="prodns")
                nc.vector.tensor_mul(pre, logits[:, ds(ns * 512, 512)],
                                     tago[:, ds(ns * 512, 512)])
                nc.vector.tensor_reduce(racc[:, 8 + ns: 9 + ns], pre,
                                        axis=mybir.AxisListType.X,
                                        op=mybir.AluOpType.add)
                psv = ps_small.tile([15, 512], F32, tag="small")
                nc.tensor.matmul(psv, transn, tago[:, ds(ns * 512, 512)],
                                 start=True, stop=True)
                w = 512 if ns < 7 else 512 - BL3
                pr = stagep.tile([15, 512], F32, tag="prodns")
                nc.vector.tensor_mul(pr[:, :w], psv[:, :w],
                                     tago[:, ds(ns * 512 + BL3, w)])
                nc.vector.tensor_reduce(racc[:, ns: ns + 1], pr[:, :w],
                                        axis=mybir.AxisListType.X,
                                        op=mybir.AluOpType.add)
            nv = stagep.tile([15, 1], F32, tag="nv")
            nc.vector.tensor_reduce(nv, racc, axis=mybir.AxisListType.X,
                                    op=mybir.AluOpType.add)
            ones15 = consts.tile([15, 1], F32, tag="ones15")
            nc.vector.memset(ones15, 1.0)
            psn = ps_small.tile([1, 1], F32, tag="small")
            nc.tensor.matmul(psn, ones15, nv, start=True, stop=True)
            num11 = work.tile([1, 1], F32, tag="num11")
            nc.vector.tensor_copy(num11, psn)

            # ---- CRF forward scan, probability space ----
            # two independent half-batch chains so the 15x15 matmul of one
            # overlaps the vector multiply of the other. p0 first, then
            # E = alpha*exp(logits) in place (last col already has e^end).
            pA = seqs.tile([15, 4], F32, tag="pvecA")
            pB = seqs.tile([15, 4], F32, tag="pvecB")
            nc.scalar.activation(pA, logits[:, 0:4], AF.Exp)
            nc.scalar.activation(pB, logits[:, 4:8], AF.Exp)
            E = logits
            nc.scalar.activation(E, logits, AF.Exp, bias=lnalpha)

            def crf_step(col):
                zA = ps_small.tile([15, 4], F32, tag="small", name="zA")
                nc.tensor.matmul(zA, mexp, pA, start=True, stop=True)
                zB = ps_small.tile([15, 4], F32, tag="small", name="zB")
                nc.tensor.matmul(zB, mexp, pB, start=True, stop=True)
                nc.vector.tensor_mul(pA, zA, E[:, col: col + 4] if isinstance(col, int)
                                     else E[:, ds(col, 4)])
                nc.vector.tensor_mul(pB, zB, E[:, col + 4: col + 8] if isinstance(col, int)
                                     else E[:, ds(col + 4, 4)])

            for t in range(1, 16):
                crf_step(t * BL3)
            with tc.For_i(0, 496, UNROLL) as tv:
                for j in range(UNROLL):
                    crf_step((16 + j) * BL3 + tv * BL3)

            # ---- denominator + output ----
            den11 = work.tile([1, 1], F32, tag="den11")
            for idx, pv in enumerate((pA, pB)):
                psd = ps_small.tile([1, 4], F32, tag="small", name=f"psd{idx}")
                nc.tensor.matmul(psd, ones15, pv, start=True, stop=True)
                ln4 = stagep.tile([1, 4], F32, tag="ln8", name=f"ln4{idx}")
                nc.scalar.activation(ln4, psd, AF.Ln)
                if idx == 0:
                    nc.vector.tensor_reduce(den11, ln4, axis=mybir.AxisListType.X,
                                            op=mybir.AluOpType.add)
                else:
                    dh = work.tile([1, 1], F32, tag="denh")
                    nc.vector.tensor_reduce(dh, ln4, axis=mybir.AxisListType.X,
                                            op=mybir.AluOpType.add)
                    nc.vector.tensor_add(den11, den11, dh)
            res = work.tile([1, 1], F32, tag="res")
            nc.vector.tensor_sub(res, den11, num11)
            nc.sync.dma_start(out=out_ext[:, :], in_=res)

    nc.finalize()
    _BUILD_CACHE["nc"] = nc
    return nc


# ---- host-side input prep ---------------------------------------------------

# gate perm [i(256), f(256), g(256), o(256)] -> [i, f, o, g~]
_GPERM = np.concatenate([np.arange(0, 512), np.arange(768, 1024), np.arange(512, 768)])

# core c -> (pathway, direction, half): 0..3 char f/f/b/b, 4..7 word
_ROLES = [("c", 0, 0), ("c", 0, 1), ("c", 1, 0), ("c", 1, 1),
          ("w", 0, 0), ("w", 0, 1), ("w", 1, 0), ("w", 1, 1)]


def _wih_prep(W, dk_n):
    Wp = W[_GPERM]
    return np.ascontiguousarray(
        Wp.reshape(8, 128, dk_n, 128).transpose(3, 2, 0, 1).reshape(128, dk_n * 8 * 128)
    ).astype(np.float16)


def _make_in_maps(inputs):
    char_ids = np.asarray(inputs["char_ids"])
    tags = np.asarray(inputs["tags"])
    wemb = np.asarray(inputs["word_embeddings"], np.float32)
    emb = np.asarray(inputs["char_emb_table"], np.float32)
    trans = np.asarray(inputs["crf_trans"], np.float32)

    alpha = 1.0 / (15.0 * float(np.exp(trans).mean()))
    common = {}
    w1 = np.asarray(inputs["cls_w1"], np.float32)
    common["cls1"] = np.ascontiguousarray(
        w1.reshape(4, 128, 8, 128).transpose(3, 2, 0, 1).reshape(128, 8 * 4 * 128)
    ).astype(np.float16)
    common["clsb1"] = np.ascontiguousarray(
        np.asarray(inputs["cls_b1"], np.float32).reshape(4, 128).T).astype(np.float32)
    w2 = np.asarray(inputs["cls_w2"], np.float32)
    common["cls2"] = np.ascontiguousarray(
        w2.reshape(15, 4, 128).transpose(2, 1, 0).reshape(128, 4 * 15)).astype(np.float16)
    common["clsb2"] = np.asarray(inputs["cls_b2"], np.float32).reshape(15, 1).copy()
    common["mexp"] = np.exp(trans).astype(np.float32)
    common["transn"] = trans.astype(np.float16)
    common["crfstart"] = np.asarray(inputs["crf_start"], np.float32).reshape(15, 1).copy()
    common["crfend"] = np.asarray(inputs["crf_end"], np.float32).reshape(15, 1).copy()
    common["lnalpha"] = np.full((15, 1), np.log(alpha), np.float32)

    in_maps = []
    for c in range(NC_N):
        pw, d, hf = _ROLES[c]
        lo, hi = hf * BL2, (hf + 1) * BL2
        m = dict(common)

        # phase-1 weights/input
        if pw == "c":
            Wih1 = np.zeros((1024, 768), np.float32)
            Wih1[:, :128] = np.asarray(inputs["c0_Wih"], np.float32)[d]
            Whh1 = np.asarray(inputs["c0_Whh"], np.float32)[d]
            b1 = (np.asarray(inputs["c0_bih"], np.float32)[d]
                  + np.asarray(inputs["c0_bhh"], np.float32)[d])
            ce = emb[char_ids[lo:hi]]  # (32, 512, 128)
            X = np.zeros((128, DK1, T, BL2), np.float32)
            X[:, 0] = ce.transpose(2, 1, 0)
            Wl1 = np.asarray(inputs["c1_Wih"], np.float32)[d]
            Whh2 = np.asarray(inputs["c1_Whh"], np.float32)[d]
            b2 = (np.asarray(inputs["c1_bih"], np.float32)[d]
                  + np.asarray(inputs["c1_bhh"], np.float32)[d])
        else:
            Wih1 = np.asarray(inputs["w0_Wih"], np.float32)[d]
            Whh1 = np.asarray(inputs["w0_Whh"], np.float32)[d]
            b1 = (np.asarray(inputs["w0_bih"], np.float32)[d]
                  + np.asarray(inputs["w0_bhh"], np.float32)[d])
            X = wemb[lo:hi].reshape(BL2, T, DK1, 128).transpose(3, 2, 1, 0)
            Wl1 = np.asarray(inputs["w1_Wih"], np.float32)[d]
            Whh2 = np.asarray(inputs["w1_Whh"], np.float32)[d]
            b2 = (np.asarray(inputs["w1_bih"], np.float32)[d]
                  + np.asarray(inputs["w1_bhh"], np.float32)[d])
        if d == 1:  # backward: reverse local time
            X = X[:, :, ::-1]
        m["x1"] = np.ascontiguousarray(X.reshape(128, DK1, T * BL2)).astype(np.float16)
        m["wih1"] = _wih_prep(Wih1, DK1)
        m["whh1"] = _wih_prep(Whh1, 2)
        m["bias1"] = np.ascontiguousarray(b1[_GPERM].reshape(8, 128).T).astype(np.float32)

        # phase-2 weights: columns [own(256) | peer(256)]
        if d == 1:
            Wl1 = Wl1[:, np.r_[256:512, 0:256]]
        m["wih2"] = _wih_prep(Wl1, DK2)
        m["whh2"] = _wih_prep(Whh2, 2)
        m["bias2"] = np.ascontiguousarray(b2[_GPERM].reshape(8, 128).T).astype(np.float32)
        # blend: f-core (d=0) picks AG region 1 (the b-core), b-core picks 0
        msk = np.zeros((128, 2), np.float32)
        msk[:, 1 - d] = 1.0
        m["masks"] = msk

        # phase-3 tags for this core's 8 sequences
        seqs3 = np.r_[4 * c: 4 * c + 4, 32 + 4 * c: 32 + 4 * c + 4]
        oh = (np.arange(K)[:, None, None] == tags[seqs3][None]).astype(np.float32)
        # (15, 8seq, 512t) -> (15, t, b)
        m["tagoneT"] = np.ascontiguousarray(
            oh.transpose(0, 2, 1).reshape(K, TB3)).astype(np.float16)
        in_maps.append(m)
    return in_maps, alpha


def kernel(**inputs):
    nc = _build_nc()
    in_maps, alpha = _make_in_maps(inputs)
    res = run_bass_kernel_spmd(nc, in_maps, core_ids=list(range(NC_N)))
    total = sum(float(res.results[c]["out"][0, 0]) for c in range(NC_N))
    total -= B * (T - 1) * np.log(alpha)
    return np.float32(total / B)


# revision 24
# speedup vs baseline: 1.1379x; 1.0106x over previous
"""BiLSTM dual-pathway + CRF NLL kernel for 8 Trainium2 NeuronCores.

Sharding: direction-parallel for the LSTM recurrences, data-parallel for the
classifier/CRF. Phase 1 runs the four layer-0 directions (char fwd/bwd, word
fwd/bwd) on 8 cores as (direction x batch-half), batch 32 per core, so the
recurrent matmuls run at N=32 instead of N=8 and the sequential chain count
drops from 4096 to 1024 steps. A pairwise AllGather exchanges the L0 hidden
states between fwd/bwd cores, phase 2 runs the four layer-1 directions the
same way, then an 8-rank AllToAll redistributes hidden states to a
data-parallel layout (8 sequences per core) for the classifier and CRF.

SPMD uniformity: every core runs the identical program. Backward directions
receive host-time-reversed inputs; reversed reads of peer hidden states are
fixed negative-stride APs, with host-permuted weight columns absorbing the
f/b role differences. Peer-region selection after the AllGather uses per-core
0/1 blend masks delivered as input data.

The CRF forward scan runs in probability space with a constant per-step
prescale alpha folded into the emission exponentials (corrected analytically
on the host), so each step is one resident-weight 15x15 matmul plus one
vector multiply -- no per-step exp/ln activation-table swaps.
"""

import sys

sys.path.insert(0, "/opt/trn_rl_repo")

import numpy as np

import concourse.bass as bass
import concourse.mybir as mybir
from concourse import bacc
from concourse.bass import ds
from concourse.tile import TileContext
from concourse.bass_utils import run_bass_kernel_spmd

F16 = mybir.dt.float16
F32 = mybir.dt.float32
AF = mybir.ActivationFunctionType

B, T, V, K = 64, 512, 40, 15
NC_N = 8
BL2 = 32            # batch per core in phases 1-2
BL3 = 8             # sequences per core in phase 3
TB3 = T * BL3       # 4096 classifier/CRF columns per core
DK1, DK2 = 6, 4     # input chunks for L0 (word=768, char padded) and L1 (512)
UNROLL = 16
HCOL = T + 1        # h buffer columns per sequence (col 0 = zero init)

_BUILD_CACHE = {}


def _emit_rec_block(nc, stagep, ps_rec, hc, cst, whh, xgs, xg_dram, tv, fodder,
                    prefetch_tv=None):
    """Emit 16 recurrence steps for time block tv.

    fodder: list of (items, per_step) - closures emitting one PE-side quantum
    each (GEMM matmuls for other pipeline stages), dispensed between the
    step's recurrent matmuls and its vector/scalar chain so they execute in
    the PE's dependency-stall gaps.
    """
    CB = UNROLL * BL2
    cursors = [[items, 0, per_step] for items, per_step in fodder]
    for j in range(UNROLL):
        psA = ps_rec.tile([128, 4, BL2], F32, tag="recpsA")
        psB = ps_rec.tile([128, 4, BL2], F32, tag="recpsB")
        # i,f gates (chunks 0-3) first into psA so the first g-add can
        # start while the o/g~ matmuls (psB) still run
        for m in (0, 1, 2, 3):
            for k in range(2):
                nc.tensor.matmul(psA[:, m], whh[:, k, m],
                                 hc[:, k, j], start=(k == 0), stop=(k == 1))
        for m in (6, 7, 4, 5):
            for k in range(2):
                nc.tensor.matmul(psB[:, m - 4], whh[:, k, m],
                                 hc[:, k, j], start=(k == 0), stop=(k == 1))
        for cur in cursors:
            for _ in range(cur[2]):
                if cur[1] < len(cur[0]):
                    cur[0][cur[1]]()
                    cur[1] += 1
        g = stagep.tile([128, 8, BL2], F32, tag="g")
        xsl = xgs[:, :, ds(j * BL2, BL2)]
        nc.vector.tensor_add(g[:, 0:4], psA, xsl[:, 0:4])
        sig = stagep.tile([128, 6, BL2], F32, tag="sig")
        nc.scalar.activation(sig[:, 0:4], g[:, 0:4], AF.Sigmoid)
        nc.vector.tensor_mul(cst, cst, sig[:, 2:4])
        nc.vector.tensor_add(g[:, 4:8], psB, xsl[:, 4:8])
        tgg = stagep.tile([128, 2, BL2], F32, tag="tgg")
        nc.scalar.activation(tgg, g[:, 6:8], AF.Tanh)
        tmp = stagep.tile([128, 2, BL2], F32, tag="tmpig")
        nc.vector.tensor_mul(tmp, sig[:, 0:2], tgg)
        nc.scalar.activation(sig[:, 4:6], g[:, 4:6], AF.Sigmoid)
        nc.vector.tensor_add(cst, cst, tmp)
        tch = stagep.tile([128, 2, BL2], F32, tag="tch")
        nc.scalar.activation(tch, cst, AF.Tanh)
        nc.vector.tensor_mul(hc[:, :, j + 1], sig[:, 4:6], tch)
    for cur in cursors:
        while cur[1] < len(cur[0]):
            cur[0][cur[1]]()
            cur[1] += 1
    if prefetch_tv is not None:
        # refill this block's (just-consumed) xgs buffer with the same-parity
        # block two ahead -- a full block of DMA lead time
        nc.sync.dma_start(out=xgs, in_=xg_dram[:, :, ds(prefetch_tv * BL2, CB)])


def _build_nc():
    if "nc" in _BUILD_CACHE:
        return _BUILD_CACHE["nc"]
    nc = bacc.Bacc(target_bir_lowering=False, num_devices=NC_N)

    # ---- external parameters -------------------------------------------------
    x1_ext = nc.declare_dram_parameter("x1", [128, DK1, T * BL2], F16, isOutput=False)
    wih1_ext = nc.declare_dram_parameter("wih1", [128, DK1 * 8 * 128], F16, isOutput=False)
    whh1_ext = nc.declare_dram_parameter("whh1", [128, 2 * 8 * 128], F16, isOutput=False)
    bias1_ext = nc.declare_dram_parameter("bias1", [128, 8], F32, isOutput=False)
    wih2_ext = nc.declare_dram_parameter("wih2", [128, DK2 * 8 * 128], F16, isOutput=False)
    whh2_ext = nc.declare_dram_parameter("whh2", [128, 2 * 8 * 128], F16, isOutput=False)
    bias2_ext = nc.declare_dram_parameter("bias2", [128, 8], F32, isOutput=False)
    masks_ext = nc.declare_dram_parameter("masks", [128, 2], F32, isOutput=False)
    cls1_ext = nc.declare_dram_parameter("cls1", [128, 8 * 4 * 128], F16, isOutput=False)
    clsb1_ext = nc.declare_dram_parameter("clsb1", [128, 4], F32, isOutput=False)
    cls2_ext = nc.declare_dram_parameter("cls2", [128, 4 * 15], F16, isOutput=False)
    clsb2_ext = nc.declare_dram_parameter("clsb2", [15, 1], F32, isOutput=False)
    mexp_ext = nc.declare_dram_parameter("mexp", [15, 15], F32, isOutput=False)
    transn_ext = nc.declare_dram_parameter("transn", [15, 15], F16, isOutput=False)
    start_ext = nc.declare_dram_parameter("crfstart", [15, 1], F32, isOutput=False)
    end_ext = nc.declare_dram_parameter("crfend", [15, 1], F32, isOutput=False)
    lna_ext = nc.declare_dram_parameter("lnalpha", [15, 1], F32, isOutput=False)
    tago_ext = nc.declare_dram_parameter("tagoneT", [15, TB3], F16, isOutput=False)
    out_ext = nc.declare_dram_parameter("out", [1, 1], F32, isOutput=True)

    # ---- internal DRAM -------------------------------------------------------
    HSZ = 2 * BL2 * T  # 32768 cols/partition of h (f16)
    xg1_dram = nc.dram_tensor("xg1", [128, 8, T * BL2], F16)
    xg2_dram = nc.dram_tensor("xg2", [128, 8, T * BL2], F16)
    HSZH = HSZ // 2  # per-partition elements of a t-half of h
    hminA = nc.dram_tensor("hminA", [128, HSZH], F16)
    hminB = nc.dram_tensor("hminB", [128, HSZH], F16)
    agoutA = nc.dram_tensor("agoutA", [2, 128, HSZH], F16)
    agoutB = nc.dram_tensor("agoutB", [2, 128, HSZH], F16)
    sendbA = nc.dram_tensor("sendbA", [8, 128, 2 * 4 * (T // 2)], F16)
    sendbB = nc.dram_tensor("sendbB", [8, 128, 2 * 4 * (T // 2)], F16)
    recvbA = nc.dram_tensor("recvbA", [8, 128, 2 * 4 * (T // 2)], F16)
    recvbB = nc.dram_tensor("recvbB", [8, 128, 2 * 4 * (T // 2)], F16)

    with TileContext(nc) as tc:
        with (
            tc.tile_pool(name="consts", bufs=1) as consts,
            tc.tile_pool(name="seqs", bufs=1) as seqs,
            tc.tile_pool(name="work", bufs=2) as work,
            tc.tile_pool(name="stage", bufs=2) as stagep,
            tc.tile_pool(name="ps_big", bufs=3, space="PSUM") as ps_big,
            tc.tile_pool(name="ps_rec", bufs=1, space="PSUM") as ps_rec,
            tc.tile_pool(name="ps_small", bufs=3, space="PSUM") as ps_small,
        ):
            # h buffer, seq-major: [128, chunk2, b32, T+1], reused by phases 1+2
            h_sb = seqs.tile([128, 2, BL2, HCOL], F16, tag="h_sb")
            hc = seqs.tile([128, 2, UNROLL + 1, BL2], F16, tag="hcomp")
            cst = seqs.tile([128, 2, BL2], F32, tag="cstate")
            masks = consts.tile([128, 2], F32, tag="masks")
            nc.sync.dma_start(out=masks, in_=masks_ext[:, :])

            # ================= PHASE 1 (L0) =================
            wih1 = consts.tile([128, DK1, 8, 128], F16, tag="wbig")
            nc.sync.dma_start(
                out=wih1,
                in_=wih1_ext.ap().rearrange("p (k m c) -> p k m c", k=DK1, m=8))
            whh1 = consts.tile([128, 2, 8, 128], F16, tag="whh")
            nc.sync.dma_start(
                out=whh1,
                in_=whh1_ext.ap().rearrange("p (k m c) -> p k m c", k=2, m=8))
            bias1 = consts.tile([128, 8], F32, tag="bias1")
            nc.sync.dma_start(out=bias1, in_=bias1_ext[:, :])
            # wih2/bias2 load now: the L1 input GEMM's slotA half runs
            # interleaved inside the phase-1 recurrence
            wih2 = consts.tile([128, DK2, 8, 128], F16, tag="wih2")
            nc.sync.dma_start(
                out=wih2,
                in_=wih2_ext.ap().rearrange("p (k m c) -> p k m c", k=DK2, m=8))
            bias2 = consts.tile([128, 8], F32, tag="bias2")
            nc.sync.dma_start(out=bias2, in_=bias2_ext[:, :])

            CB = UNROLL * BL2
            # phase-1 h layout in DRAM is t-major: (p, c, t, b), split in
            # two t-halves so the first AllGather can overlap the rec tail
            hmvA = hminA.ap().rearrange("p (c t b) -> p c t b", c=2, t=T // 2)
            hmvB = hminB.ap().rearrange("p (c t b) -> p c t b", c=2, t=T // 2)

            def g1_items(tv_lead):
                # xg1 block tv_lead: input DMA now, 48 matmul quanta
                xb = stagep.tile([128, DK1, CB], F16, tag="xb1")
                nc.sync.dma_start(out=xb, in_=x1_ext[:, :, ds(tv_lead * BL2, CB)])
                items, pss = [], {}
                for m in range(8):
                    for k in range(DK1):
                        def it(m=m, k=k):
                            if k == 0:
                                pss[m] = ps_big.tile([128, CB], F32, tag="gemmps",
                                                     name=f"g1ps{m}")
                            nc.tensor.matmul(pss[m], wih1[:, k, m], xb[:, k],
                                             start=(k == 0), stop=(k == DK1 - 1))
                            if k == DK1 - 1:
                                st = stagep.tile([128, CB], F16, tag="xgstage",
                                                 name=f"g1st{m}")
                                nc.scalar.activation(st, pss[m], AF.Identity,
                                                     bias=bias1[:, m: m + 1])
                                nc.sync.dma_start(
                                    out=xg1_dram[:, m, ds(tv_lead * BL2, CB)], in_=st)
                        items.append(it)
                return items

            def p1_end(tv, half):
                if half == 0:
                    nc.sync.dma_start(out=hmvA[:, :, ds(tv, UNROLL)],
                                      in_=hc[:, :, 1: UNROLL + 1])
                else:
                    nc.sync.dma_start(out=hmvB[:, :, ds(tv - T // 2, UNROLL)],
                                      in_=hc[:, :, 1: UNROLL + 1])
                nc.gpsimd.tensor_copy(hc[:, :, 0], hc[:, :, UNROLL])

            xgsA = stagep.tile([128, 8, CB], F16, tag="xgs", name="xgsA")
            xgsB = stagep.tile([128, 8, CB], F16, tag="xgs", name="xgsB")

            def rec1_block(tv, fodder, xgs, pf, half):
                _emit_rec_block(nc, stagep, ps_rec, hc, cst, whh1, xgs,
                                xg1_dram, tv, fodder, prefetch_tv=pf)
                p1_end(tv, half)

            nc.vector.memset(cst, 0.0)
            nc.vector.memset(hc[:, :, 0], 0.0)
            for blk in range(2):
                for it in g1_items(16 * blk):
                    it()
            nc.sync.dma_start(out=xgsA, in_=xg1_dram[:, :, ds(0, CB)])
            nc.sync.dma_start(out=xgsB, in_=xg1_dram[:, :, ds(16 * BL2, CB)])
            rec1_block(0, [(g1_items(32), 3)], xgsA, 32, 0)
            rec1_block(16, [(g1_items(48), 3)], xgsB, 48, 0)

            def p1_loop(tv, half):
                itemsA = g1_items(tv + 32)
                itemsB = g1_items(tv + 48)  # DMA issued early: a block of lead
                _emit_rec_block(nc, stagep, ps_rec, hc, cst, whh1, xgsA,
                                xg1_dram, tv, [(itemsA, 3)], prefetch_tv=tv + 32)
                p1_end(tv, half)
                _emit_rec_block(nc, stagep, ps_rec, hc, cst, whh1, xgsB,
                                xg1_dram, tv + 16, [(itemsB, 3)],
                                prefetch_tv=tv + 48)
                p1_end(tv + 16, half)

            with tc.For_i(32, 256, 32) as tv:
                p1_loop(tv, 0)
            # first-half h is complete: exchange it while the rec tail runs
            nc.gpsimd.collective_compute(
                "AllGather", mybir.AluOpType.bypass,
                replica_groups=[[0, 2], [1, 3], [4, 6], [5, 7]],
                ins=[hminA.ap()], outs=[agoutA.ap()],
            )
            with tc.For_i(256, 480, 32) as tv:
                p1_loop(tv, 1)
            rec1_block(480, [], xgsA, None, 1)
            rec1_block(496, [], xgsB, None, 1)
            nc.gpsimd.collective_compute(
                "AllGather", mybir.AluOpType.bypass,
                replica_groups=[[0, 2], [1, 3], [4, 6], [5, 7]],
                ins=[hminB.ap()], outs=[agoutB.ap()],
            )

            # ================= PHASE 2 (L1) =================
            whh2 = consts.tile([128, 2, 8, 128], F16, tag="whh")
            nc.sync.dma_start(
                out=whh2,
                in_=whh2_ext.ap().rearrange("p (k m c) -> p k m c", k=2, m=8))

            def slot_ap(tensor, region, c, w_local):
                # [128, UNROLL*32] chunk c of a t-major half-h DRAM region
                # (p, c2, T/2, b32): contiguous ascending t window (w_local
                # in half-local units); reversed slots flip t SBUF-side.
                off = (region * 128 * HSZH + c * ((T // 2) * BL2)
                       + w_local * BL2)
                return bass.AP(tensor=tensor, offset=off,
                               ap=[[HSZH, 128], [1, UNROLL * BL2]])

            def g2_items(tv_lead, sa_t, sa_w, ag_t, rev_w):
                # full xg2 block: slotA = own h fwd, slotB = peer h reversed
                sa = stagep.tile([128, 2, UNROLL, BL2], F16, tag="slotA")
                r0 = stagep.tile([128, 2, UNROLL, BL2], F16, tag="slotR0")
                r1 = stagep.tile([128, 2, UNROLL, BL2], F16, tag="slotR1")
                for c in range(2):
                    nc.sync.dma_start(out=sa[:, c],
                                      in_=slot_ap(sa_t, 0, c, sa_w))
                    nc.sync.dma_start(out=r0[:, c],
                                      in_=slot_ap(ag_t, 0, c, rev_w))
                    nc.sync.dma_start(out=r1[:, c],
                                      in_=slot_ap(ag_t, 1, c, rev_w))
                sbr = stagep.tile([128, 2, UNROLL, BL2], F16, tag="slotBr")
                items, pss = [], {}

                def blend0():
                    nc.vector.tensor_scalar_mul(r0, r0, masks[:, 0:1])

                def blend1():
                    nc.vector.tensor_scalar_mul(r1, r1, masks[:, 1:2])

                def blend2():
                    # add, writing with the t window reversed
                    p_step = sbr.ap[0][0]
                    rev = bass.AP(tensor=sbr.tensor,
                                  offset=sbr.offset + (UNROLL - 1) * BL2,
                                  ap=[[p_step, 128], [UNROLL * BL2, 2],
                                      [-BL2, UNROLL], [1, BL2]])
                    nc.vector.tensor_add(rev, r0, r1)

                items += [blend0, blend1, blend2]

                def src_k(k):
                    t = sa if k < 2 else sbr
                    return t[:, k % 2].rearrange("p t b -> p (t b)")

                for m in range(8):
                    for k in range(DK2):
                        def it(m=m, k=k):
                            if k == 0:
                                pss[m] = ps_big.tile([128, CB], F32, tag="gemmps",
                                                     name=f"g2ps{m}")
                            nc.tensor.matmul(pss[m], wih2[:, k, m], src_k(k),
                                             start=(k == 0), stop=(k == DK2 - 1))
                            if k == DK2 - 1:
                                st = stagep.tile([128, CB], F16, tag="xgstage",
                                                 name=f"g2st{m}")
                                nc.scalar.activation(st, pss[m], AF.Identity,
                                                     bias=bias2[:, m: m + 1])
                                nc.sync.dma_start(
                                    out=xg2_dram[:, m, ds(tv_lead * BL2, CB)], in_=st)
                        items.append(it)
                return items

            def p2_end(tv):
                nc.gpsimd.tensor_copy(
                    h_sb[:, :, :, ds(tv + 1, UNROLL)],
                    hc[:, :, 1: UNROLL + 1].rearrange("p c t b -> p c b t"))
                nc.gpsimd.tensor_copy(hc[:, :, 0], hc[:, :, UNROLL])

            nc.vector.memset(cst, 0.0)
            nc.vector.memset(hc[:, :, 0], 0.0)
            HT = T // 2
            # leads 0,1: own h from half A, peer (reversed) from half B
            for blk in (0, 1):
                L = 16 * blk
                for it in g2_items(L, hminA, L, agoutB, (T - UNROLL) - L - HT):
                    it()
            nc.sync.dma_start(out=xgsA, in_=xg2_dram[:, :, ds(0, CB)])
            nc.sync.dma_start(out=xgsB, in_=xg2_dram[:, :, ds(16 * BL2, CB)])

            def p2_loop(tv, sa_t, sa_base, ag_t, rev_base):
                # halves at leads tv+32, tv+48; window args are half-local
                itemsA = g2_items(tv + 32, sa_t, sa_base + tv, ag_t,
                                  rev_base - tv)
                itemsB = g2_items(tv + 48, sa_t, sa_base + tv + 16, ag_t,
                                  rev_base - tv - 16)
                _emit_rec_block(nc, stagep, ps_rec, hc, cst, whh2, xgsA,
                                xg2_dram, tv, [(itemsA, 3)], prefetch_tv=tv + 32)
                p2_end(tv)
                _emit_rec_block(nc, stagep, ps_rec, hc, cst, whh2, xgsB,
                                xg2_dram, tv + 16, [(itemsB, 3)],
                                prefetch_tv=tv + 48)
                p2_end(tv + 16)

            with tc.For_i(0, 224, 32) as tv:
                # leads 2..15: own h half A, peer half B
                p2_loop(tv, hminA, 32, agoutB, (T - UNROLL) - 32 - HT)
            with tc.For_i(224, 256, 32) as tv:
                # blocks 14,15: own h half B, peer half A
                p2_loop(tv, hminB, 32 - HT, agoutA, (T - UNROLL) - 32)
            # first t-half of phase-2 h is complete: redistribute it while
            # the rec tail runs
            sbA_view = sendbA.ap().rearrange("j p (c b t) -> j p c b t",
                                             c=2, b=4)
            for j in range(8):
                for c in range(2):
                    nc.sync.dma_start(out=sbA_view[j, :, c],
                                      in_=h_sb[:, c, ds(4 * j, 4), 1: 1 + HT])
            nc.gpsimd.collective_compute(
                "AllToAll", mybir.AluOpType.bypass,
                replica_groups=[list(range(8))],
                ins=[sendbA.ap()], outs=[recvbA.ap()],
            )
            with tc.For_i(256, 480, 32) as tv:
                p2_loop(tv, hminB, 32 - HT, agoutA, (T - UNROLL) - 32)
            _emit_rec_block(nc, stagep, ps_rec, hc, cst, whh2, xgsA,
                            xg2_dram, 480, [])
            p2_end(480)
            _emit_rec_block(nc, stagep, ps_rec, hc, cst, whh2, xgsB,
                            xg2_dram, 496, [])
            p2_end(496)

            sbB_view = sendbB.ap().rearrange("j p (c b t) -> j p c b t",
                                             c=2, b=4)
            for j in range(8):
                for c in range(2):
                    nc.sync.dma_start(out=sbB_view[j, :, c],
                                      in_=h_sb[:, c, ds(4 * j, 4),
                                               1 + HT: HCOL])
            nc.gpsimd.collective_compute(
                "AllToAll", mybir.AluOpType.bypass,
                replica_groups=[list(range(8))],
                ins=[sendbB.ap()], outs=[recvbB.ap()],
            )

            # ================= PHASE 3: classifier + CRF =================
            cls1 = consts.tile([128, 8, 4, 128], F16, tag="wbig")
            nc.sync.dma_start(
                out=cls1,
                in_=cls1_ext.ap().rearrange("p (k m c) -> p k m c", k=8, m=4))
            clsb1 = consts.tile([128, 4], F32, tag="clsb1")
            nc.sync.dma_start(out=clsb1, in_=clsb1_ext[:, :])
            cls2 = consts.tile([128, 4, 15], F16, tag="cls2")
            nc.sync.dma_start(
                out=cls2, in_=cls2_ext.ap().rearrange("p (k j) -> p k j", k=4))
            clsb2 = consts.tile([15, 1], F32, tag="clsb2")
            nc.sync.dma_start(out=clsb2, in_=clsb2_ext[:, :])
            mexp = consts.tile([15, 15], F32, tag="mexp")
            nc.sync.dma_start(out=mexp, in_=mexp_ext[:, :])
            transn = consts.tile([15, 15], F16, tag="transn")
            nc.sync.dma_start(out=transn, in_=transn_ext[:, :])
            crfstart = consts.tile([15, 1], F32, tag="crfstart")
            nc.sync.dma_start(out=crfstart, in_=start_ext[:, :])
            crfend = consts.tile([15, 1], F32, tag="crfend")
            nc.sync.dma_start(out=crfend, in_=end_ext[:, :])
            lnalpha = consts.tile([15, 1], F32, tag="lnalpha")
            nc.sync.dma_start(out=lnalpha, in_=lna_ext[:, :])
            tago = consts.tile([15, TB3], F16, tag="tago")
            nc.sync.dma_start(out=tago, in_=tago_ext[:, :])

            logits = seqs.tile([15, TB3], F32, tag="logits")

            NT = 64  # t-steps per classifier n-tile (NT*BL3 = 512 cols)
            HT3 = T // 2
            SHSZH = 128 * 2 * 4 * HT3  # elements per half recv shard

            def comb_ap(kk, bh, ns):
                # [128, 4, NT]: dir kk//2, chunk kk%2, batch-half bh, n-tile
                # ns; ascending t window (reversal done in the rhs view),
                # routed to the t-half recv tensor containing the window
                d, c = kk // 2, kk % 2
                rev = d in (1, 3)  # c1b, w1b stored time-reversed
                w = (ns * NT) if not rev else ((T - NT) - ns * NT)
                tensor = recvbA if w < HT3 else recvbB
                off = ((2 * d + bh) * SHSZH + c * (4 * HT3)
                       + (w if w < HT3 else w - HT3))
                return bass.AP(tensor=tensor, offset=off,
                               ap=[[2 * 4 * HT3, 128], [HT3, 4], [1, NT]])

            for ns in range(8):
                comb = stagep.tile([128, 8, BL3, NT], F16, tag="comb", bufs=2)
                for kk in range(8):
                    for half in range(2):
                        nc.sync.dma_start(out=comb[:, kk, ds(4 * half, 4)],
                                          in_=comb_ap(kk, half, ns))
                hmt = []
                for m in range(4):
                    ps = ps_big.tile([128, NT * BL3], F32, tag="gemmps")
                    for kk in range(8):
                        if kk // 2 in (1, 3):
                            p_step = comb.ap[0][0]
                            off = (comb.offset + kk * BL3 * NT + (NT - 1))
                            rhs = bass.AP(tensor=comb.tensor, offset=off,
                                          ap=[[p_step, 128], [-1, NT],
                                              [NT, BL3]])
                        else:
                            rhs = comb[:, kk].rearrange("p b t -> p t b")
                        nc.tensor.matmul(ps, cls1[:, kk, m], rhs,
                                         start=(kk == 0), stop=(kk == 7))
                    hm = stagep.tile([128, NT * BL3], F16, tag="hm", bufs=4,
                                     name=f"hm{m}")
                    nc.scalar.activation(hm, ps, AF.Relu, bias=clsb1[:, m: m + 1])
                    hmt.append(hm)
                ps2 = ps_small.tile([15, NT * BL3], F32, tag="small")
                for m in range(4):
                    nc.tensor.matmul(ps2, cls2[:, m], hmt[m],
                                     start=(m == 0), stop=(m == 3))
                nc.vector.tensor_scalar_add(
                    logits[:, ds(ns * NT * BL3, NT * BL3)], ps2, clsb2)

            # fold CRF start/end into first/last emission columns
            nc.vector.tensor_scalar_add(logits[:, 0:BL3], logits[:, 0:BL3], crfstart)
            nc.vector.tensor_scalar_add(logits[:, TB3 - BL3: TB3],
                                        logits[:, TB3 - BL3: TB3], crfend)

            # ---- CRF numerator ----
            racc = work.tile([15, 16], F32, tag="racc")
            nc.vector.memset(racc, 0.0)
            for ns in range(8):
                pre = stagep.tile([15, 512], F32, tag="prodns")
                nc.vector.tensor_mul(pre, logits[:, ds(ns * 512, 512)],
                                     tago[:, ds(ns * 512, 512)])
                nc.vector.tensor_reduce(racc[:, 8 + ns: 9 + ns], pre,
                                        axis=mybir.AxisListType.X,
                                        op=mybir.AluOpType.add)
                psv = ps_small.tile([15, 512], F32, tag="small")
                nc.tensor.matmul(psv, transn, tago[:, ds(ns * 512, 512)],
                                 start=True, stop=True)
                w = 512 if ns < 7 else 512 - BL3
                pr = stagep.tile([15, 512], F32, tag="prodns")
                nc.vector.tensor_mul(pr[:, :w], psv[:, :w],
                                     tago[:, ds(ns * 512 + BL3, w)])
                nc.vector.tensor_reduce(racc[:, ns: ns + 1], pr[:, :w],
                                        axis=mybir.AxisListType.X,
                                        op=mybir.AluOpType.add)
            nv = stagep.tile([15, 1], F32, tag="nv")
            nc.vector.tensor_reduce(nv, racc, axis=mybir.AxisListType.X,
                                    op=mybir.AluOpType.add)
            ones15 = consts.tile([15, 1], F32, tag="ones15")
            nc.vector.memset(ones15, 1.0)
            psn = ps_small.tile([1, 1], F32, tag="small")
            nc.tensor.matmul(psn, ones15, nv, start=True, stop=True)
            num11 = work.tile([1, 1], F32, tag="num11")
            nc.vector.tensor_copy(num11, psn)

            # ---- CRF forward scan, probability space ----
            # two independent half-batch chains so the 15x15 matmul of one
            # overlaps the vector multiply of the other. p0 first, then
            # E = alpha*exp(logits) in place (last col already has e^end).
            pA = seqs.tile([15, 4], F32, tag="pvecA")
            pB = seqs.tile([15, 4], F32, tag="pvecB")
            nc.scalar.activation(pA, logits[:, 0:4], AF.Exp)
            nc.scalar.activation(pB, logits[:, 4:8], AF.Exp)
            E = logits
            nc.scalar.activation(E, logits, AF.Exp, bias=lnalpha)

            def crf_step(col):
                zA = ps_small.tile([15, 4], F32, tag="small", name="zA")
                nc.tensor.matmul(zA, mexp, pA, start=True, stop=True)
                zB = ps_small.tile([15, 4], F32, tag="small", name="zB")
                nc.tensor.matmul(zB, mexp, pB, start=True, stop=True)
                nc.vector.tensor_mul(pA, zA, E[:, col: col + 4] if isinstance(col, int)
                                     else E[:, ds(col, 4)])
                nc.vector.tensor_mul(pB, zB, E[:, col + 4: col + 8] if isinstance(col, int)
                                     else E[:, ds(col + 4, 4)])

            for t in range(1, 16):
                crf_step(t * BL3)
            with tc.For_i(0, 496, UNROLL) as tv:
                for j in range(UNROLL):
                    crf_step((16 + j) * BL3 + tv * BL3)

            # ---- denominator + output ----
            den11 = work.tile([1, 1], F32, tag="den11")
            for idx, pv in enumerate((pA, pB)):
                psd = ps_small.tile([1, 4], F32, tag="small", name=f"psd{idx}")
                nc.tensor.matmul(psd, ones15, pv, start=True, stop=True)
                ln4 = stagep.tile([1, 4], F32, tag="ln8", name=f"ln4{idx}")
                nc.scalar.activation(ln4, psd, AF.Ln)
                if idx == 0:
                    nc.vector.tensor_reduce(den11, ln4, axis=mybir.AxisListType.X,
                                            op=mybir.AluOpType.add)
                else:
                    dh = work.tile([1, 1], F32, tag="denh")
                    nc.vector.tensor_reduce(dh, ln4, axis=mybir.AxisListType.X,
                                            op=mybir.AluOpType.add)
                    nc.vector.tensor_add(den11, den11, dh)
            res = work.tile([1, 1], F32, tag="res")
            nc.vector.tensor_sub(res, den11, num11)
            nc.sync.dma_start(out=out_ext[:, :], in_=res)

    nc.finalize()
    _BUILD_CACHE["nc"] = nc
    return nc


# ---- host-side input prep ---------------------------------------------------

# gate perm [i(256), f(256), g(256), o(256)] -> [i, f, o, g~]
_GPERM = np.concatenate([np.arange(0, 512), np.arange(768, 1024), np.arange(512, 768)])

# core c -> (pathway, direction, half): 0..3 char f/f/b/b, 4..7 word
_ROLES = [("c", 0, 0), ("c", 0, 1), ("c", 1, 0), ("c", 1, 1),
          ("w", 0, 0), ("w", 0, 1), ("w", 1, 0), ("w", 1, 1)]


def _wih_prep(W, dk_n):
    Wp = W[_GPERM]
    return np.ascontiguousarray(
        Wp.reshape(8, 128, dk_n, 128).transpose(3, 2, 0, 1).reshape(128, dk_n * 8 * 128)
    ).astype(np.float16)


def _make_in_maps(inputs):
    char_ids = np.asarray(inputs["char_ids"])
    tags = np.asarray(inputs["tags"])
    wemb = np.asarray(inputs["word_embeddings"], np.float32)
    emb = np.asarray(inputs["char_emb_table"], np.float32)
    trans = np.asarray(inputs["crf_trans"], np.float32)

    alpha = 1.0 / (15.0 * float(np.exp(trans).mean()))
    common = {}
    w1 = np.asarray(inputs["cls_w1"], np.float32)
    common["cls1"] = np.ascontiguousarray(
        w1.reshape(4, 128, 8, 128).transpose(3, 2, 0, 1).reshape(128, 8 * 4 * 128)
    ).astype(np.float16)
    common["clsb1"] = np.ascontiguousarray(
        np.asarray(inputs["cls_b1"], np.float32).reshape(4, 128).T).astype(np.float32)
    w2 = np.asarray(inputs["cls_w2"], np.float32)
    common["cls2"] = np.ascontiguousarray(
        w2.reshape(15, 4, 128).transpose(2, 1, 0).reshape(128, 4 * 15)).astype(np.float16)
    common["clsb2"] = np.asarray(inputs["cls_b2"], np.float32).reshape(15, 1).copy()
    common["mexp"] = np.exp(trans).astype(np.float32)
    common["transn"] = trans.astype(np.float16)
    common["crfstart"] = np.asarray(inputs["crf_start"], np.float32).reshape(15, 1).copy()
    common["crfend"] = np.asarray(inputs["crf_end"], np.float32).reshape(15, 1).copy()
    common["lnalpha"] = np.full((15, 1), np.log(alpha), np.float32)

    in_maps = []
    for c in range(NC_N):
        pw, d, hf = _ROLES[c]
        lo, hi = hf * BL2, (hf + 1) * BL2
        m = dict(common)

        # phase-1 weights/input
        if pw == "c":
            Wih1 = np.zeros((1024, 768), np.float32)
            Wih1[:, :128] = np.asarray(inputs["c0_Wih"], np.float32)[d]
            Whh1 = np.asarray(inputs["c0_Whh"], np.float32)[d]
            b1 = (np.asarray(inputs["c0_bih"], np.float32)[d]
                  + np.asarray(inputs["c0_bhh"], np.float32)[d])
            ce = emb[char_ids[lo:hi]]  # (32, 512, 128)
            X = np.zeros((128, DK1, T, BL2), np.float32)
            X[:, 0] = ce.transpose(2, 1, 0)
            Wl1 = np.asarray(inputs["c1_Wih"], np.float32)[d]
            Whh2 = np.asarray(inputs["c1_Whh"], np.float32)[d]
            b2 = (np.asarray(inputs["c1_bih"], np.float32)[d]
                  + np.asarray(inputs["c1_bhh"], np.float32)[d])
        else:
            Wih1 = np.asarray(inputs["w0_Wih"], np.float32)[d]
            Whh1 = np.asarray(inputs["w0_Whh"], np.float32)[d]
            b1 = (np.asarray(inputs["w0_bih"], np.float32)[d]
                  + np.asarray(inputs["w0_bhh"], np.float32)[d])
            X = wemb[lo:hi].reshape(BL2, T, DK1, 128).transpose(3, 2, 1, 0)
            Wl1 = np.asarray(inputs["w1_Wih"], np.float32)[d]
            Whh2 = np.asarray(inputs["w1_Whh"], np.float32)[d]
            b2 = (np.asarray(inputs["w1_bih"], np.float32)[d]
                  + np.asarray(inputs["w1_bhh"], np.float32)[d])
        if d == 1:  # backward: reverse local time
            X = X[:, :, ::-1]
        m["x1"] = np.ascontiguousarray(X.reshape(128, DK1, T * BL2)).astype(np.float16)
        m["wih1"] = _wih_prep(Wih1, DK1)
        m["whh1"] = _wih_prep(Whh1, 2)
        m["bias1"] = np.ascontiguousarray(b1[_GPERM].reshape(8, 128).T).astype(np.float32)

        # phase-2 weights: columns [own(256) | peer(256)]
        if d == 1:
            Wl1 = Wl1[:, np.r_[256:512, 0:256]]
        m["wih2"] = _wih_prep(Wl1, DK2)
        m["whh2"] = _wih_prep(Whh2, 2)
        m["bias2"] = np.ascontiguousarray(b2[_GPERM].reshape(8, 128).T).astype(np.float32)
        # blend: f-core (d=0) picks AG region 1 (the b-core), b-core picks 0
        msk = np.zeros((128, 2), np.float32)
        msk[:, 1 - d] = 1.0
        m["masks"] = msk

        # phase-3 tags for this core's 8 sequences
        seqs3 = np.r_[4 * c: 4 * c + 4, 32 + 4 * c: 32 + 4 * c + 4]
        oh = (np.arange(K)[:, None, None] == tags[seqs3][None]).astype(np.float32)
        # (15, 8seq, 512t) -> (15, t, b)
        m["tagoneT"] = np.ascontiguousarray(
            oh.transpose(0, 2, 1).reshape(K, TB3)).astype(np.float16)
        in_maps.append(m)
    return in_maps, alpha


def kernel(**inputs):
    nc = _build_nc()
    in_maps, alpha = _make_in_maps(inputs)
    res = run_bass_kernel_spmd(nc, in_maps, core_ids=list(range(NC_N)))
    total = sum(float(res.results[c]["out"][0, 0]) for c in range(NC_N))
    total -= B * (T - 1) * np.log(alpha)
    return np.float32(total / B)


# revision 25
# speedup vs baseline: 1.1605x; 1.0198x over previous
"""BiLSTM dual-pathway + CRF NLL kernel for 8 Trainium2 NeuronCores.

Sharding: direction-parallel for the LSTM recurrences, data-parallel for the
classifier/CRF. Phase 1 runs the four layer-0 directions (char fwd/bwd, word
fwd/bwd) on 8 cores as (direction x batch-half), batch 32 per core, so the
recurrent matmuls run at N=32 instead of N=8 and the sequential chain count
drops from 4096 to 1024 steps. A pairwise AllGather exchanges the L0 hidden
states between fwd/bwd cores, phase 2 runs the four layer-1 directions the
same way, then an 8-rank AllToAll redistributes hidden states to a
data-parallel layout (8 sequences per core) for the classifier and CRF.

SPMD uniformity: every core runs the identical program. Backward directions
receive host-time-reversed inputs; reversed reads of peer hidden states are
fixed negative-stride APs, with host-permuted weight columns absorbing the
f/b role differences. Peer-region selection after the AllGather uses per-core
0/1 blend masks delivered as input data.

The CRF forward scan runs in probability space with a constant per-step
prescale alpha folded into the emission exponentials (corrected analytically
on the host), so each step is one resident-weight 15x15 matmul plus one
vector multiply -- no per-step exp/ln activation-table swaps.
"""

import sys

sys.path.insert(0, "/opt/trn_rl_repo")

import numpy as np

import concourse.bass as bass
import concourse.mybir as mybir
from concourse import bacc
from concourse.bass import ds
from concourse.tile import TileContext
from concourse.bass_utils import run_bass_kernel_spmd

F16 = mybir.dt.float16
F32 = mybir.dt.float32
AF = mybir.ActivationFunctionType

B, T, V, K = 64, 512, 40, 15
NC_N = 8
BL2 = 32            # batch per core in phases 1-2
BL3 = 8             # sequences per core in phase 3
TB3 = T * BL3       # 4096 classifier/CRF columns per core
DK1, DK2 = 6, 4     # input chunks for L0 (word=768, char padded) and L1 (512)
UNROLL = 16
HCOL = T + 1        # h buffer columns per sequence (col 0 = zero init)

_BUILD_CACHE = {}


def _emit_rec_block(nc, stagep, ps_rec, hc, cst, whh, xgs, xg_dram, tv, fodder,
                    prefetch_tv=None):
    """Emit 16 recurrence steps for time block tv.

    fodder: list of (items, per_step) - closures emitting one PE-side quantum
    each (GEMM matmuls for other pipeline stages), dispensed between the
    step's recurrent matmuls and its vector/scalar chain so they execute in
    the PE's dependency-stall gaps.
    """
    CB = UNROLL * BL2
    cursors = [[items, 0, per_step] for items, per_step in fodder]
    for j in range(UNROLL):
        psA = ps_rec.tile([128, 4, BL2], F32, tag="recpsA")
        psB = ps_rec.tile([128, 4, BL2], F32, tag="recpsB")
        # i,f gates (chunks 0-3) first into psA so the first g-add can
        # start while the o/g~ matmuls (psB) still run
        for m in (0, 1, 2, 3):
            for k in range(2):
                nc.tensor.matmul(psA[:, m], whh[:, k, m],
                                 hc[:, k, j], start=(k == 0), stop=(k == 1))
        for m in (6, 7, 4, 5):
            for k in range(2):
                nc.tensor.matmul(psB[:, m - 4], whh[:, k, m],
                                 hc[:, k, j], start=(k == 0), stop=(k == 1))
        for cur in cursors:
            for _ in range(cur[2]):
                if cur[1] < len(cur[0]):
                    cur[0][cur[1]]()
                    cur[1] += 1
        g = stagep.tile([128, 8, BL2], F16, tag="g")
        xsl = xgs[:, :, ds(j * BL2, BL2)]
        nc.vector.tensor_add(g[:, 0:4], psA, xsl[:, 0:4])
        sig = stagep.tile([128, 6, BL2], F16, tag="sig")
        nc.scalar.activation(sig[:, 0:4], g[:, 0:4], AF.Sigmoid)
        nc.vector.tensor_mul(cst, cst, sig[:, 2:4])
        nc.vector.tensor_add(g[:, 4:8], psB, xsl[:, 4:8])
        tgg = stagep.tile([128, 2, BL2], F16, tag="tgg")
        nc.scalar.activation(tgg, g[:, 6:8], AF.Tanh)
        tmp = stagep.tile([128, 2, BL2], F16, tag="tmpig")
        nc.vector.tensor_mul(tmp, sig[:, 0:2], tgg)
        nc.scalar.activation(sig[:, 4:6], g[:, 4:6], AF.Sigmoid)
        nc.vector.tensor_add(cst, cst, tmp)
        tch = stagep.tile([128, 2, BL2], F16, tag="tch")
        nc.scalar.activation(tch, cst, AF.Tanh)
        nc.vector.tensor_mul(hc[:, :, j + 1], sig[:, 4:6], tch)
    for cur in cursors:
        while cur[1] < len(cur[0]):
            cur[0][cur[1]]()
            cur[1] += 1
    if prefetch_tv is not None:
        # refill this block's (just-consumed) xgs buffer with the same-parity
        # block two ahead -- a full block of DMA lead time
        nc.sync.dma_start(out=xgs, in_=xg_dram[:, :, ds(prefetch_tv * BL2, CB)])


def _build_nc():
    if "nc" in _BUILD_CACHE:
        return _BUILD_CACHE["nc"]
    nc = bacc.Bacc(target_bir_lowering=False, num_devices=NC_N)

    # ---- external parameters -------------------------------------------------
    x1_ext = nc.declare_dram_parameter("x1", [128, DK1, T * BL2], F16, isOutput=False)
    wih1_ext = nc.declare_dram_parameter("wih1", [128, DK1 * 8 * 128], F16, isOutput=False)
    whh1_ext = nc.declare_dram_parameter("whh1", [128, 2 * 8 * 128], F16, isOutput=False)
    bias1_ext = nc.declare_dram_parameter("bias1", [128, 8], F32, isOutput=False)
    wih2_ext = nc.declare_dram_parameter("wih2", [128, DK2 * 8 * 128], F16, isOutput=False)
    whh2_ext = nc.declare_dram_parameter("whh2", [128, 2 * 8 * 128], F16, isOutput=False)
    bias2_ext = nc.declare_dram_parameter("bias2", [128, 8], F32, isOutput=False)
    masks_ext = nc.declare_dram_parameter("masks", [128, 2], F32, isOutput=False)
    cls1_ext = nc.declare_dram_parameter("cls1", [128, 8 * 4 * 128], F16, isOutput=False)
    clsb1_ext = nc.declare_dram_parameter("clsb1", [128, 4], F32, isOutput=False)
    cls2_ext = nc.declare_dram_parameter("cls2", [128, 4 * 15], F16, isOutput=False)
    clsb2_ext = nc.declare_dram_parameter("clsb2", [15, 1], F32, isOutput=False)
    mexp_ext = nc.declare_dram_parameter("mexp", [15, 15], F32, isOutput=False)
    transn_ext = nc.declare_dram_parameter("transn", [15, 15], F16, isOutput=False)
    start_ext = nc.declare_dram_parameter("crfstart", [15, 1], F32, isOutput=False)
    end_ext = nc.declare_dram_parameter("crfend", [15, 1], F32, isOutput=False)
    lna_ext = nc.declare_dram_parameter("lnalpha", [15, 1], F32, isOutput=False)
    tago_ext = nc.declare_dram_parameter("tagoneT", [15, TB3], F16, isOutput=False)
    out_ext = nc.declare_dram_parameter("out", [1, 1], F32, isOutput=True)

    # ---- internal DRAM -------------------------------------------------------
    HSZ = 2 * BL2 * T  # 32768 cols/partition of h (f16)
    xg1_dram = nc.dram_tensor("xg1", [128, 8, T * BL2], F16)
    xg2_dram = nc.dram_tensor("xg2", [128, 8, T * BL2], F16)
    HSZH = HSZ // 2  # per-partition elements of a t-half of h
    hminA = nc.dram_tensor("hminA", [128, HSZH], F16)
    hminB = nc.dram_tensor("hminB", [128, HSZH], F16)
    agoutA = nc.dram_tensor("agoutA", [2, 128, HSZH], F16)
    agoutB = nc.dram_tensor("agoutB", [2, 128, HSZH], F16)
    sendbA = nc.dram_tensor("sendbA", [8, 128, 2 * 4 * (T // 2)], F16)
    sendbB = nc.dram_tensor("sendbB", [8, 128, 2 * 4 * (T // 2)], F16)
    recvbA = nc.dram_tensor("recvbA", [8, 128, 2 * 4 * (T // 2)], F16)
    recvbB = nc.dram_tensor("recvbB", [8, 128, 2 * 4 * (T // 2)], F16)

    with TileContext(nc) as tc:
        with (
            tc.tile_pool(name="consts", bufs=1) as consts,
            tc.tile_pool(name="seqs", bufs=1) as seqs,
            tc.tile_pool(name="work", bufs=2) as work,
            tc.tile_pool(name="stage", bufs=2) as stagep,
            tc.tile_pool(name="ps_big", bufs=3, space="PSUM") as ps_big,
            tc.tile_pool(name="ps_rec", bufs=1, space="PSUM") as ps_rec,
            tc.tile_pool(name="ps_small", bufs=3, space="PSUM") as ps_small,
        ):
            # h buffer, seq-major: [128, chunk2, b32, T+1], reused by phases 1+2
            h_sb = seqs.tile([128, 2, BL2, HCOL], F16, tag="h_sb")
            hc = seqs.tile([128, 2, UNROLL + 1, BL2], F16, tag="hcomp")
            cst = seqs.tile([128, 2, BL2], F32, tag="cstate")
            masks = consts.tile([128, 2], F32, tag="masks")
            nc.sync.dma_start(out=masks, in_=masks_ext[:, :])

            # ================= PHASE 1 (L0) =================
            wih1 = consts.tile([128, DK1, 8, 128], F16, tag="wbig")
            nc.sync.dma_start(
                out=wih1,
                in_=wih1_ext.ap().rearrange("p (k m c) -> p k m c", k=DK1, m=8))
            whh1 = consts.tile([128, 2, 8, 128], F16, tag="whh")
            nc.sync.dma_start(
                out=whh1,
                in_=whh1_ext.ap().rearrange("p (k m c) -> p k m c", k=2, m=8))
            bias1 = consts.tile([128, 8], F32, tag="bias1")
            nc.sync.dma_start(out=bias1, in_=bias1_ext[:, :])
            # wih2/bias2 load now: the L1 input GEMM's slotA half runs
            # interleaved inside the phase-1 recurrence
            wih2 = consts.tile([128, DK2, 8, 128], F16, tag="wih2")
            nc.sync.dma_start(
                out=wih2,
                in_=wih2_ext.ap().rearrange("p (k m c) -> p k m c", k=DK2, m=8))
            bias2 = consts.tile([128, 8], F32, tag="bias2")
            nc.sync.dma_start(out=bias2, in_=bias2_ext[:, :])

            CB = UNROLL * BL2
            # phase-1 h layout in DRAM is t-major: (p, c, t, b), split in
            # two t-halves so the first AllGather can overlap the rec tail
            hmvA = hminA.ap().rearrange("p (c t b) -> p c t b", c=2, t=T // 2)
            hmvB = hminB.ap().rearrange("p (c t b) -> p c t b", c=2, t=T // 2)

            def g1_items(tv_lead):
                # xg1 block tv_lead: input DMA now, 48 matmul quanta
                xb = stagep.tile([128, DK1, CB], F16, tag="xb1")
                nc.sync.dma_start(out=xb, in_=x1_ext[:, :, ds(tv_lead * BL2, CB)])
                items, pss = [], {}
                for m in range(8):
                    for k in range(DK1):
                        def it(m=m, k=k):
                            if k == 0:
                                pss[m] = ps_big.tile([128, CB], F32, tag="gemmps",
                                                     name=f"g1ps{m}")
                            nc.tensor.matmul(pss[m], wih1[:, k, m], xb[:, k],
                                             start=(k == 0), stop=(k == DK1 - 1))
                            if k == DK1 - 1:
                                st = stagep.tile([128, CB], F16, tag="xgstage",
                                                 name=f"g1st{m}")
                                nc.scalar.activation(st, pss[m], AF.Identity,
                                                     bias=bias1[:, m: m + 1])
                                nc.sync.dma_start(
                                    out=xg1_dram[:, m, ds(tv_lead * BL2, CB)], in_=st)
                        items.append(it)
                return items

            def p1_end(tv, half):
                if half == 0:
                    nc.sync.dma_start(out=hmvA[:, :, ds(tv, UNROLL)],
                                      in_=hc[:, :, 1: UNROLL + 1])
                else:
                    nc.sync.dma_start(out=hmvB[:, :, ds(tv - T // 2, UNROLL)],
                                      in_=hc[:, :, 1: UNROLL + 1])
                nc.gpsimd.tensor_copy(hc[:, :, 0], hc[:, :, UNROLL])

            xgsA = stagep.tile([128, 8, CB], F16, tag="xgs", name="xgsA")
            xgsB = stagep.tile([128, 8, CB], F16, tag="xgs", name="xgsB")

            def rec1_block(tv, fodder, xgs, pf, half):
                _emit_rec_block(nc, stagep, ps_rec, hc, cst, whh1, xgs,
                                xg1_dram, tv, fodder, prefetch_tv=pf)
                p1_end(tv, half)

            nc.vector.memset(cst, 0.0)
            nc.vector.memset(hc[:, :, 0], 0.0)
            for blk in range(2):
                for it in g1_items(16 * blk):
                    it()
            nc.sync.dma_start(out=xgsA, in_=xg1_dram[:, :, ds(0, CB)])
            nc.sync.dma_start(out=xgsB, in_=xg1_dram[:, :, ds(16 * BL2, CB)])
            rec1_block(0, [(g1_items(32), 3)], xgsA, 32, 0)
            rec1_block(16, [(g1_items(48), 3)], xgsB, 48, 0)

            def p1_loop(tv, half):
                itemsA = g1_items(tv + 32)
                itemsB = g1_items(tv + 48)  # DMA issued early: a block of lead
                _emit_rec_block(nc, stagep, ps_rec, hc, cst, whh1, xgsA,
                                xg1_dram, tv, [(itemsA, 3)], prefetch_tv=tv + 32)
                p1_end(tv, half)
                _emit_rec_block(nc, stagep, ps_rec, hc, cst, whh1, xgsB,
                                xg1_dram, tv + 16, [(itemsB, 3)],
                                prefetch_tv=tv + 48)
                p1_end(tv + 16, half)

            with tc.For_i(32, 256, 32) as tv:
                p1_loop(tv, 0)
            # first-half h is complete: exchange it while the rec tail runs
            nc.gpsimd.collective_compute(
                "AllGather", mybir.AluOpType.bypass,
                replica_groups=[[0, 2], [1, 3], [4, 6], [5, 7]],
                ins=[hminA.ap()], outs=[agoutA.ap()],
            )
            with tc.For_i(256, 480, 32) as tv:
                p1_loop(tv, 1)
            rec1_block(480, [], xgsA, None, 1)
            rec1_block(496, [], xgsB, None, 1)
            nc.gpsimd.collective_compute(
                "AllGather", mybir.AluOpType.bypass,
                replica_groups=[[0, 2], [1, 3], [4, 6], [5, 7]],
                ins=[hminB.ap()], outs=[agoutB.ap()],
            )

            # ================= PHASE 2 (L1) =================
            whh2 = consts.tile([128, 2, 8, 128], F16, tag="whh")
            nc.sync.dma_start(
                out=whh2,
                in_=whh2_ext.ap().rearrange("p (k m c) -> p k m c", k=2, m=8))

            def slot_ap(tensor, region, c, w_local):
                # [128, UNROLL*32] chunk c of a t-major half-h DRAM region
                # (p, c2, T/2, b32): contiguous ascending t window (w_local
                # in half-local units); reversed slots flip t SBUF-side.
                off = (region * 128 * HSZH + c * ((T // 2) * BL2)
                       + w_local * BL2)
                return bass.AP(tensor=tensor, offset=off,
                               ap=[[HSZH, 128], [1, UNROLL * BL2]])

            def g2_items(tv_lead, sa_t, sa_w, ag_t, rev_w):
                # full xg2 block: slotA = own h fwd, slotB = peer h reversed
                sa = stagep.tile([128, 2, UNROLL, BL2], F16, tag="slotA")
                r0 = stagep.tile([128, 2, UNROLL, BL2], F16, tag="slotR0")
                r1 = stagep.tile([128, 2, UNROLL, BL2], F16, tag="slotR1")
                for c in range(2):
                    nc.sync.dma_start(out=sa[:, c],
                                      in_=slot_ap(sa_t, 0, c, sa_w))
                    nc.sync.dma_start(out=r0[:, c],
                                      in_=slot_ap(ag_t, 0, c, rev_w))
                    nc.sync.dma_start(out=r1[:, c],
                                      in_=slot_ap(ag_t, 1, c, rev_w))
                sbr = stagep.tile([128, 2, UNROLL, BL2], F16, tag="slotBr")
                items, pss = [], {}

                def blend0():
                    nc.vector.tensor_scalar_mul(r0, r0, masks[:, 0:1])

                def blend1():
                    nc.vector.tensor_scalar_mul(r1, r1, masks[:, 1:2])

                def blend2():
                    # add, writing with the t window reversed
                    p_step = sbr.ap[0][0]
                    rev = bass.AP(tensor=sbr.tensor,
                                  offset=sbr.offset + (UNROLL - 1) * BL2,
                                  ap=[[p_step, 128], [UNROLL * BL2, 2],
                                      [-BL2, UNROLL], [1, BL2]])
                    nc.vector.tensor_add(rev, r0, r1)

                items += [blend0, blend1, blend2]

                def src_k(k):
                    t = sa if k < 2 else sbr
                    return t[:, k % 2].rearrange("p t b -> p (t b)")

                for m in range(8):
                    for k in range(DK2):
                        def it(m=m, k=k):
                            if k == 0:
                                pss[m] = ps_big.tile([128, CB], F32, tag="gemmps",
                                                     name=f"g2ps{m}")
                            nc.tensor.matmul(pss[m], wih2[:, k, m], src_k(k),
                                             start=(k == 0), stop=(k == DK2 - 1))
                            if k == DK2 - 1:
                                st = stagep.tile([128, CB], F16, tag="xgstage",
                                                 name=f"g2st{m}")
                                nc.scalar.activation(st, pss[m], AF.Identity,
                                                     bias=bias2[:, m: m + 1])
                                nc.sync.dma_start(
                                    out=xg2_dram[:, m, ds(tv_lead * BL2, CB)], in_=st)
                        items.append(it)
                return items

            def p2_end(tv):
                nc.gpsimd.tensor_copy(
                    h_sb[:, :, :, ds(tv + 1, UNROLL)],
                    hc[:, :, 1: UNROLL + 1].rearrange("p c t b -> p c b t"))
                nc.gpsimd.tensor_copy(hc[:, :, 0], hc[:, :, UNROLL])

            nc.vector.memset(cst, 0.0)
            nc.vector.memset(hc[:, :, 0], 0.0)
            HT = T // 2
            # leads 0,1: own h from half A, peer (reversed) from half B
            for blk in (0, 1):
                L = 16 * blk
                for it in g2_items(L, hminA, L, agoutB, (T - UNROLL) - L - HT):
                    it()
            nc.sync.dma_start(out=xgsA, in_=xg2_dram[:, :, ds(0, CB)])
            nc.sync.dma_start(out=xgsB, in_=xg2_dram[:, :, ds(16 * BL2, CB)])

            def p2_loop(tv, sa_t, sa_base, ag_t, rev_base):
                # halves at leads tv+32, tv+48; window args are half-local
                itemsA = g2_items(tv + 32, sa_t, sa_base + tv, ag_t,
                                  rev_base - tv)
                itemsB = g2_items(tv + 48, sa_t, sa_base + tv + 16, ag_t,
                                  rev_base - tv - 16)
                _emit_rec_block(nc, stagep, ps_rec, hc, cst, whh2, xgsA,
                                xg2_dram, tv, [(itemsA, 3)], prefetch_tv=tv + 32)
                p2_end(tv)
                _emit_rec_block(nc, stagep, ps_rec, hc, cst, whh2, xgsB,
                                xg2_dram, tv + 16, [(itemsB, 3)],
                                prefetch_tv=tv + 48)
                p2_end(tv + 16)

            with tc.For_i(0, 224, 32) as tv:
                # leads 2..15: own h half A, peer half B
                p2_loop(tv, hminA, 32, agoutB, (T - UNROLL) - 32 - HT)
            with tc.For_i(224, 256, 32) as tv:
                # blocks 14,15: own h half B, peer half A
                p2_loop(tv, hminB, 32 - HT, agoutA, (T - UNROLL) - 32)
            # first t-half of phase-2 h is complete: redistribute it while
            # the rec tail runs
            sbA_view = sendbA.ap().rearrange("j p (c b t) -> j p c b t",
                                             c=2, b=4)
            for j in range(8):
                for c in range(2):
                    nc.sync.dma_start(out=sbA_view[j, :, c],
                                      in_=h_sb[:, c, ds(4 * j, 4), 1: 1 + HT])
            nc.gpsimd.collective_compute(
                "AllToAll", mybir.AluOpType.bypass,
                replica_groups=[list(range(8))],
                ins=[sendbA.ap()], outs=[recvbA.ap()],
            )
            with tc.For_i(256, 480, 32) as tv:
                p2_loop(tv, hminB, 32 - HT, agoutA, (T - UNROLL) - 32)
            _emit_rec_block(nc, stagep, ps_rec, hc, cst, whh2, xgsA,
                            xg2_dram, 480, [])
            p2_end(480)
            _emit_rec_block(nc, stagep, ps_rec, hc, cst, whh2, xgsB,
                            xg2_dram, 496, [])
            p2_end(496)

            sbB_view = sendbB.ap().rearrange("j p (c b t) -> j p c b t",
                                             c=2, b=4)
            for j in range(8):
                for c in range(2):
                    nc.sync.dma_start(out=sbB_view[j, :, c],
                                      in_=h_sb[:, c, ds(4 * j, 4),
                                               1 + HT: HCOL])
            nc.gpsimd.collective_compute(
                "AllToAll", mybir.AluOpType.bypass,
                replica_groups=[list(range(8))],
                ins=[sendbB.ap()], outs=[recvbB.ap()],
            )

            # ================= PHASE 3: classifier + CRF =================
            cls1 = consts.tile([128, 8, 4, 128], F16, tag="wbig")
            nc.sync.dma_start(
                out=cls1,
                in_=cls1_ext.ap().rearrange("p (k m c) -> p k m c", k=8, m=4))
            clsb1 = consts.tile([128, 4], F32, tag="clsb1")
            nc.sync.dma_start(out=clsb1, in_=clsb1_ext[:, :])
            cls2 = consts.tile([128, 4, 15], F16, tag="cls2")
            nc.sync.dma_start(
                out=cls2, in_=cls2_ext.ap().rearrange("p (k j) -> p k j", k=4))
            clsb2 = consts.tile([15, 1], F32, tag="clsb2")
            nc.sync.dma_start(out=clsb2, in_=clsb2_ext[:, :])
            mexp = consts.tile([15, 15], F32, tag="mexp")
            nc.sync.dma_start(out=mexp, in_=mexp_ext[:, :])
            transn = consts.tile([15, 15], F16, tag="transn")
            nc.sync.dma_start(out=transn, in_=transn_ext[:, :])
            crfstart = consts.tile([15, 1], F32, tag="crfstart")
            nc.sync.dma_start(out=crfstart, in_=start_ext[:, :])
            crfend = consts.tile([15, 1], F32, tag="crfend")
            nc.sync.dma_start(out=crfend, in_=end_ext[:, :])
            lnalpha = consts.tile([15, 1], F32, tag="lnalpha")
            nc.sync.dma_start(out=lnalpha, in_=lna_ext[:, :])
            tago = consts.tile([15, TB3], F16, tag="tago")
            nc.sync.dma_start(out=tago, in_=tago_ext[:, :])

            logits = seqs.tile([15, TB3], F32, tag="logits")

            NT = 64  # t-steps per classifier n-tile (NT*BL3 = 512 cols)
            HT3 = T // 2
            SHSZH = 128 * 2 * 4 * HT3  # elements per half recv shard

            def comb_ap(kk, bh, ns):
                # [128, 4, NT]: dir kk//2, chunk kk%2, batch-half bh, n-tile
                # ns; ascending t window (reversal done in the rhs view),
                # routed to the t-half recv tensor containing the window
                d, c = kk // 2, kk % 2
                rev = d in (1, 3)  # c1b, w1b stored time-reversed
                w = (ns * NT) if not rev else ((T - NT) - ns * NT)
                tensor = recvbA if w < HT3 else recvbB
                off = ((2 * d + bh) * SHSZH + c * (4 * HT3)
                       + (w if w < HT3 else w - HT3))
                return bass.AP(tensor=tensor, offset=off,
                               ap=[[2 * 4 * HT3, 128], [HT3, 4], [1, NT]])

            for ns in range(8):
                comb = stagep.tile([128, 8, BL3, NT], F16, tag="comb", bufs=2)
                for kk in range(8):
                    for half in range(2):
                        nc.sync.dma_start(out=comb[:, kk, ds(4 * half, 4)],
                                          in_=comb_ap(kk, half, ns))
                hmt = []
                for m in range(4):
                    ps = ps_big.tile([128, NT * BL3], F32, tag="gemmps")
                    for kk in range(8):
                        if kk // 2 in (1, 3):
                            p_step = comb.ap[0][0]
                            off = (comb.offset + kk * BL3 * NT + (NT - 1))
                            rhs = bass.AP(tensor=comb.tensor, offset=off,
                                          ap=[[p_step, 128], [-1, NT],
                                              [NT, BL3]])
                        else:
                            rhs = comb[:, kk].rearrange("p b t -> p t b")
                        nc.tensor.matmul(ps, cls1[:, kk, m], rhs,
                                         start=(kk == 0), stop=(kk == 7))
                    hm = stagep.tile([128, NT * BL3], F16, tag="hm", bufs=4,
                                     name=f"hm{m}")
                    nc.scalar.activation(hm, ps, AF.Relu, bias=clsb1[:, m: m + 1])
                    hmt.append(hm)
                ps2 = ps_small.tile([15, NT * BL3], F32, tag="small")
                for m in range(4):
                    nc.tensor.matmul(ps2, cls2[:, m], hmt[m],
                                     start=(m == 0), stop=(m == 3))
                nc.vector.tensor_scalar_add(
                    logits[:, ds(ns * NT * BL3, NT * BL3)], ps2, clsb2)

            # fold CRF start/end into first/last emission columns
            nc.vector.tensor_scalar_add(logits[:, 0:BL3], logits[:, 0:BL3], crfstart)
            nc.vector.tensor_scalar_add(logits[:, TB3 - BL3: TB3],
                                        logits[:, TB3 - BL3: TB3], crfend)

            # ---- CRF numerator ----
            racc = work.tile([15, 16], F32, tag="racc")
            nc.vector.memset(racc, 0.0)
            for ns in range(8):
                pre = stagep.tile([15, 512], F32, tag="prodns")
                nc.vector.tensor_mul(pre, logits[:, ds(ns * 512, 512)],
                                     tago[:, ds(ns * 512, 512)])
                nc.vector.tensor_reduce(racc[:, 8 + ns: 9 + ns], pre,
                                        axis=mybir.AxisListType.X,
                                        op=mybir.AluOpType.add)
                psv = ps_small.tile([15, 512], F32, tag="small")
                nc.tensor.matmul(psv, transn, tago[:, ds(ns * 512, 512)],
                                 start=True, stop=True)
                w = 512 if ns < 7 else 512 - BL3
                pr = stagep.tile([15, 512], F32, tag="prodns")
                nc.vector.tensor_mul(pr[:, :w], psv[:, :w],
                                     tago[:, ds(ns * 512 + BL3, w)])
                nc.vector.tensor_reduce(racc[:, ns: ns + 1], pr[:, :w],
                                        axis=mybir.AxisListType.X,
                                        op=mybir.AluOpType.add)
            nv = stagep.tile([15, 1], F32, tag="nv")
            nc.vector.tensor_reduce(nv, racc, axis=mybir.AxisListType.X,
                                    op=mybir.AluOpType.add)
            ones15 = consts.tile([15, 1], F32, tag="ones15")
            nc.vector.memset(ones15, 1.0)
            psn = ps_small.tile([1, 1], F32, tag="small")
            nc.tensor.matmul(psn, ones15, nv, start=True, stop=True)
            num11 = work.tile([1, 1], F32, tag="num11")
            nc.vector.tensor_copy(num11, psn)

            # ---- CRF forward scan, probability space ----
            # two independent half-batch chains so the 15x15 matmul of one
            # overlaps the vector multiply of the other. p0 first, then
            # E = alpha*exp(logits) in place (last col already has e^end).
            pA = seqs.tile([15, 4], F32, tag="pvecA")
            pB = seqs.tile([15, 4], F32, tag="pvecB")
            nc.scalar.activation(pA, logits[:, 0:4], AF.Exp)
            nc.scalar.activation(pB, logits[:, 4:8], AF.Exp)
            E = logits
            nc.scalar.activation(E, logits, AF.Exp, bias=lnalpha)

            def crf_step(col):
                zA = ps_small.tile([15, 4], F32, tag="small", name="zA")
                nc.tensor.matmul(zA, mexp, pA, start=True, stop=True)
                zB = ps_small.tile([15, 4], F32, tag="small", name="zB")
                nc.tensor.matmul(zB, mexp, pB, start=True, stop=True)
                nc.vector.tensor_mul(pA, zA, E[:, col: col + 4] if isinstance(col, int)
                                     else E[:, ds(col, 4)])
                nc.vector.tensor_mul(pB, zB, E[:, col + 4: col + 8] if isinstance(col, int)
                                     else E[:, ds(col + 4, 4)])

            for t in range(1, 16):
                crf_step(t * BL3)
            with tc.For_i(0, 496, UNROLL) as tv:
                for j in range(UNROLL):
                    crf_step((16 + j) * BL3 + tv * BL3)

            # ---- denominator + output ----
            den11 = work.tile([1, 1], F32, tag="den11")
            for idx, pv in enumerate((pA, pB)):
                psd = ps_small.tile([1, 4], F32, tag="small", name=f"psd{idx}")
                nc.tensor.matmul(psd, ones15, pv, start=True, stop=True)
                ln4 = stagep.tile([1, 4], F32, tag="ln8", name=f"ln4{idx}")
                nc.scalar.activation(ln4, psd, AF.Ln)
                if idx == 0:
                    nc.vector.tensor_reduce(den11, ln4, axis=mybir.AxisListType.X,
                                            op=mybir.AluOpType.add)
                else:
                    dh = work.tile([1, 1], F32, tag="denh")
                    nc.vector.tensor_reduce(dh, ln4, axis=mybir.AxisListType.X,
                                            op=mybir.AluOpType.add)
                    nc.vector.tensor_add(den11, den11, dh)
            res = work.tile([1, 1], F32, tag="res")
            nc.vector.tensor_sub(res, den11, num11)
            nc.sync.dma_start(out=out_ext[:, :], in_=res)

    nc.finalize()
    _BUILD_CACHE["nc"] = nc
    return nc


# ---- host-side input prep ---------------------------------------------------

# gate perm [i(256), f(256), g(256), o(256)] -> [i, f, o, g~]
_GPERM = np.concatenate([np.arange(0, 512), np.arange(768, 1024), np.arange(512, 768)])

# core c -> (pathway, direction, half): 0..3 char f/f/b/b, 4..7 word
_ROLES = [("c", 0, 0), ("c", 0, 1), ("c", 1, 0), ("c", 1, 1),
          ("w", 0, 0), ("w", 0, 1), ("w", 1, 0), ("w", 1, 1)]


def _wih_prep(W, dk_n):
    Wp = W[_GPERM]
    return np.ascontiguousarray(
        Wp.reshape(8, 128, dk_n, 128).transpose(3, 2, 0, 1).reshape(128, dk_n * 8 * 128)
    ).astype(np.float16)


def _make_in_maps(inputs):
    char_ids = np.asarray(inputs["char_ids"])
    tags = np.asarray(inputs["tags"])
    wemb = np.asarray(inputs["word_embeddings"], np.float32)
    emb = np.asarray(inputs["char_emb_table"], np.float32)
    trans = np.asarray(inputs["crf_trans"], np.float32)

    alpha = 1.0 / (15.0 * float(np.exp(trans).mean()))
    common = {}
    w1 = np.asarray(inputs["cls_w1"], np.float32)
    common["cls1"] = np.ascontiguousarray(
        w1.reshape(4, 128, 8, 128).transpose(3, 2, 0, 1).reshape(128, 8 * 4 * 128)
    ).astype(np.float16)
    common["clsb1"] = np.ascontiguousarray(
        np.asarray(inputs["cls_b1"], np.float32).reshape(4, 128).T).astype(np.float32)
    w2 = np.asarray(inputs["cls_w2"], np.float32)
    common["cls2"] = np.ascontiguousarray(
        w2.reshape(15, 4, 128).transpose(2, 1, 0).reshape(128, 4 * 15)).astype(np.float16)
    common["clsb2"] = np.asarray(inputs["cls_b2"], np.float32).reshape(15, 1).copy()
    common["mexp"] = np.exp(trans).astype(np.float32)
    common["transn"] = trans.astype(np.float16)
    common["crfstart"] = np.asarray(inputs["crf_start"], np.float32).reshape(15, 1).copy()
    common["crfend"] = np.asarray(inputs["crf_end"], np.float32).reshape(15, 1).copy()
    common["lnalpha"] = np.full((15, 1), np.log(alpha), np.float32)

    in_maps = []
    for c in range(NC_N):
        pw, d, hf = _ROLES[c]
        lo, hi = hf * BL2, (hf + 1) * BL2
        m = dict(common)

        # phase-1 weights/input
        if pw == "c":
            Wih1 = np.zeros((1024, 768), np.float32)
            Wih1[:, :128] = np.asarray(inputs["c0_Wih"], np.float32)[d]
            Whh1 = np.asarray(inputs["c0_Whh"], np.float32)[d]
            b1 = (np.asarray(inputs["c0_bih"], np.float32)[d]
                  + np.asarray(inputs["c0_bhh"], np.float32)[d])
            ce = emb[char_ids[lo:hi]]  # (32, 512, 128)
            X = np.zeros((128, DK1, T, BL2), np.float32)
            X[:, 0] = ce.transpose(2, 1, 0)
            Wl1 = np.asarray(inputs["c1_Wih"], np.float32)[d]
            Whh2 = np.asarray(inputs["c1_Whh"], np.float32)[d]
            b2 = (np.asarray(inputs["c1_bih"], np.float32)[d]
                  + np.asarray(inputs["c1_bhh"], np.float32)[d])
        else:
            Wih1 = np.asarray(inputs["w0_Wih"], np.float32)[d]
            Whh1 = np.asarray(inputs["w0_Whh"], np.float32)[d]
            b1 = (np.asarray(inputs["w0_bih"], np.float32)[d]
                  + np.asarray(inputs["w0_bhh"], np.float32)[d])
            X = wemb[lo:hi].reshape(BL2, T, DK1, 128).transpose(3, 2, 1, 0)
            Wl1 = np.asarray(inputs["w1_Wih"], np.float32)[d]
            Whh2 = np.asarray(inputs["w1_Whh"], np.float32)[d]
            b2 = (np.asarray(inputs["w1_bih"], np.float32)[d]
                  + np.asarray(inputs["w1_bhh"], np.float32)[d])
        if d == 1:  # backward: reverse local time
            X = X[:, :, ::-1]
        m["x1"] = np.ascontiguousarray(X.reshape(128, DK1, T * BL2)).astype(np.float16)
        m["wih1"] = _wih_prep(Wih1, DK1)
        m["whh1"] = _wih_prep(Whh1, 2)
        m["bias1"] = np.ascontiguousarray(b1[_GPERM].reshape(8, 128).T).astype(np.float32)

        # phase-2 weights: columns [own(256) | peer(256)]
        if d == 1:
            Wl1 = Wl1[:, np.r_[256:512, 0:256]]
        m["wih2"] = _wih_prep(Wl1, DK2)
        m["whh2"] = _wih_prep(Whh2, 2)
        m["bias2"] = np.ascontiguousarray(b2[_GPERM].reshape(8, 128).T).astype(np.float32)
        # blend: f-core (d=0) picks AG region 1 (the b-core), b-core picks 0
        msk = np.zeros((128, 2), np.float32)
        msk[:, 1 - d] = 1.0
        m["masks"] = msk

        # phase-3 tags for this core's 8 sequences
        seqs3 = np.r_[4 * c: 4 * c + 4, 32 + 4 * c: 32 + 4 * c + 4]
        oh = (np.arange(K)[:, None, None] == tags[seqs3][None]).astype(np.float32)
        # (15, 8seq, 512t) -> (15, t, b)
        m["tagoneT"] = np.ascontiguousarray(
            oh.transpose(0, 2, 1).reshape(K, TB3)).astype(np.float16)
        in_maps.append(m)
    return in_maps, alpha


def kernel(**inputs):
    nc = _build_nc()
    in_maps, alpha = _make_in_maps(inputs)
    res = run_bass_kernel_spmd(nc, in_maps, core_ids=list(range(NC_N)))
    total = sum(float(res.results[c]["out"][0, 0]) for c in range(NC_N))
    total -= B * (T - 1) * np.log(alpha)
    return np.float32(total / B)


# revision 26
# speedup vs baseline: 1.1635x; 1.0026x over previous
"""BiLSTM dual-pathway + CRF NLL kernel for 8 Trainium2 NeuronCores.

Sharding: direction-parallel for the LSTM recurrences, data-parallel for the
classifier/CRF. Phase 1 runs the four layer-0 directions (char fwd/bwd, word
fwd/bwd) on 8 cores as (direction x batch-half), batch 32 per core, so the
recurrent matmuls run at N=32 instead of N=8 and the sequential chain count
drops from 4096 to 1024 steps. A pairwise AllGather exchanges the L0 hidden
states between fwd/bwd cores, phase 2 runs the four layer-1 directions the
same way, then an 8-rank AllToAll redistributes hidden states to a
data-parallel layout (8 sequences per core) for the classifier and CRF.

SPMD uniformity: every core runs the identical program. Backward directions
receive host-time-reversed inputs; reversed reads of peer hidden states are
fixed negative-stride APs, with host-permuted weight columns absorbing the
f/b role differences. Peer-region selection after the AllGather uses per-core
0/1 blend masks delivered as input data.

The CRF forward scan runs in probability space with a constant per-step
prescale alpha folded into the emission exponentials (corrected analytically
on the host), so each step is one resident-weight 15x15 matmul plus one
vector multiply -- no per-step exp/ln activation-table swaps.
"""

import sys

sys.path.insert(0, "/opt/trn_rl_repo")

import numpy as np

import concourse.bass as bass
import concourse.mybir as mybir
from concourse import bacc
from concourse.bass import ds
from concourse.tile import TileContext
from concourse.bass_utils import run_bass_kernel_spmd

F16 = mybir.dt.float16
F32 = mybir.dt.float32
AF = mybir.ActivationFunctionType

B, T, V, K = 64, 512, 40, 15
NC_N = 8
BL2 = 32            # batch per core in phases 1-2
BL3 = 8             # sequences per core in phase 3
TB3 = T * BL3       # 4096 classifier/CRF columns per core
DK1, DK2 = 6, 4     # input chunks for L0 (word=768, char padded) and L1 (512)
UNROLL = 16
HCOL = T + 1        # h buffer columns per sequence (col 0 = zero init)

_BUILD_CACHE = {}


def _emit_rec_block(nc, stagep, ps_rec, hc, cst, whh, xgs, xg_dram, tv, fodder,
                    prefetch_tv=None):
    """Emit 16 recurrence steps for time block tv.

    fodder: list of (items, per_step) - closures emitting one PE-side quantum
    each (GEMM matmuls for other pipeline stages), dispensed between the
    step's recurrent matmuls and its vector/scalar chain so they execute in
    the PE's dependency-stall gaps.
    """
    CB = UNROLL * BL2
    cursors = [[items, 0, per_step] for items, per_step in fodder]
    for j in range(UNROLL):
        psA = ps_rec.tile([128, 4, BL2], F32, tag="recpsA")
        psB = ps_rec.tile([128, 4, BL2], F32, tag="recpsB")
        # i,f gates (chunks 0-3) first into psA so the first g-add can
        # start while the o/g~ matmuls (psB) still run
        for m in (0, 1, 2, 3):
            for k in range(2):
                nc.tensor.matmul(psA[:, m], whh[:, k, m],
                                 hc[:, k, j], start=(k == 0), stop=(k == 1))
        for m in (6, 7, 4, 5):
            for k in range(2):
                nc.tensor.matmul(psB[:, m - 4], whh[:, k, m],
                                 hc[:, k, j], start=(k == 0), stop=(k == 1))
        for cur in cursors:
            for _ in range(cur[2]):
                if cur[1] < len(cur[0]):
                    cur[0][cur[1]]()
                    cur[1] += 1
        g = stagep.tile([128, 8, BL2], F16, tag="g")
        xsl = xgs[:, :, ds(j * BL2, BL2)]
        nc.vector.tensor_add(g[:, 0:4], psA, xsl[:, 0:4])
        sig = stagep.tile([128, 6, BL2], F16, tag="sig")
        nc.scalar.activation(sig[:, 0:4], g[:, 0:4], AF.Sigmoid)
        nc.vector.tensor_mul(cst, cst, sig[:, 2:4])
        nc.vector.tensor_add(g[:, 4:8], psB, xsl[:, 4:8])
        tgg = stagep.tile([128, 2, BL2], F16, tag="tgg")
        nc.scalar.activation(tgg, g[:, 6:8], AF.Tanh)
        tmp = stagep.tile([128, 2, BL2], F16, tag="tmpig")
        nc.vector.tensor_mul(tmp, sig[:, 0:2], tgg)
        nc.scalar.activation(sig[:, 4:6], g[:, 4:6], AF.Sigmoid)
        nc.vector.tensor_add(cst, cst, tmp)
        tch = stagep.tile([128, 2, BL2], F16, tag="tch")
        nc.scalar.activation(tch, cst, AF.Tanh)
        nc.vector.tensor_mul(hc[:, :, j + 1], sig[:, 4:6], tch)
    for cur in cursors:
        while cur[1] < len(cur[0]):
            cur[0][cur[1]]()
            cur[1] += 1
    if prefetch_tv is not None:
        # refill this block's (just-consumed) xgs buffer with the same-parity
        # block two ahead -- a full block of DMA lead time
        nc.sync.dma_start(out=xgs, in_=xg_dram[:, :, ds(prefetch_tv * BL2, CB)])


def _build_nc():
    if "nc" in _BUILD_CACHE:
        return _BUILD_CACHE["nc"]
    nc = bacc.Bacc(target_bir_lowering=False, num_devices=NC_N)

    # ---- external parameters -------------------------------------------------
    x1_ext = nc.declare_dram_parameter("x1", [128, DK1, T * BL2], F16, isOutput=False)
    wih1_ext = nc.declare_dram_parameter("wih1", [128, DK1 * 8 * 128], F16, isOutput=False)
    whh1_ext = nc.declare_dram_parameter("whh1", [128, 2 * 8 * 128], F16, isOutput=False)
    bias1_ext = nc.declare_dram_parameter("bias1", [128, 8], F32, isOutput=False)
    wih2_ext = nc.declare_dram_parameter("wih2", [128, DK2 * 8 * 128], F16, isOutput=False)
    whh2_ext = nc.declare_dram_parameter("whh2", [128, 2 * 8 * 128], F16, isOutput=False)
    bias2_ext = nc.declare_dram_parameter("bias2", [128, 8], F32, isOutput=False)
    masks_ext = nc.declare_dram_parameter("masks", [128, 2], F32, isOutput=False)
    cls1_ext = nc.declare_dram_parameter("cls1", [128, 8 * 4 * 128], F16, isOutput=False)
    clsb1_ext = nc.declare_dram_parameter("clsb1", [128, 4], F32, isOutput=False)
    cls2_ext = nc.declare_dram_parameter("cls2", [128, 4 * 15], F16, isOutput=False)
    clsb2_ext = nc.declare_dram_parameter("clsb2", [15, 1], F32, isOutput=False)
    mexp_ext = nc.declare_dram_parameter("mexp", [15, 15], F32, isOutput=False)
    transn_ext = nc.declare_dram_parameter("transn", [15, 15], F16, isOutput=False)
    start_ext = nc.declare_dram_parameter("crfstart", [15, 1], F32, isOutput=False)
    end_ext = nc.declare_dram_parameter("crfend", [15, 1], F32, isOutput=False)
    lna_ext = nc.declare_dram_parameter("lnalpha", [15, 1], F32, isOutput=False)
    tago_ext = nc.declare_dram_parameter("tagoneT", [15, TB3], F16, isOutput=False)
    out_ext = nc.declare_dram_parameter("out", [1, 1], F32, isOutput=True)

    # ---- internal DRAM -------------------------------------------------------
    HSZ = 2 * BL2 * T  # 32768 cols/partition of h (f16)
    xg1_dram = nc.dram_tensor("xg1", [128, 8, T * BL2], F16)
    xg2_dram = nc.dram_tensor("xg2", [128, 8, T * BL2], F16)
    HSZH = HSZ // 2  # per-partition elements of a t-half of h
    hminA = nc.dram_tensor("hminA", [128, HSZH], F16)
    hminB = nc.dram_tensor("hminB", [128, HSZH], F16)
    agoutA = nc.dram_tensor("agoutA", [2, 128, HSZH], F16)
    agoutB = nc.dram_tensor("agoutB", [2, 128, HSZH], F16)
    sendbA = nc.dram_tensor("sendbA", [8, 128, 2 * 4 * (T // 2)], F16)
    sendbB = nc.dram_tensor("sendbB", [8, 128, 2 * 4 * (T // 2)], F16)
    recvbA = nc.dram_tensor("recvbA", [8, 128, 2 * 4 * (T // 2)], F16)
    recvbB = nc.dram_tensor("recvbB", [8, 128, 2 * 4 * (T // 2)], F16)

    with TileContext(nc) as tc:
        with (
            tc.tile_pool(name="consts", bufs=1) as consts,
            tc.tile_pool(name="seqs", bufs=1) as seqs,
            tc.tile_pool(name="work", bufs=2) as work,
            tc.tile_pool(name="stage", bufs=2) as stagep,
            tc.tile_pool(name="ps_big", bufs=3, space="PSUM") as ps_big,
            tc.tile_pool(name="ps_rec", bufs=1, space="PSUM") as ps_rec,
            tc.tile_pool(name="ps_small", bufs=3, space="PSUM") as ps_small,
        ):
            # h buffer, seq-major: [128, chunk2, b32, T+1], reused by phases 1+2
            h_sb = seqs.tile([128, 2, BL2, HCOL], F16, tag="h_sb")
            hc = seqs.tile([128, 2, UNROLL + 1, BL2], F16, tag="hcomp")
            cst = seqs.tile([128, 2, BL2], F32, tag="cstate")
            masks = consts.tile([128, 2], F32, tag="masks")
            nc.sync.dma_start(out=masks, in_=masks_ext[:, :])

            # ================= PHASE 1 (L0) =================
            wih1 = consts.tile([128, DK1, 8, 128], F16, tag="wbig")
            nc.sync.dma_start(
                out=wih1,
                in_=wih1_ext.ap().rearrange("p (k m c) -> p k m c", k=DK1, m=8))
            whh1 = consts.tile([128, 2, 8, 128], F16, tag="whh")
            nc.sync.dma_start(
                out=whh1,
                in_=whh1_ext.ap().rearrange("p (k m c) -> p k m c", k=2, m=8))
            bias1 = consts.tile([128, 8], F32, tag="bias1")
            nc.sync.dma_start(out=bias1, in_=bias1_ext[:, :])
            # wih2/bias2 load now: the L1 input GEMM's slotA half runs
            # interleaved inside the phase-1 recurrence
            wih2 = consts.tile([128, DK2, 8, 128], F16, tag="wih2")
            nc.sync.dma_start(
                out=wih2,
                in_=wih2_ext.ap().rearrange("p (k m c) -> p k m c", k=DK2, m=8))
            bias2 = consts.tile([128, 8], F32, tag="bias2")
            nc.sync.dma_start(out=bias2, in_=bias2_ext[:, :])

            CB = UNROLL * BL2
            # phase-1 h layout in DRAM is t-major: (p, c, t, b), split in
            # two t-halves so the first AllGather can overlap the rec tail
            hmvA = hminA.ap().rearrange("p (c t b) -> p c t b", c=2, t=T // 2)
            hmvB = hminB.ap().rearrange("p (c t b) -> p c t b", c=2, t=T // 2)

            def g1_items(tv_lead):
                # xg1 block tv_lead: input DMA now, 48 matmul quanta
                xb = stagep.tile([128, DK1, CB], F16, tag="xb1")
                nc.sync.dma_start(out=xb, in_=x1_ext[:, :, ds(tv_lead * BL2, CB)])
                items, pss = [], {}
                for m in range(8):
                    for k in range(DK1):
                        def it(m=m, k=k):
                            if k == 0:
                                pss[m] = ps_big.tile([128, CB], F32, tag="gemmps",
                                                     name=f"g1ps{m}")
                            nc.tensor.matmul(pss[m], wih1[:, k, m], xb[:, k],
                                             start=(k == 0), stop=(k == DK1 - 1))
                            if k == DK1 - 1:
                                st = stagep.tile([128, CB], F16, tag="xgstage",
                                                 name=f"g1st{m}")
                                nc.scalar.activation(st, pss[m], AF.Identity,
                                                     bias=bias1[:, m: m + 1])
                                nc.sync.dma_start(
                                    out=xg1_dram[:, m, ds(tv_lead * BL2, CB)], in_=st)
                        items.append(it)
                return items

            def p1_end(tv, half):
                if half == 0:
                    nc.sync.dma_start(out=hmvA[:, :, ds(tv, UNROLL)],
                                      in_=hc[:, :, 1: UNROLL + 1])
                else:
                    nc.sync.dma_start(out=hmvB[:, :, ds(tv - T // 2, UNROLL)],
                                      in_=hc[:, :, 1: UNROLL + 1])
                nc.gpsimd.tensor_copy(hc[:, :, 0], hc[:, :, UNROLL])

            xgsA = stagep.tile([128, 8, CB], F16, tag="xgs", name="xgsA")
            xgsB = stagep.tile([128, 8, CB], F16, tag="xgs", name="xgsB")

            def rec1_block(tv, fodder, xgs, pf, half):
                _emit_rec_block(nc, stagep, ps_rec, hc, cst, whh1, xgs,
                                xg1_dram, tv, fodder, prefetch_tv=pf)
                p1_end(tv, half)

            nc.vector.memset(cst, 0.0)
            nc.vector.memset(hc[:, :, 0], 0.0)
            for blk in range(2):
                for it in g1_items(16 * blk):
                    it()
            nc.sync.dma_start(out=xgsA, in_=xg1_dram[:, :, ds(0, CB)])
            nc.sync.dma_start(out=xgsB, in_=xg1_dram[:, :, ds(16 * BL2, CB)])
            rec1_block(0, [(g1_items(32), 3)], xgsA, 32, 0)
            rec1_block(16, [(g1_items(48), 3)], xgsB, 48, 0)

            def p1_loop(tv, half):
                itemsA = g1_items(tv + 32)
                itemsB = g1_items(tv + 48)  # DMA issued early: a block of lead
                _emit_rec_block(nc, stagep, ps_rec, hc, cst, whh1, xgsA,
                                xg1_dram, tv, [(itemsA, 3)], prefetch_tv=tv + 32)
                p1_end(tv, half)
                _emit_rec_block(nc, stagep, ps_rec, hc, cst, whh1, xgsB,
                                xg1_dram, tv + 16, [(itemsB, 3)],
                                prefetch_tv=tv + 48)
                p1_end(tv + 16, half)

            with tc.For_i(32, 256, 32) as tv:
                p1_loop(tv, 0)
            # first-half h is complete: exchange it while the rec tail runs
            nc.gpsimd.collective_compute(
                "AllGather", mybir.AluOpType.bypass,
                replica_groups=[[0, 2], [1, 3], [4, 6], [5, 7]],
                ins=[hminA.ap()], outs=[agoutA.ap()],
            )
            with tc.For_i(256, 480, 32) as tv:
                p1_loop(tv, 1)
            rec1_block(480, [], xgsA, None, 1)
            rec1_block(496, [], xgsB, None, 1)
            nc.gpsimd.collective_compute(
                "AllGather", mybir.AluOpType.bypass,
                replica_groups=[[0, 2], [1, 3], [4, 6], [5, 7]],
                ins=[hminB.ap()], outs=[agoutB.ap()],
            )

            # ================= PHASE 2 (L1) =================
            whh2 = consts.tile([128, 2, 8, 128], F16, tag="whh")
            nc.sync.dma_start(
                out=whh2,
                in_=whh2_ext.ap().rearrange("p (k m c) -> p k m c", k=2, m=8))

            def slot_ap(tensor, region, c, w_local):
                # [128, UNROLL*32] chunk c of a t-major half-h DRAM region
                # (p, c2, T/2, b32): contiguous ascending t window (w_local
                # in half-local units); reversed slots flip t SBUF-side.
                off = (region * 128 * HSZH + c * ((T // 2) * BL2)
                       + w_local * BL2)
                return bass.AP(tensor=tensor, offset=off,
                               ap=[[HSZH, 128], [1, UNROLL * BL2]])

            def g2_items(tv_lead, sa_t, sa_w, ag_t, rev_w):
                # full xg2 block: slotA = own h fwd, slotB = peer h reversed
                sa = stagep.tile([128, 2, UNROLL, BL2], F16, tag="slotA")
                r0 = stagep.tile([128, 2, UNROLL, BL2], F16, tag="slotR0")
                r1 = stagep.tile([128, 2, UNROLL, BL2], F16, tag="slotR1")
                for c in range(2):
                    nc.sync.dma_start(out=sa[:, c],
                                      in_=slot_ap(sa_t, 0, c, sa_w))
                    nc.sync.dma_start(out=r0[:, c],
                                      in_=slot_ap(ag_t, 0, c, rev_w))
                    nc.sync.dma_start(out=r1[:, c],
                                      in_=slot_ap(ag_t, 1, c, rev_w))
                sbr = stagep.tile([128, 2, UNROLL, BL2], F16, tag="slotBr")
                items, pss = [], {}

                def blend0():
                    nc.vector.tensor_scalar_mul(r0, r0, masks[:, 0:1])

                def blend1():
                    nc.vector.tensor_scalar_mul(r1, r1, masks[:, 1:2])

                def blend2():
                    # add, writing with the t window reversed
                    p_step = sbr.ap[0][0]
                    rev = bass.AP(tensor=sbr.tensor,
                                  offset=sbr.offset + (UNROLL - 1) * BL2,
                                  ap=[[p_step, 128], [UNROLL * BL2, 2],
                                      [-BL2, UNROLL], [1, BL2]])
                    nc.vector.tensor_add(rev, r0, r1)

                items += [blend0, blend1, blend2]

                def src_k(k):
                    t = sa if k < 2 else sbr
                    return t[:, k % 2].rearrange("p t b -> p (t b)")

                for m in range(8):
                    for k in range(DK2):
                        def it(m=m, k=k):
                            if k == 0:
                                pss[m] = ps_big.tile([128, CB], F32, tag="gemmps",
                                                     name=f"g2ps{m}")
                            nc.tensor.matmul(pss[m], wih2[:, k, m], src_k(k),
                                             start=(k == 0), stop=(k == DK2 - 1))
                            if k == DK2 - 1:
                                st = stagep.tile([128, CB], F16, tag="xgstage",
                                                 name=f"g2st{m}")
                                nc.scalar.activation(st, pss[m], AF.Identity,
                                                     bias=bias2[:, m: m + 1])
                                nc.sync.dma_start(
                                    out=xg2_dram[:, m, ds(tv_lead * BL2, CB)], in_=st)
                        items.append(it)
                return items

            def p2_end(tv):
                nc.gpsimd.tensor_copy(
                    h_sb[:, :, :, ds(tv + 1, UNROLL)],
                    hc[:, :, 1: UNROLL + 1].rearrange("p c t b -> p c b t"))
                nc.gpsimd.tensor_copy(hc[:, :, 0], hc[:, :, UNROLL])

            nc.vector.memset(cst, 0.0)
            nc.vector.memset(hc[:, :, 0], 0.0)
            HT = T // 2
            # leads 0,1: own h from half A, peer (reversed) from half B
            for blk in (0, 1):
                L = 16 * blk
                for it in g2_items(L, hminA, L, agoutB, (T - UNROLL) - L - HT):
                    it()
            nc.sync.dma_start(out=xgsA, in_=xg2_dram[:, :, ds(0, CB)])
            nc.sync.dma_start(out=xgsB, in_=xg2_dram[:, :, ds(16 * BL2, CB)])

            def p2_loop(tv, sa_t, sa_base, ag_t, rev_base):
                # halves at leads tv+32, tv+48; window args are half-local
                itemsA = g2_items(tv + 32, sa_t, sa_base + tv, ag_t,
                                  rev_base - tv)
                itemsB = g2_items(tv + 48, sa_t, sa_base + tv + 16, ag_t,
                                  rev_base - tv - 16)
                _emit_rec_block(nc, stagep, ps_rec, hc, cst, whh2, xgsA,
                                xg2_dram, tv, [(itemsA, 3)], prefetch_tv=tv + 32)
                p2_end(tv)
                _emit_rec_block(nc, stagep, ps_rec, hc, cst, whh2, xgsB,
                                xg2_dram, tv + 16, [(itemsB, 3)],
                                prefetch_tv=tv + 48)
                p2_end(tv + 16)

            with tc.For_i(0, 224, 32) as tv:
                # leads 2..15: own h half A, peer half B
                p2_loop(tv, hminA, 32, agoutB, (T - UNROLL) - 32 - HT)
            with tc.For_i(224, 256, 32) as tv:
                # blocks 14,15: own h half B, peer half A
                p2_loop(tv, hminB, 32 - HT, agoutA, (T - UNROLL) - 32)
            # first t-half of phase-2 h is complete: redistribute it while
            # the rec tail runs
            sbA_view = sendbA.ap().rearrange("j p (c b t) -> j p c b t",
                                             c=2, b=4)
            for j in range(8):
                for c in range(2):
                    nc.sync.dma_start(out=sbA_view[j, :, c],
                                      in_=h_sb[:, c, ds(4 * j, 4), 1: 1 + HT])
            nc.gpsimd.collective_compute(
                "AllToAll", mybir.AluOpType.bypass,
                replica_groups=[list(range(8))],
                ins=[sendbA.ap()], outs=[recvbA.ap()],
            )
            with tc.For_i(256, 480, 32) as tv:
                p2_loop(tv, hminB, 32 - HT, agoutA, (T - UNROLL) - 32)
            _emit_rec_block(nc, stagep, ps_rec, hc, cst, whh2, xgsA,
                            xg2_dram, 480, [])
            p2_end(480)
            _emit_rec_block(nc, stagep, ps_rec, hc, cst, whh2, xgsB,
                            xg2_dram, 496, [])
            p2_end(496)

            sbB_view = sendbB.ap().rearrange("j p (c b t) -> j p c b t",
                                             c=2, b=4)
            for j in range(8):
                for c in range(2):
                    nc.sync.dma_start(out=sbB_view[j, :, c],
                                      in_=h_sb[:, c, ds(4 * j, 4),
                                               1 + HT: HCOL])
            nc.gpsimd.collective_compute(
                "AllToAll", mybir.AluOpType.bypass,
                replica_groups=[list(range(8))],
                ins=[sendbB.ap()], outs=[recvbB.ap()],
            )

            # ================= PHASE 3: classifier + CRF =================
            cls1 = consts.tile([128, 8, 4, 128], F16, tag="wbig")
            nc.sync.dma_start(
                out=cls1,
                in_=cls1_ext.ap().rearrange("p (k m c) -> p k m c", k=8, m=4))
            clsb1 = consts.tile([128, 4], F32, tag="clsb1")
            nc.sync.dma_start(out=clsb1, in_=clsb1_ext[:, :])
            cls2 = consts.tile([128, 4, 15], F16, tag="cls2")
            nc.sync.dma_start(
                out=cls2, in_=cls2_ext.ap().rearrange("p (k j) -> p k j", k=4))
            clsb2 = consts.tile([15, 1], F32, tag="clsb2")
            nc.sync.dma_start(out=clsb2, in_=clsb2_ext[:, :])
            mexp = consts.tile([15, 15], F32, tag="mexp")
            nc.sync.dma_start(out=mexp, in_=mexp_ext[:, :])
            transn = consts.tile([15, 15], F16, tag="transn")
            nc.sync.dma_start(out=transn, in_=transn_ext[:, :])
            crfstart = consts.tile([15, 1], F32, tag="crfstart")
            nc.sync.dma_start(out=crfstart, in_=start_ext[:, :])
            crfend = consts.tile([15, 1], F32, tag="crfend")
            nc.sync.dma_start(out=crfend, in_=end_ext[:, :])
            lnalpha = consts.tile([15, 1], F32, tag="lnalpha")
            nc.sync.dma_start(out=lnalpha, in_=lna_ext[:, :])
            tago = consts.tile([15, TB3], F16, tag="tago")
            nc.sync.dma_start(out=tago, in_=tago_ext[:, :])

            logits = seqs.tile([15, TB3], F32, tag="logits")

            NT = 64  # t-steps per classifier n-tile (NT*BL3 = 512 cols)
            HT3 = T // 2
            SHSZH = 128 * 2 * 4 * HT3  # elements per half recv shard

            def comb_ap(kk, bh, ns):
                # [128, 4, NT]: dir kk//2, chunk kk%2, batch-half bh, n-tile
                # ns; ascending t window (reversal done in the rhs view),
                # routed to the t-half recv tensor containing the window
                d, c = kk // 2, kk % 2
                rev = d in (1, 3)  # c1b, w1b stored time-reversed
                w = (ns * NT) if not rev else ((T - NT) - ns * NT)
                tensor = recvbA if w < HT3 else recvbB
                off = ((2 * d + bh) * SHSZH + c * (4 * HT3)
                       + (w if w < HT3 else w - HT3))
                return bass.AP(tensor=tensor, offset=off,
                               ap=[[2 * 4 * HT3, 128], [HT3, 4], [1, NT]])

            for ns in range(8):
                comb = stagep.tile([128, 8, BL3, NT], F16, tag="comb", bufs=3)
                for kk in range(8):
                    for half in range(2):
                        nc.sync.dma_start(out=comb[:, kk, ds(4 * half, 4)],
                                          in_=comb_ap(kk, half, ns))
                hmt = []
                for m in range(4):
                    ps = ps_big.tile([128, NT * BL3], F32, tag="gemmps")
                    for kk in range(8):
                        if kk // 2 in (1, 3):
                            p_step = comb.ap[0][0]
                            off = (comb.offset + kk * BL3 * NT + (NT - 1))
                            rhs = bass.AP(tensor=comb.tensor, offset=off,
                                          ap=[[p_step, 128], [-1, NT],
                                              [NT, BL3]])
                        else:
                            rhs = comb[:, kk].rearrange("p b t -> p t b")
                        nc.tensor.matmul(ps, cls1[:, kk, m], rhs,
                                         start=(kk == 0), stop=(kk == 7))
                    hm = stagep.tile([128, NT * BL3], F16, tag="hm", bufs=4,
                                     name=f"hm{m}")
                    nc.scalar.activation(hm, ps, AF.Relu, bias=clsb1[:, m: m + 1])
                    hmt.append(hm)
                ps2 = ps_small.tile([15, NT * BL3], F32, tag="small")
                for m in range(4):
                    nc.tensor.matmul(ps2, cls2[:, m], hmt[m],
                                     start=(m == 0), stop=(m == 3))
                nc.vector.tensor_scalar_add(
                    logits[:, ds(ns * NT * BL3, NT * BL3)], ps2, clsb2)

            # fold CRF start/end into first/last emission columns
            nc.vector.tensor_scalar_add(logits[:, 0:BL3], logits[:, 0:BL3], crfstart)
            nc.vector.tensor_scalar_add(logits[:, TB3 - BL3: TB3],
                                        logits[:, TB3 - BL3: TB3], crfend)

            # ---- CRF numerator ----
            racc = work.tile([15, 16], F32, tag="racc")
            nc.vector.memset(racc, 0.0)
            for ns in range(8):
                pre = stagep.tile([15, 512], F32, tag="prodns")
                nc.vector.tensor_mul(pre, logits[:, ds(ns * 512, 512)],
                                     tago[:, ds(ns * 512, 512)])
                nc.vector.tensor_reduce(racc[:, 8 + ns: 9 + ns], pre,
                                        axis=mybir.AxisListType.X,
                                        op=mybir.AluOpType.add)
                psv = ps_small.tile([15, 512], F32, tag="small")
                nc.tensor.matmul(psv, transn, tago[:, ds(ns * 512, 512)],
                                 start=True, stop=True)
                w = 512 if ns < 7 else 512 - BL3
                pr = stagep.tile([15, 512], F32, tag="prodns")
                nc.vector.tensor_mul(pr[:, :w], psv[:, :w],
                                     tago[:, ds(ns * 512 + BL3, w)])
                nc.vector.tensor_reduce(racc[:, ns: ns + 1], pr[:, :w],
                                        axis=mybir.AxisListType.X,
                                        op=mybir.AluOpType.add)
            nv = stagep.tile([15, 1], F32, tag="nv")
            nc.vector.tensor_reduce(nv, racc, axis=mybir.AxisListType.X,
                                    op=mybir.AluOpType.add)
            ones15 = consts.tile([15, 1], F32, tag="ones15")
            nc.vector.memset(ones15, 1.0)
            psn = ps_small.tile([1, 1], F32, tag="small")
            nc.tensor.matmul(psn, ones15, nv, start=True, stop=True)
            num11 = work.tile([1, 1], F32, tag="num11")
            nc.vector.tensor_copy(num11, psn)

            # ---- CRF forward scan, probability space ----
            # two independent half-batch chains so the 15x15 matmul of one
            # overlaps the vector multiply of the other. p0 first, then
            # E = alpha*exp(logits) in place (last col already has e^end).
            pA = seqs.tile([15, 4], F32, tag="pvecA")
            pB = seqs.tile([15, 4], F32, tag="pvecB")
            nc.scalar.activation(pA, logits[:, 0:4], AF.Exp)
            nc.scalar.activation(pB, logits[:, 4:8], AF.Exp)
            E = logits
            nc.scalar.activation(E, logits, AF.Exp, bias=lnalpha)

            def crf_step(col):
                zA = ps_small.tile([15, 4], F32, tag="small", name="zA")
                nc.tensor.matmul(zA, mexp, pA, start=True, stop=True)
                zB = ps_small.tile([15, 4], F32, tag="small", name="zB")
                nc.tensor.matmul(zB, mexp, pB, start=True, stop=True)
                nc.vector.tensor_mul(pA, zA, E[:, col: col + 4] if isinstance(col, int)
                                     else E[:, ds(col, 4)])
                nc.vector.tensor_mul(pB, zB, E[:, col + 4: col + 8] if isinstance(col, int)
                                     else E[:, ds(col + 4, 4)])

            for t in range(1, 16):
                crf_step(t * BL3)
            with tc.For_i(0, 496, UNROLL) as tv:
                for j in range(UNROLL):
                    crf_step((16 + j) * BL3 + tv * BL3)

            # ---- denominator + output ----
            den11 = work.tile([1, 1], F32, tag="den11")
            for idx, pv in enumerate((pA, pB)):
                psd = ps_small.tile([1, 4], F32, tag="small", name=f"psd{idx}")
                nc.tensor.matmul(psd, ones15, pv, start=True, stop=True)
                ln4 = stagep.tile([1, 4], F32, tag="ln8", name=f"ln4{idx}")
                nc.scalar.activation(ln4, psd, AF.Ln)
                if idx == 0:
                    nc.vector.tensor_reduce(den11, ln4, axis=mybir.AxisListType.X,
                                            op=mybir.AluOpType.add)
                else:
                    dh = work.tile([1, 1], F32, tag="denh")
                    nc.vector.tensor_reduce(dh, ln4, axis=mybir.AxisListType.X,
                                            op=mybir.AluOpType.add)
                    nc.vector.tensor_add(den11, den11, dh)
            res = work.tile([1, 1], F32, tag="res")
            nc.vector.tensor_sub(res, den11, num11)
            nc.sync.dma_start(out=out_ext[:, :], in_=res)

    nc.finalize()
    _BUILD_CACHE["nc"] = nc
    return nc


# ---- host-side input prep ---------------------------------------------------

# gate perm [i(256), f(256), g(256), o(256)] -> [i, f, o, g~]
_GPERM = np.concatenate([np.arange(0, 512), np.arange(768, 1024), np.arange(512, 768)])

# core c -> (pathway, direction, half): 0..3 char f/f/b/b, 4..7 word
_ROLES = [("c", 0, 0), ("c", 0, 1), ("c", 1, 0), ("c", 1, 1),
          ("w", 0, 0), ("w", 0, 1), ("w", 1, 0), ("w", 1, 1)]


def _wih_prep(W, dk_n):
    Wp = W[_GPERM]
    return np.ascontiguousarray(
        Wp.reshape(8, 128, dk_n, 128).transpose(3, 2, 0, 1).reshape(128, dk_n * 8 * 128)
    ).astype(np.float16)


def _make_in_maps(inputs):
    char_ids = np.asarray(inputs["char_ids"])
    tags = np.asarray(inputs["tags"])
    wemb = np.asarray(inputs["word_embeddings"], np.float32)
    emb = np.asarray(inputs["char_emb_table"], np.float32)
    trans = np.asarray(inputs["crf_trans"], np.float32)

    alpha = 1.0 / (15.0 * float(np.exp(trans).mean()))
    common = {}
    w1 = np.asarray(inputs["cls_w1"], np.float32)
    common["cls1"] = np.ascontiguousarray(
        w1.reshape(4, 128, 8, 128).transpose(3, 2, 0, 1).reshape(128, 8 * 4 * 128)
    ).astype(np.float16)
    common["clsb1"] = np.ascontiguousarray(
        np.asarray(inputs["cls_b1"], np.float32).reshape(4, 128).T).astype(np.float32)
    w2 = np.asarray(inputs["cls_w2"], np.float32)
    common["cls2"] = np.ascontiguousarray(
        w2.reshape(15, 4, 128).transpose(2, 1, 0).reshape(128, 4 * 15)).astype(np.float16)
    common["clsb2"] = np.asarray(inputs["cls_b2"], np.float32).reshape(15, 1).copy()
    common["mexp"] = np.exp(trans).astype(np.float32)
    common["transn"] = trans.astype(np.float16)
    common["crfstart"] = np.asarray(inputs["crf_start"], np.float32).reshape(15, 1).copy()
    common["crfend"] = np.asarray(inputs["crf_end"], np.float32).reshape(15, 1).copy()
    common["lnalpha"] = np.full((15, 1), np.log(alpha), np.float32)

    in_maps = []
    for c in range(NC_N):
        pw, d, hf = _ROLES[c]
        lo, hi = hf * BL2, (hf + 1) * BL2
        m = dict(common)

        # phase-1 weights/input
        if pw == "c":
            Wih1 = np.zeros((1024, 768), np.float32)
            Wih1[:, :128] = np.asarray(inputs["c0_Wih"], np.float32)[d]
            Whh1 = np.asarray(inputs["c0_Whh"], np.float32)[d]
            b1 = (np.asarray(inputs["c0_bih"], np.float32)[d]
                  + np.asarray(inputs["c0_bhh"], np.float32)[d])
            ce = emb[char_ids[lo:hi]]  # (32, 512, 128)
            X = np.zeros((128, DK1, T, BL2), np.float32)
            X[:, 0] = ce.transpose(2, 1, 0)
            Wl1 = np.asarray(inputs["c1_Wih"], np.float32)[d]
            Whh2 = np.asarray(inputs["c1_Whh"], np.float32)[d]
            b2 = (np.asarray(inputs["c1_bih"], np.float32)[d]
                  + np.asarray(inputs["c1_bhh"], np.float32)[d])
        else:
            Wih1 = np.asarray(inputs["w0_Wih"], np.float32)[d]
            Whh1 = np.asarray(inputs["w0_Whh"], np.float32)[d]
            b1 = (np.asarray(inputs["w0_bih"], np.float32)[d]
                  + np.asarray(inputs["w0_bhh"], np.float32)[d])
            X = wemb[lo:hi].reshape(BL2, T, DK1, 128).transpose(3, 2, 1, 0)
            Wl1 = np.asarray(inputs["w1_Wih"], np.float32)[d]
            Whh2 = np.asarray(inputs["w1_Whh"], np.float32)[d]
            b2 = (np.asarray(inputs["w1_bih"], np.float32)[d]
                  + np.asarray(inputs["w1_bhh"], np.float32)[d])
        if d == 1:  # backward: reverse local time
            X = X[:, :, ::-1]
        m["x1"] = np.ascontiguousarray(X.reshape(128, DK1, T * BL2)).astype(np.float16)
        m["wih1"] = _wih_prep(Wih1, DK1)
        m["whh1"] = _wih_prep(Whh1, 2)
        m["bias1"] = np.ascontiguousarray(b1[_GPERM].reshape(8, 128).T).astype(np.float32)

        # phase-2 weights: columns [own(256) | peer(256)]
        if d == 1:
            Wl1 = Wl1[:, np.r_[256:512, 0:256]]
        m["wih2"] = _wih_prep(Wl1, DK2)
        m["whh2"] = _wih_prep(Whh2, 2)
        m["bias2"] = np.ascontiguousarray(b2[_GPERM].reshape(8, 128).T).astype(np.float32)
        # blend: f-core (d=0) picks AG region 1 (the b-core), b-core picks 0
        msk = np.zeros((128, 2), np.float32)
        msk[:, 1 - d] = 1.0
        m["masks"] = msk

        # phase-3 tags for this core's 8 sequences
        seqs3 = np.r_[4 * c: 4 * c + 4, 32 + 4 * c: 32 + 4 * c + 4]
        oh = (np.arange(K)[:, None, None] == tags[seqs3][None]).astype(np.float32)
        # (15, 8seq, 512t) -> (15, t, b)
        m["tagoneT"] = np.ascontiguousarray(
            oh.transpose(0, 2, 1).reshape(K, TB3)).astype(np.float16)
        in_maps.append(m)
    return in_maps, alpha


def kernel(**inputs):
    nc = _build_nc()
    in_maps, alpha = _make_in_maps(inputs)
    res = run_bass_kernel_spmd(nc, in_maps, core_ids=list(range(NC_N)))
    total = sum(float(res.results[c]["out"][0, 0]) for c in range(NC_N))
    total -= B * (T - 1) * np.log(alpha)
    return np.float32(total / B)
